# revision 1
# baseline (speedup 1.0000x reference)
"""Trainium2 Bass kernel for nn_ConformerBlock_50525995270849.

Takes FULL unsharded inputs (as produced by setup_inputs()) and returns the
FULL [B, D, T] fp32 output, running on 8 NeuronCores via run_bass_kernel_spmd.

Sharding: core c = (batch b=c//2, T-half parity p=c%2). Each core computes
FFN1+QKV over full T (K/V need all positions), attention for its extended
640-column local query frame (global cols [384p, 384p+640)), and the conv
module + FFN2 for its own 512 columns (local frame cols [128p, 128p+512)).
GroupNorm(1,C) statistics are globally reduced with the pair core via two
tiny AllReduce collectives. The gated relative-position bias is added to the
score PSUM with per-t-block diag(f) matmuls against diagonal table
expansions (negative-free-step DMA from per-core shifted tables).

Algebra validated against the reference in numpy (rel err 3.2e-3, bf16).
"""
import sys
import os

sys.path.insert(0, "/opt/trn_rl_repo")

import numpy as np
import ml_dtypes

B, T, D, H, DH = 4, 1024, 512, 8, 64
FF = 4 * D
KW = 31
NB, MAXD = 320, 800
PAD = KW // 2
NCORES = 8
P = 128
FRAME = 640          # local query frame width
OWN = 512            # own region width
TABW = 1664          # per-core bias table width

bfnp = ml_dtypes.bfloat16
_CACHE = {}


def bucket1d():
    half, thr = NB // 2, NB // 4
    r = np.arange(-(T - 1), T, dtype=np.int32)
    sign = (r >= 0).astype(np.int32)
    ap = np.abs(r)
    log_ratio = np.log(np.maximum(ap, 1).astype(np.float32) / thr) / np.float32(
        np.log(MAXD / thr))
    log_pos = np.minimum(np.rint(thr + log_ratio * (half - thr)).astype(np.int32),
                         half - 1)
    return np.clip(np.where(ap < thr, ap, log_pos) + sign * half, 0, NB - 1)


# column layout of the packed per-partition bias tensor cbias [128, NCB]
_CB = {}
_ncb = 0
for _name, _n in [("b1a", 16), ("b2a", 4), ("bq", 4), ("bk", 4), ("bo", 4),
                  ("Wg", 8), ("Wb", 8), ("dwb", 4), ("g2", 4), ("b2g", 4),
                  ("bpw2", 4), ("b1b", 16), ("b2b", 4), ("sh", 8), ("dw", 31 * 4)]:
    _CB[_name] = _ncb
    _ncb += _n
NCB = _ncb


def _build_program():
    import concourse.bass as bass
    import concourse.tile as tile
    from concourse import bacc, mybir
    from contextlib import ExitStack

    f32 = mybir.dt.float32
    bf16 = mybir.dt.bfloat16
    AF = mybir.ActivationFunctionType
    OP = mybir.AluOpType
    AX = mybir.AxisListType

    nc = bacc.Bacc("TRN2", target_bir_lowering=False, debug=False,
                   num_devices=NCORES)

    di = lambda n, s, dt: nc.dram_tensor(n, s, dt, kind="ExternalInput")
    xb_d = di("xb", [D, T], bf16)
    w1a_d = di("w1a", [D, FF], bf16)
    w2a_d = di("w2a", [FF, D], bf16)
    wqkvo_d = di("wqkvo", [D, 4 * D], bf16)   # [wq | wk | wv | wo]
    pw1g_d = di("pw1g", [D, 2 * D], bf16)
    pw2t_d = di("pw2t", [D, D], bf16)
    w1b_d = di("w1b", [D, FF], bf16)
    w2b_d = di("w2b", [FF, D], bf16)
    gg_d = di("gg", [D, 16], bf16)
    i128_d = di("i128", [P, P], bf16)
    ones1_d = di("ones1", [1, P], bf16)
    bvrow_d = di("bvrow", [1, D], bf16)
    cbias_d = di("cbias", [P, NCB], f32)
    dexp_d = di("dexp", [H, 5, P, 1024], bf16)
    toff_d = di("toff", [1, 1], mybir.dt.uint32)
    own0_d = di("own0", [1, 1], mybir.dt.uint32)
    y_d = nc.dram_tensor("y", [D, OWN], f32, kind="ExternalOutput").ap()

    with tile.TileContext(nc) as tc, ExitStack() as ctx:
        pool = lambda name, bufs, **kw: ctx.enter_context(
            tc.tile_pool(name=name, bufs=bufs, **kw))
        cp = pool("const", 1)
        big = pool("big", 1)
        wpA = pool("wpA", 4)        # [128, 2048] bf16 weight row-tiles
        wpB = pool("wpB", 17)       # [128, 512] bf16 weight row-tiles
        hp = pool("hp", 17)         # FFN hidden tiles [128, 512] bf16
        qrot = pool("qrot", 2)      # q full-T tiles (consumed by qloc copy)
        Pp = pool("Pp", 9)          # attention probs tiles [128, 640] bf16
        dgp = pool("dgp", 6)        # diag tiles [128, 128] bf16
        dbp = pool("dbp", 7)        # bias table tiles [128, 1024] bf16
        scp = pool("scp", 2)        # small scratch
        drp = pool("drp", 1, space="DRAM")
        psA = pool("psA", 2, space="PSUM")
        psB = pool("psB", 3, space="PSUM")

        # ---- registers (per-engine) for the two dynamic offsets ----
        tr_v = nc.vector.alloc_register("toff_v")
        nc.vector.reg_load(tr_v, toff_d[0:1, 0:1])
        toff_v = nc.vector.snap(tr_v, donate=True, min_val=0, max_val=384)
        ow_v = nc.vector.alloc_register("own0_v")
        nc.vector.reg_load(ow_v, own0_d[0:1, 0:1])
        own0_v = nc.vector.snap(ow_v, donate=True, min_val=0, max_val=128)
        ow_s = nc.scalar.alloc_register("own0_s")
        nc.scalar.reg_load(ow_s, own0_d[0:1, 0:1])
        own0_s = nc.scalar.snap(ow_s, donate=True, min_val=0, max_val=128)

        # ---- constants ----
        cbias = cp.tile([P, NCB], f32)
        nc.sync.dma_start(cbias[:], cbias_d.ap())
        i128 = cp.tile([P, P], bf16)
        nc.sync.dma_start(i128[:], i128_d.ap())
        ones1 = cp.tile([1, P], bf16)
        nc.sync.dma_start(ones1[:], ones1_d.ap())
        bvrow = cp.tile([1, D], bf16)
        nc.sync.dma_start(bvrow[:], bvrow_d.ap())
        gg_sb = [cp.tile([P, 16], bf16, tag=f"gg{k}", name=f"gg{k}")
                 for k in range(4)]
        for k in range(4):
            nc.sync.dma_start(gg_sb[k][:], gg_d.ap()[k * P:(k + 1) * P, :])

        def cb(name, i):
            return cbias[:, _CB[name] + i:_CB[name] + i + 1]

        # ---- persistent activation buffers ----
        bt = lambda tg, w, dt=bf16: big.tile([P, w], dt, tag=tg, name=tg)
        xb = [bt(f"xb{k}", T) for k in range(4)]
        for k in range(4):
            nc.sync.dma_start(xb[k][:], xb_d.ap()[k * P:(k + 1) * P, :])
        s1b = [bt(f"s1b{k}", T) for k in range(4)]
        kkb = [bt(f"kkb{k}", T) for k in range(4)]
        vT = [bt(f"vT{t}", 520) for t in range(8)]
        qloc = [bt(f"qloc{k}", FRAME) for k in range(4)]
        fsb = [bt(f"fsb{t}", 8, f32) for t in range(5)]
        ohat = [bt(f"ohat{k}", FRAME) for k in range(4)]
        s2f = [bt(f"s2f{k}", FRAME, f32) for k in range(4)]
        s2b = [bt(f"s2b{k}", FRAME) for k in range(4)]
        a_sb = [bt(f"a{m}", FRAME) for m in range(4)]
        sg_sb = [bt(f"sg{m}", FRAME) for m in range(4)]
        glu = [bt(f"glu{m}", FRAME + 2 * PAD) for m in range(4)]
        dcb = [bt(f"dcb{m}", FRAME) for m in range(4)]
        slown = [bt(f"slown{m}", OWN) for m in range(4)]
        c2f = [bt(f"c2f{m}", OWN, f32) for m in range(4)]
        c2b = [bt(f"c2b{m}", OWN) for m in range(4)]
        stats = bt("stats", 8, f32)
        stats2 = bt("stats2", 2, f32)
        statsB = bt("statsB", 8, f32)
        stats2B = bt("stats2B", 2, f32)
        gla = bt("gla", 2, f32)     # gn1 allreduced [sum, sq]
        glb = bt("glb", 2, f32)     # gn2 allreduced
        r1b = bt("r1b", 2, f32)     # [r1, -m1*r1]
        r2b = bt("r2b", 2, f32)     # [r2, -m2]
        biasg = bt("biasg", 8, f32)
        sact = bt("sact", 4, f32)
        bact = bt("bact", 4, f32)
        scr2 = bt("scr2", 8, f32)   # scalar scratch columns

        # =========== Phase 1: FFN1 over full T ===========
        w1a_t = []
        for k in range(4):
            w = wpA.tile([P, FF], bf16, tag="wA", name=f"w1a{k}")
            nc.sync.dma_start(w[:], w1a_d.ap()[k * P:(k + 1) * P, :])
            w1a_t.append(w)
        w2a_t = []
        for k in range(16):
            w = wpB.tile([P, D], bf16, tag="wB", name=f"w2a{k}")
            nc.sync.dma_start(w[:], w2a_d.ap()[k * P:(k + 1) * P, :])
            w2a_t.append(w)
        for tch in range(2):
            c0 = tch * 512
            hs = []
            for m in range(16):
                ps = psA.tile([P, 512], f32, tag="psA", name=f"ps1_{tch}_{m}")
                for k in range(4):
                    nc.tensor.matmul(ps[:], w1a_t[k][:, m * P:(m + 1) * P],
                                     xb[k][:, c0:c0 + 512],
                                     start=(k == 0), stop=(k == 3))
                ht = hp.tile([P, 512], bf16, tag="h", name=f"h_{tch}_{m}")
                nc.scalar.activation(ht[:], ps[:], AF.Gelu, bias=cb("b1a", m))
                hs.append(ht)
            for m in range(4):
                ps = psA.tile([P, 512], f32, tag="psA", name=f"ps2_{tch}_{m}")
                for k in range(16):
                    nc.tensor.matmul(ps[:], w2a_t[k][:, m * P:(m + 1) * P],
                                     hs[k][:], start=(k == 0), stop=(k == 15))
                nc.vector.scalar_tensor_tensor(
                    s1b[m][:, c0:c0 + 512], ps[:], cb("b2a", m),
                    xb[m][:, c0:c0 + 512], OP.add, OP.add)

        # =========== Phase 2: QKV ===========
        wq_t, wk_t, wv_t, wo_t = [], [], [], []
        for k in range(4):
            w = wpA.tile([P, 4 * D], bf16, tag="wA", name=f"wqkvo{k}")
            nc.sync.dma_start(w[:], wqkvo_d.ap()[k * P:(k + 1) * P, :])
            wq_t.append(w[:, 0:D])
            wk_t.append(w[:, D:2 * D])
            wv_t.append(w[:, 2 * D:3 * D])
            wo_t.append(w[:, 3 * D:4 * D])
        # q (full T, rotating) -> qloc (static local frame)
        qfull = []
        for m in range(4):
            qf = qrot.tile([P, T], bf16, tag="qf", name=f"qf{m}")
            for tch in range(2):
                c0 = tch * 512
                ps = psA.tile([P, 512], f32, tag="psA", name=f"psq{m}{tch}")
                for k in range(4):
                    nc.tensor.matmul(ps[:], wq_t[k][:, m * P:(m + 1) * P],
                                     s1b[k][:, c0:c0 + 512],
                                     start=(k == 0), stop=(k == 3))
                nc.scalar.activation(qf[:, c0:c0 + 512], ps[:], AF.Identity,
                                     bias=cb("bq", m))
            nc.vector.tensor_copy(qloc[m][:],
                                  qf[:, bass.ds(toff_v, FRAME)])
            qfull.append(qf)
        # k (full T, persistent)
        for m in range(4):
            for tch in range(2):
                c0 = tch * 512
                ps = psA.tile([P, 512], f32, tag="psA", name=f"psk{m}{tch}")
                for k in range(4):
                    nc.tensor.matmul(ps[:], wk_t[k][:, m * P:(m + 1) * P],
                                     s1b[k][:, c0:c0 + 512],
                                     start=(k == 0), stop=(k == 3))
                nc.scalar.activation(kkb[m][:, c0:c0 + 512], ps[:], AF.Identity,
                                     bias=cb("bk", m))
        # vT: [t, dv] with ones columns (65-block layout)
        for t in range(8):
            ps = psA.tile([P, 512], f32, tag="psA", name=f"psv{t}")
            for k in range(4):
                nc.tensor.matmul(ps[:], s1b[k][:, t * P:(t + 1) * P],
                                 wv_t[k][:], start=(k == 0), stop=False)
            nc.tensor.matmul(ps[:], ones1[:, 0:P], bvrow[:],
                             start=False, stop=True)
            src3 = ps[:].rearrange("p (h c) -> p h c", c=64)
            dst3 = vT[t][:, 0:520].rearrange("p (h c) -> p h c", c=65)[:, :, 0:64]
            nc.scalar.activation(dst3, src3, AF.Copy)
            onescol = vT[t][:, 0:520].rearrange("p (h c) -> p h c", c=65)[:, :, 64:65]
            nc.gpsimd.memset(onescol, 1.0)

        # =========== Phase 3: gates -> f ===========
        for tt in range(5):
            ps = psA.tile([P, 512], f32, tag="psA", name=f"psg{tt}")
            for k in range(4):
                nc.tensor.matmul(ps[:, 0:16], qloc[k][:, tt * P:(tt + 1) * P],
                                 gg_sb[k][:], start=(k == 0), stop=(k == 3))
            sgt = scp.tile([P, 16], f32, tag="sgt", name=f"sgt{tt}")
            nc.scalar.activation(sgt[:], ps[:, 0:16], AF.Sigmoid)
            gm = scp.tile([P, 8], f32, tag="gm", name=f"gm{tt}")
            nc.vector.tensor_tensor(gm[:], sgt[:, 0:8], sgt[:, 8:16], OP.mult)
            gd = scp.tile([P, 8], f32, tag="gd", name=f"gd{tt}")
            nc.vector.tensor_tensor(gd[:], sgt[:, 8:16], gm[:], OP.subtract)
            gs = scp.tile([P, 8], f32, tag="gs", name=f"gs{tt}")
            nc.vector.tensor_tensor(gs[:], gd[:], cbias[:, _CB["sh"]:_CB["sh"] + 8],
                                    OP.mult)
            nc.vector.scalar_tensor_tensor(fsb[tt][:], gs[:], 1.0, sgt[:, 0:8],
                                           OP.add, OP.add)

        # =========== Phase 4: attention per head ===========
        for h in range(H):
            kt = h // 2
            pb = 64 * (h % 2)
            dgs = []
            dbts = []
            for j in range(5):
                dg = dgp.tile([P, P], bf16, tag="dg", name=f"dg{h}_{j}")
                nc.vector.tensor_scalar(dg[:], i128[:], fsb[j][:, h:h + 1], None,
                                        OP.mult)
                dgs.append(dg)
                dbt = dbp.tile([P, 1024], bf16, tag="db", name=f"db{h}_{j}")
                nc.sync.dma_start(dbt[:], dexp_d.ap()[h, j])
                dbts.append(dbt)
            psv = psB.tile([P, FRAME], f32, tag="psB", name=f"psav{h}")
            Pts = []
            for st in range(8):
                ps = psB.tile([P, FRAME], f32, tag="psB", name=f"pssc{h}_{st}")
                nc.tensor.matmul(ps[:, 0:512],
                                 kkb[kt][pb:pb + 64, st * P:(st + 1) * P],
                                 qloc[kt][pb:pb + 64, 0:512],
                                 start=True, stop=False)
                nc.tensor.matmul(ps[:, 512:FRAME],
                                 kkb[kt][pb:pb + 64, st * P:(st + 1) * P],
                                 qloc[kt][pb:pb + 64, 512:FRAME],
                                 start=True, stop=False)
                for j in range(5):
                    nc.tensor.matmul(ps[:, j * P:(j + 1) * P],
                                     dbts[j][:, st * P:(st + 1) * P], dgs[j][:],
                                     start=False, stop=(j == 4))
                Pt = Pp.tile([P, FRAME], bf16, tag="P", name=f"P{h}_{st}")
                nc.scalar.activation(Pt[:], ps[:], AF.Exp)
                Pts.append(Pt)
            for st in range(8):
                nc.tensor.matmul(psv[0:65, 0:512], vT[st][:, 65 * h:65 * h + 65],
                                 Pts[st][:, 0:512],
                                 start=(st == 0), stop=(st == 7))
                nc.tensor.matmul(psv[0:65, 512:FRAME],
                                 vT[st][:, 65 * h:65 * h + 65],
                                 Pts[st][:, 512:FRAME],
                                 start=(st == 0), stop=(st == 7))
            rc = scp.tile([1, FRAME], bf16, tag="rc", name=f"rc{h}")
            with nc.allow_low_precision(reason="softmax recip colsum, bf16 ok"):
                nc.vector.reciprocal(rc[:], psv[64:65, 0:FRAME])
            pb1 = psA.tile([P, 512], f32, tag="psA", name=f"psbc{h}a")
            nc.tensor.matmul(pb1[0:64, 0:512], ones1[:, 0:64], rc[:, 0:512],
                             start=True, stop=True)
            pb2 = psA.tile([P, 512], f32, tag="psA", name=f"psbc{h}b")
            nc.tensor.matmul(pb2[0:64, 0:128], ones1[:, 0:64], rc[:, 512:FRAME],
                             start=True, stop=True)
            rb = scp.tile([64, FRAME], bf16, tag="rb", name=f"rb{h}")
            nc.scalar.activation(rb[:, 0:512], pb1[0:64, 0:512], AF.Copy)
            nc.scalar.activation(rb[:, 512:FRAME], pb2[0:64, 0:128], AF.Copy)
            nc.vector.tensor_tensor(ohat[kt][pb:pb + 64, :], psv[0:64, 0:FRAME],
                                    rb[:], OP.mult)

        # =========== Phase 5: out-proj + residual -> s2 ===========
        for m in range(4):
            ps = psB.tile([P, FRAME], f32, tag="psB", name=f"pso{m}")
            for k in range(4):
                nc.tensor.matmul(ps[:, 0:512], wo_t[k][:, m * P:(m + 1) * P],
                                 ohat[k][:, 0:512], start=(k == 0), stop=(k == 3))
                nc.tensor.matmul(ps[:, 512:FRAME], wo_t[k][:, m * P:(m + 1) * P],
                                 ohat[k][:, 512:FRAME],
                                 start=(k == 0), stop=(k == 3))
            nc.vector.scalar_tensor_tensor(
                s2f[m][:], ps[:], cb("bo", m),
                s1b[m][:, bass.ds(toff_v, FRAME)], OP.add, OP.add)
            nc.vector.tensor_copy(s2b[m][:], s2f[m][:])

        # =========== Phase 6: gn1 stats + pair AllReduce ===========
        nc.gpsimd.memset(stats[:], 0.0)
        sqscr = [scp.tile([P, OWN], bf16, tag="sqscr", name=f"sqs{m}")
                 for m in range(4)]
        for m in range(4):
            nc.vector.tensor_reduce(stats[:, m:m + 1],
                                    s2f[m][:, bass.ds(own0_v, OWN)],
                                    AX.X, OP.add)
            nc.scalar.activation(sqscr[m][:], s2f[m][:, bass.ds(own0_s, OWN)],
                                 AF.Square, accum_out=stats[:, 4 + m:5 + m])
        nc.vector.tensor_reduce(stats2[:, 0:1], stats[:, 0:4], AX.X, OP.add)
        nc.vector.tensor_reduce(stats2[:, 1:2], stats[:, 4:8], AX.X, OP.add)
        cc1i = drp.tile([P, 2], f32, tag="cc1i", name="cc1i")
        cc1o = drp.tile([P, 2], f32, tag="cc1o", name="cc1o")
        nc.sync.dma_start(cc1i[:], stats2[:])
        nc.gpsimd.collective_compute(
            "AllReduce", OP.add,
            replica_groups=[[0, 1], [2, 3], [4, 5], [6, 7]],
            ins=[cc1i[:]], outs=[cc1o[:]])
        nc.sync.dma_start(gla[:], cc1o[:])
        import concourse.bass_isa as bass_isa
        nc.gpsimd.partition_all_reduce(gla[:], gla[:], P, bass_isa.ReduceOp.add)
        # r1 = 1/sqrt(var+eps); r1b = [r1, -m1*r1]
        n_inv = 1.0 / float(D * T)
        nc.vector.tensor_scalar(scr2[:, 0:1], gla[:, 0:1], n_inv, None, OP.mult)
        nc.vector.tensor_scalar(scr2[:, 1:2], gla[:, 1:2], n_inv, None, OP.mult)
        nc.vector.tensor_tensor(scr2[:, 2:3], scr2[:, 0:1], scr2[:, 0:1], OP.mult)
        nc.vector.tensor_tensor(scr2[:, 3:4], scr2[:, 1:2], scr2[:, 2:3],
                                OP.subtract)
        nc.vector.tensor_scalar(scr2[:, 3:4], scr2[:, 3:4], 1e-5, None, OP.add)
        nc.scalar.activation(scr2[:, 4:5], scr2[:, 3:4], AF.Sqrt)
        nc.vector.reciprocal(r1b[:, 0:1], scr2[:, 4:5])
        nc.vector.tensor_tensor(scr2[:, 5:6], scr2[:, 0:1], r1b[:, 0:1], OP.mult)
        nc.vector.tensor_scalar(r1b[:, 1:2], scr2[:, 5:6], -1.0, None, OP.mult)
        for m in range(8):
            nc.vector.scalar_tensor_tensor(biasg[:, m:m + 1], cb("Wg", m),
                                           r1b[:, 1:2], cb("Wb", m),
                                           OP.mult, OP.add)

        # =========== Phase 7: pw1 + GLU ===========
        pw1_t = []
        for k in range(4):
            w = wpA.tile([P, 2 * D], bf16, tag="wA", name=f"pw1g{k}")
            nc.sync.dma_start(w[:], pw1g_d.ap()[k * P:(k + 1) * P, :])
            pw1_t.append(w)
        for m in range(8):
            ps = psB.tile([P, FRAME], f32, tag="psB", name=f"psp1{m}")
            for k in range(4):
                nc.tensor.matmul(ps[:, 0:512], pw1_t[k][:, m * P:(m + 1) * P],
                                 s2b[k][:, 0:512], start=(k == 0), stop=(k == 3))
                nc.tensor.matmul(ps[:, 512:FRAME], pw1_t[k][:, m * P:(m + 1) * P],
                                 s2b[k][:, 512:FRAME],
                                 start=(k == 0), stop=(k == 3))
            if m < 4:
                nc.vector.tensor_scalar(a_sb[m][:], ps[:], r1b[:, 0:1],
                                        biasg[:, m:m + 1], OP.mult, OP.add)
            else:
                nc.scalar.activation(sg_sb[m - 4][:], ps[:], AF.Sigmoid,
                                     bias=biasg[:, m:m + 1], scale=r1b[:, 0:1])
        for m in range(4):
            nc.gpsimd.memset(glu[m][:, 0:PAD], 0.0)
            nc.gpsimd.memset(glu[m][:, PAD + FRAME:FRAME + 2 * PAD], 0.0)
            nc.vector.tensor_tensor(glu[m][:, PAD:PAD + FRAME], a_sb[m][:],
                                    sg_sb[m][:], OP.mult)

        # =========== Phase 8: depthwise conv (diag matmuls) ===========
        for m in range(4):
            ps = psB.tile([P, FRAME], f32, tag="psB", name=f"psdc{m}")
            for k in range(KW):
                dg = dgp.tile([P, P], bf16, tag="dg", name=f"dwg{m}_{k}")
                nc.vector.tensor_scalar(dg[:], i128[:], cb("dw", m * KW + k),
                                        None, OP.mult)
                nc.tensor.matmul(ps[:, 0:512], dg[:], glu[m][:, k:k + 512],
                                 start=(k == 0), stop=False)
                nc.tensor.matmul(ps[:, 512:FRAME], dg[:],
                                 glu[m][:, k + 512:k + FRAME],
                                 start=(k == 0), stop=(k == KW - 1))
            nc.scalar.activation(dcb[m][:], ps[:], AF.Copy)

        # =========== Phase 9: gn2 stats + pair AllReduce ===========
        nc.gpsimd.memset(statsB[:], 0.0)
        sqscr2 = [scp.tile([P, OWN], bf16, tag="sqscr", name=f"sqs2{m}")
                  for m in range(4)]
        sc_t = scp.tile([P, 4], f32, tag="sct", name="sct")
        for m in range(4):
            nc.vector.tensor_reduce(sc_t[:, m:m + 1],
                                    dcb[m][:, bass.ds(own0_v, OWN)],
                                    AX.X, OP.add)
            nc.scalar.activation(sqscr2[m][:], dcb[m][:, bass.ds(own0_s, OWN)],
                                 AF.Square, accum_out=statsB[:, 4 + m:5 + m])
        for m in range(4):
            # sum_adj = sc + 512*dwb ; sq_adj = sq + 2*dwb*sc + 512*dwb^2
            nc.vector.scalar_tensor_tensor(statsB[:, m:m + 1], cb("dwb", m),
                                           512.0, sc_t[:, m:m + 1],
                                           OP.mult, OP.add)
            nc.vector.tensor_tensor(scr2[:, 6:7], cb("dwb", m), sc_t[:, m:m + 1],
                                    OP.mult)
            nc.vector.scalar_tensor_tensor(scr2[:, 7:8], scr2[:, 6:7], 2.0,
                                           statsB[:, 4 + m:5 + m],
                                           OP.mult, OP.add)
            nc.vector.tensor_tensor(scr2[:, 6:7], cb("dwb", m), cb("dwb", m),
                                    OP.mult)
            nc.vector.scalar_tensor_tensor(statsB[:, 4 + m:5 + m], scr2[:, 6:7],
                                           512.0, scr2[:, 7:8], OP.mult, OP.add)
        nc.vector.tensor_reduce(stats2B[:, 0:1], statsB[:, 0:4], AX.X, OP.add)
        nc.vector.tensor_reduce(stats2B[:, 1:2], statsB[:, 4:8], AX.X, OP.add)
        cc2i = drp.tile([P, 2], f32, tag="cc2i", name="cc2i")
        cc2o = drp.tile([P, 2], f32, tag="cc2o", name="cc2o")
        nc.sync.dma_start(cc2i[:], stats2B[:])
        nc.gpsimd.collective_compute(
            "AllReduce", OP.add,
            replica_groups=[[0, 1], [2, 3], [4, 5], [6, 7]],
            ins=[cc2i[:]], outs=[cc2o[:]])
        nc.sync.dma_start(glb[:], cc2o[:])
        nc.gpsimd.partition_all_reduce(glb[:], glb[:], P, bass_isa.ReduceOp.add)
        nc.vector.tensor_scalar(scr2[:, 0:1], glb[:, 0:1], n_inv, None, OP.mult)
        nc.vector.tensor_scalar(scr2[:, 1:2], glb[:, 1:2], n_inv, None, OP.mult)
        nc.vector.tensor_tensor(scr2[:, 2:3], scr2[:, 0:1], scr2[:, 0:1], OP.mult)
        nc.vector.tensor_tensor(scr2[:, 3:4], scr2[:, 1:2], scr2[:, 2:3],
                                OP.subtract)
        nc.vector.tensor_scalar(scr2[:, 3:4], scr2[:, 3:4], 1e-5, None, OP.add)
        nc.scalar.activation(scr2[:, 4:5], scr2[:, 3:4], AF.Sqrt)
        nc.vector.reciprocal(r2b[:, 0:1], scr2[:, 4:5])
        nc.vector.tensor_scalar(r2b[:, 1:2], scr2[:, 0:1], -1.0, None, OP.mult)
        for m in range(4):
            # sact = r2*g2 ; bact = sact*(dwb - m2) + b2g
            nc.vector.tensor_tensor(sact[:, m:m + 1], cb("g2", m), r2b[:, 0:1],
                                    OP.mult)
            nc.vector.tensor_tensor(scr2[:, 6:7], cb("dwb", m), r2b[:, 1:2],
                                    OP.add)  # dwb + (-m2)
            nc.vector.tensor_tensor(scr2[:, 7:8], scr2[:, 6:7], sact[:, m:m + 1],
                                    OP.mult)
            nc.vector.tensor_tensor(bact[:, m:m + 1], scr2[:, 7:8], cb("b2g", m),
                                    OP.add)
        # silu over own region only
        for m in range(4):
            nc.scalar.activation(slown[m][:], dcb[m][:, bass.ds(own0_s, OWN)],
                                 AF.Silu, bias=bact[:, m:m + 1],
                                 scale=sact[:, m:m + 1])

        # =========== Phase 10: pw2 + residual -> c2 ===========
        pw2_t = []
        for k in range(4):
            w = wpB.tile([P, D], bf16, tag="wB", name=f"pw2t{k}")
            nc.sync.dma_start(w[:], pw2t_d.ap()[k * P:(k + 1) * P, :])
            pw2_t.append(w)
        for m in range(4):
            ps = psA.tile([P, 512], f32, tag="psA", name=f"psp2{m}")
            for k in range(4):
                nc.tensor.matmul(ps[:], pw2_t[k][:, m * P:(m + 1) * P],
                                 slown[k][:], start=(k == 0), stop=(k == 3))
            nc.vector.scalar_tensor_tensor(
                c2f[m][:], ps[:], cb("bpw2", m),
                s2f[m][:, bass.ds(own0_v, OWN)], OP.add, OP.add)
            nc.vector.tensor_copy(c2b[m][:], c2f[m][:])

        # =========== Phase 11: FFN2 over own region ===========
        w1b_t = []
        for k in range(4):
            w = wpA.tile([P, FF], bf16, tag="wA", name=f"w1b{k}")
            nc.sync.dma_start(w[:], w1b_d.ap()[k * P:(k + 1) * P, :])
            w1b_t.append(w)
        w2b_t = []
        for k in range(16):
            w = wpB.tile([P, D], bf16, tag="wB", name=f"w2b{k}")
            nc.sync.dma_start(w[:], w2b_d.ap()[k * P:(k + 1) * P, :])
            w2b_t.append(w)
        h2s = []
        for m in range(16):
            ps = psA.tile([P, 512], f32, tag="psA", name=f"psf2{m}")
            for k in range(4):
                nc.tensor.matmul(ps[:], w1b_t[k][:, m * P:(m + 1) * P],
                                 c2b[k][:], start=(k == 0), stop=(k == 3))
            ht = hp.tile([P, 512], bf16, tag="h", name=f"h2_{m}")
            nc.scalar.activation(ht[:], ps[:], AF.Gelu, bias=cb("b1b", m))
            h2s.append(ht)
        for m in range(4):
            ps = psA.tile([P, 512], f32, tag="psA", name=f"psy{m}")
            for k in range(16):
                nc.tensor.matmul(ps[:], w2b_t[k][:, m * P:(m + 1) * P],
                                 h2s[k][:], start=(k == 0), stop=(k == 15))
            ysb = scp.tile([P, OWN], f32, tag="ysb", name=f"y{m}")
            nc.vector.scalar_tensor_tensor(ysb[:], ps[:], cb("b2b", m),
                                           c2f[m][:], OP.add, OP.add)
            nc.sync.dma_start(y_d[m * P:(m + 1) * P, :], ysb[:])

    nc.compile()
    return nc


def _host_prep(inputs):
    inp = {k: np.asarray(v) for k, v in inputs.items()}
    f32 = np.float32
    g1d = inp["rel_embed"][bucket1d(), :].astype(f32)   # [2047, H]

    tb = lambda a: np.ascontiguousarray(a, dtype=f32).astype(bfnp)
    shared = {
        "w1a": tb(inp["ff1_w1"]),
        "w2a": tb(inp["ff1_w2"] * 0.5),
        "wqkvo": tb(np.concatenate([inp["qkv_w"][:, :D] / 8.0,
                                    inp["qkv_w"][:, D:2 * D],
                                    inp["qkv_w"][:, 2 * D:],
                                    inp["out_w"]], axis=1)),
        "pw1g": tb(inp["pw1_w"].T * inp["gn1_g"][:, None]),
        "pw2t": tb(inp["pw2_w"].T),
        "w1b": tb(inp["ff2_w1"]),
        "w2b": tb(inp["ff2_w2"] * 0.5),
        "i128": np.eye(P, dtype=f32).astype(bfnp),
        "ones1": np.ones((1, P), f32).astype(bfnp),
        "bvrow": tb(inp["qkv_b"][2 * D:][None, :]),
    }
    gg = np.zeros((D, 16), f32)
    for h in range(H):
        gg[64 * h:64 * h + 64, h] = 8.0 * inp["gate_u"][h]
        gg[64 * h:64 * h + 64, 8 + h] = 8.0 * inp["gate_w"][h]
    shared["gg"] = gg.astype(bfnp)

    cbias = np.zeros((P, NCB), f32)

    def put(name, vec, n):
        v = np.asarray(vec, f32).reshape(n, P).T          # [128, n]
        cbias[:, _CB[name]:_CB[name] + n] = v

    put("b1a", inp["ff1_b1"], 16)
    put("b2a", inp["ff1_b2"] * 0.5, 4)
    put("bq", inp["qkv_b"][:D] / 8.0, 4)
    put("bk", inp["qkv_b"][D:2 * D], 4)
    put("bo", inp["out_b"], 4)
    pw1T = inp["pw1_w"].T * inp["gn1_g"][:, None]
    put("Wg", pw1T.sum(axis=0), 8)
    put("Wb", inp["pw1_w"] @ inp["gn1_b"] + inp["pw1_b"], 8)
    put("dwb", inp["dw_b"], 4)
    put("g2", inp["gn2_g"], 4)
    put("b2g", inp["gn2_b"], 4)
    put("bpw2", inp["pw2_b"], 4)
    put("b1b", inp["ff2_b1"], 16)
    put("b2b", inp["ff2_b2"] * 0.5, 4)
    cbias[:, _CB["sh"]:_CB["sh"] + 8] = np.asarray(inp["scale_h"], f32)[None, :]
    dw = np.asarray(inp["dw_w"][:, 0, :], f32)            # [D, KW]
    for m in range(4):
        cbias[:, _CB["dw"] + m * KW:_CB["dw"] + (m + 1) * KW] = \
            dw[m * P:(m + 1) * P, :]
    shared["cbias"] = cbias

    # per-parity Toeplitz expansion: dexp[h, j, r, s] = tab_p[1023+128j+r-s]
    dexps = []
    for p in range(2):
        tab = np.zeros((H, TABW), f32)
        jj = np.arange(TABW)
        idx = 2046 - 384 * p - jj
        valid = (idx >= 0) & (idx < 2 * T - 1)
        tab[:, valid] = g1d[idx[valid]].T
        j5 = np.arange(5)[:, None, None]
        r_ = np.arange(P)[None, :, None]
        s_ = np.arange(1024)[None, None, :]
        eidx = 1023 + 128 * j5 + r_ - s_          # [5, 128, 1024] in [0, 1662]
        dexps.append(np.ascontiguousarray(tab[:, eidx]).astype(bfnp))
    in_maps = []
    for c in range(NCORES):
        b, p = c // 2, c % 2
        m = dict(shared)
        m["xb"] = np.ascontiguousarray(inp["x"][b], dtype=f32).astype(bfnp)
        m["dexp"] = dexps[p]
        m["toff"] = np.array([[384 * p]], np.uint32)
        m["own0"] = np.array([[128 * p]], np.uint32)
        in_maps.append(m)
    return in_maps


def get_program():
    if "nc" not in _CACHE:
        _CACHE["nc"] = _build_program()
    return _CACHE["nc"]


def run_cores(inputs, trace=False, **kw):
    from concourse import bass_utils
    nc = get_program()
    in_maps = _host_prep(inputs)
    return bass_utils.run_bass_kernel_spmd(
        nc, in_maps, core_ids=list(range(NCORES)), trace=trace, **kw)


def kernel(**inputs):
    res = run_cores(inputs, trace=False)
    out = np.zeros((B, D, T), np.float32)
    for c in range(NCORES):
        b, p = c // 2, c % 2
        out[b][:, 512 * p:512 * p + 512] = res.results[c]["y"]
    return out


if __name__ == "__main__":
    get_program()
    print("BUILD+COMPILE OK")



# revision 11
# speedup vs baseline: 1.1271x; 1.1271x over previous
"""Trainium2 Bass kernel for nn_ConformerBlock_50525995270849.

Takes FULL unsharded inputs (as produced by setup_inputs()) and returns the
FULL [B, D, T] fp32 output, running on 8 NeuronCores via run_bass_kernel_spmd.

Sharding: core c = (batch b=c//2, T-half parity p=c%2). Each core computes
FFN1 over full T (K/V need all positions), QK/V over full T, attention for
its extended 640-column local query frame (global cols [384p, 384p+640)),
and the conv module + FFN2 for its own 512 columns. GroupNorm(1,C) stats are
pair-reduced with a tiny AllGather; partition reduce + broadcast are done
with ones-matmuls on the PE. The gated relative-position bias is added to
the score PSUM with per-t-block diag(f) matmuls against precomputed diagonal
table expansions.

Perf notes vs the first working version (407us):
- startup DMAs chunked so FFN1 starts ~5us in
- attention bias tables / diag tiles double-buffered across heads; softmax
  normalization moved off the PE (DVE recip -> gpsimd partition_broadcast ->
  DVE mult) so the PE stream never stalls per head
- q computed directly on the local frame (dynamic-slice matmul rhs)
- pw1 output copied raw to SBUF so all 8 matmul groups run during the gn1
  AllGather; scale/bias applied in-place afterwards
- depthwise conv computes own-512 columns only from a shifted glu buffer
"""
import sys
import os

sys.path.insert(0, "/opt/trn_rl_repo")

import numpy as np
import ml_dtypes

B, T, D, H, DH = 4, 1024, 512, 8, 64
FF = 4 * D
KW = 31
NB, MAXD = 320, 800
PAD = KW // 2
NCORES = 8
P = 128
FRAME = 640          # local query frame width
OWN = 512            # own region width
TABW = 1664          # per-core bias table width
GW = OWN + 2 * PAD   # shifted glu buffer width (542)

bfnp = ml_dtypes.bfloat16
_CACHE = {}


def bucket1d():
    half, thr = NB // 2, NB // 4
    r = np.arange(-(T - 1), T, dtype=np.int32)
    sign = (r >= 0).astype(np.int32)
    ap = np.abs(r)
    log_ratio = np.log(np.maximum(ap, 1).astype(np.float32) / thr) / np.float32(
        np.log(MAXD / thr))
    log_pos = np.minimum(np.rint(thr + log_ratio * (half - thr)).astype(np.int32),
                         half - 1)
    return np.clip(np.where(ap < thr, ap, log_pos) + sign * half, 0, NB - 1)


# column layout of the packed per-partition bias tensor cbias [128, NCB]
_CB = {}
_ncb = 0
for _name, _n in [("b1a", 16), ("b2a", 4), ("bq", 4), ("bk", 4), ("bo", 4),
                  ("Wg", 8), ("Wb", 8), ("dwb", 4), ("g2", 4), ("b2g", 4),
                  ("bpw2", 4), ("b1b", 16), ("b2b", 4), ("sh", 8), ("dw", 31 * 4)]:
    _CB[_name] = _ncb
    _ncb += _n
NCB = _ncb


def _build_program():
    import concourse.bass as bass
    import concourse.tile as tile
    from concourse import bacc, mybir
    from contextlib import ExitStack

    f32 = mybir.dt.float32
    bf16 = mybir.dt.bfloat16
    AF = mybir.ActivationFunctionType
    OP = mybir.AluOpType
    AX = mybir.AxisListType

    nc = bacc.Bacc("TRN2", target_bir_lowering=False, debug=False,
                   num_devices=NCORES)

    di = lambda n, s, dt: nc.dram_tensor(n, s, dt, kind="ExternalInput")
    xb_d = di("xb", [D, T], bf16)
    w1a_d = di("w1a", [D, FF], bf16)
    w2a_d = di("w2a", [FF, D], bf16)
    wqkvo_d = di("wqkvo", [D, 4 * D], bf16)   # [wq | wk | wv | wo]
    pw1g_d = di("pw1g", [D, 2 * D], bf16)
    pw2t_d = di("pw2t", [D, D], bf16)
    w1b_d = di("w1b", [D, FF], bf16)
    w2b_d = di("w2b", [FF, D], bf16)
    gg_d = di("gg", [D, 16], bf16)
    i128_d = di("i128", [P, P], bf16)
    ones1_d = di("ones1", [1, P], bf16)
    onesf_d = di("onesf", [P, 130], f32)
    bvrow_d = di("bvrow", [1, D], bf16)
    cbias_d = di("cbias", [P, NCB], f32)
    dexp_d = di("dexp", [H, 5, P, 1024], bf16)
    toff_d = di("toff", [1, 1], mybir.dt.uint32)
    own0_d = di("own0", [1, 1], mybir.dt.uint32)
    gsrc_d = di("gsrc", [1, 1], mybir.dt.uint32)
    gdst_d = di("gdst", [1, 1], mybir.dt.uint32)
    y_d = nc.dram_tensor("y", [D, OWN], f32, kind="ExternalOutput").ap()

    with tile.TileContext(nc) as tc, ExitStack() as ctx:
        pool = lambda name, bufs, **kw: ctx.enter_context(
            tc.tile_pool(name=name, bufs=bufs, **kw))
        cp = pool("const", 1)
        big = pool("big", 1)
        wpC = pool("wpC", 8)        # [128, 1024] bf16 w1a chunk tiles
        wpA = pool("wpA", 4)        # [128, 2048] bf16 weight row-tiles
        wpB = pool("wpB", 17)       # [128, 512] bf16 weight row-tiles
        hp = pool("hp", 16)         # FFN hidden tiles [128, 512] bf16
        Pp = pool("Pp", 9)          # attention probs tiles [128, 640] bf16
        dgp = pool("dgp", 11)       # diag tiles [128, 128] bf16
        dbp = pool("dbp", 9)        # bias table tiles [128, 1024] bf16
        scp = pool("scp", 2)        # small scratch
        rcp = pool("rcp", 2)        # per-head recip rows
        rbp = pool("rbp", 1)        # per-head recip broadcast [64, FRAME]
        drp = pool("drp", 1, space="DRAM")
        psA = pool("psA", 2, space="PSUM")
        psB = pool("psB", 3, space="PSUM")

        # ---- registers (per-engine) for the dynamic offsets ----
        tr_v = nc.vector.alloc_register("toff_v")
        nc.vector.reg_load(tr_v, toff_d[0:1, 0:1])
        toff_v = nc.vector.snap(tr_v, donate=True, min_val=0, max_val=384)
        ow_v = nc.vector.alloc_register("own0_v")
        nc.vector.reg_load(ow_v, own0_d[0:1, 0:1])
        own0_v = nc.vector.snap(ow_v, donate=True, min_val=0, max_val=128)
        ow_s = nc.scalar.alloc_register("own0_s")
        nc.scalar.reg_load(ow_s, own0_d[0:1, 0:1])
        own0_s = nc.scalar.snap(ow_s, donate=True, min_val=0, max_val=128)
        tr_t = nc.tensor.alloc_register("toff_t")
        nc.tensor.reg_load(tr_t, toff_d[0:1, 0:1])
        toff_t = nc.tensor.snap(tr_t, donate=True, min_val=0, max_val=384)
        gs_v = nc.vector.alloc_register("gsrc_v")
        nc.vector.reg_load(gs_v, gsrc_d[0:1, 0:1])
        gsrc_v = nc.vector.snap(gs_v, donate=True, min_val=0, max_val=113)
        gd_v = nc.vector.alloc_register("gdst_v")
        nc.vector.reg_load(gd_v, gdst_d[0:1, 0:1])
        gdst_v = nc.vector.snap(gd_v, donate=True, min_val=0, max_val=15)

        # ---- persistent activation buffers ----
        bt = lambda tg, w, dt=bf16: big.tile([P, w], dt, tag=tg, name=tg)
        xb = [[big.tile([P, 512], bf16, tag=f"xb{k}_{t}", name=f"xb{k}_{t}")
               for t in range(2)] for k in range(4)]

        # ---- startup-critical DMAs first: xb halves + w1a chunk 0 ----
        w1a_t = [[wpC.tile([P, 1024], bf16, tag="wC", name=f"w1a{k}_{g}")
                  for g in range(2)] for k in range(4)]
        for k in range(4):
            nc.sync.dma_start(xb[k][0][:], xb_d.ap()[k * P:(k + 1) * P, 0:512])
        for k in range(4):
            nc.sync.dma_start(w1a_t[k][0][:],
                              w1a_d.ap()[k * P:(k + 1) * P, 0:1024])
        cbias = cp.tile([P, NCB], f32)
        nc.sync.dma_start(cbias[:], cbias_d.ap())
        for k in range(4):
            nc.sync.dma_start(xb[k][1][:], xb_d.ap()[k * P:(k + 1) * P, 512:1024])
        for k in range(4):
            nc.sync.dma_start(w1a_t[k][1][:],
                              w1a_d.ap()[k * P:(k + 1) * P, 1024:2048])

        # ---- remaining constants ----
        i128 = cp.tile([P, P], bf16)
        nc.sync.dma_start(i128[:], i128_d.ap())
        ones1 = cp.tile([1, P], bf16)
        nc.sync.dma_start(ones1[:], ones1_d.ap())
        onesf = cp.tile([P, 130], f32)
        nc.sync.dma_start(onesf[:], onesf_d.ap())
        bvrow = cp.tile([1, D], bf16)
        nc.sync.dma_start(bvrow[:], bvrow_d.ap())
        gg_sb = [cp.tile([P, 16], bf16, tag=f"gg{k}", name=f"gg{k}")
                 for k in range(4)]
        for k in range(4):
            nc.sync.dma_start(gg_sb[k][:], gg_d.ap()[k * P:(k + 1) * P, :])

        def cb(name, i, n=1):
            return cbias[:, _CB[name] + i:_CB[name] + i + n]

        s1b = [bt(f"s1b{k}", T) for k in range(4)]
        kkb = [bt(f"kkb{k}", T) for k in range(4)]
        vT = [bt(f"vT{t}", 520) for t in range(8)]
        qloc = [bt(f"qloc{k}", FRAME) for k in range(4)]
        fsb = [bt(f"fsb{t}", 8, f32) for t in range(5)]
        ohat = [bt(f"ohat{k}", FRAME) for k in range(4)]
        s2f = [bt(f"s2f{k}", FRAME, f32) for k in range(4)]
        s2b = [bt(f"s2b{k}", FRAME) for k in range(4)]
        a_sb = [bt(f"a{m}", FRAME) for m in range(4)]
        sg_sb = [bt(f"sg{m}", FRAME) for m in range(4)]
        glu2 = [bt(f"glu2_{m}", GW) for m in range(4)]
        dcb = [bt(f"dcb{m}", OWN) for m in range(4)]
        slown = [bt(f"slown{m}", OWN) for m in range(4)]
        c2f = [bt(f"c2f{m}", OWN, f32) for m in range(4)]
        c2b = [bt(f"c2b{m}", OWN) for m in range(4)]
        stats = bt("stats", 8, f32)
        stats2 = bt("stats2", 2, f32)
        statsB = bt("statsB", 8, f32)
        stats2B = bt("stats2B", 2, f32)
        agb1 = bt("agb1", 8, f32)   # gn1 allgathered [2 part, 8]
        agb2 = bt("agb2", 8, f32)   # gn2 allgathered
        gla = bt("gla", 2, f32)     # gn1 reduced+broadcast [sum, sq]
        glb = bt("glb", 2, f32)     # gn2
        ccs1 = bt("ccs1", 8, f32)   # [1,8] staging for collective in
        ccs2 = bt("ccs2", 8, f32)
        nc.gpsimd.memset(ccs1[:], 0.0)
        nc.gpsimd.memset(ccs2[:], 0.0)
        r1b = bt("r1b", 2, f32)     # [r1, -m1*r1]
        r2b = bt("r2b", 2, f32)     # [r2, -m2]
        biasg = bt("biasg", 8, f32)
        sact = bt("sact", 4, f32)
        bact = bt("bact", 4, f32)
        scr2 = bt("scr2", 8, f32)   # scalar scratch columns

        # =========== Phase 1: FFN1 over full T ===========
        w2a_t = []
        for k in range(16):
            w = wpB.tile([P, D], bf16, tag="wB", name=f"w2a{k}")
            nc.sync.dma_start(w[:], w2a_d.ap()[k * P:(k + 1) * P, :])
            w2a_t.append(w)
        for tch in range(2):
            c0 = tch * 512
            hs = []
            for m in range(16):
                g, mo = m // 8, (m % 8) * P
                ps = psA.tile([P, 512], f32, tag="psA", name=f"ps1_{tch}_{m}")
                for k in range(4):
                    nc.tensor.matmul(ps[:], w1a_t[k][g][:, mo:mo + P],
                                     xb[k][tch][:],
                                     start=(k == 0), stop=(k == 3))
                ht = hp.tile([P, 512], bf16, tag="h", name=f"h_{tch}_{m}")
                nc.scalar.activation(ht[:], ps[:], AF.Gelu, bias=cb("b1a", m))
                hs.append(ht)
            for m in range(4):
                ps = psA.tile([P, 512], f32, tag="psA", name=f"ps2_{tch}_{m}")
                for k in range(16):
                    nc.tensor.matmul(ps[:], w2a_t[k][:, m * P:(m + 1) * P],
                                     hs[k][:], start=(k == 0), stop=(k == 15))
                nc.vector.scalar_tensor_tensor(
                    s1b[m][:, c0:c0 + 512], ps[:], cb("b2a", m),
                    xb[m][tch][:], OP.add, OP.add)

        # =========== Phase 2: QKV ===========
        wq_t, wk_t, wv_t, wo_t = [], [], [], []
        for k in range(4):
            w = wpA.tile([P, 4 * D], bf16, tag="wA", name=f"wqkvo{k}")
            nc.sync.dma_start(w[:], wqkvo_d.ap()[k * P:(k + 1) * P, :])
            wq_t.append(w[:, 0:D])
            wk_t.append(w[:, D:2 * D])
            wv_t.append(w[:, 2 * D:3 * D])
            wo_t.append(w[:, 3 * D:4 * D])
        # q directly on the local frame (dynamic-start rhs)
        for m in range(4):
            ps = psB.tile([P, FRAME], f32, tag="psB", name=f"psq{m}")
            for k in range(4):
                nc.tensor.matmul(ps[:, 0:512], wq_t[k][:, m * P:(m + 1) * P],
                                 s1b[k][:, bass.ds(toff_t, 512)],
                                 start=(k == 0), stop=(k == 3))
                nc.tensor.matmul(ps[:, 512:FRAME],
                                 wq_t[k][:, m * P:(m + 1) * P],
                                 s1b[k][:, bass.ds(toff_t + 512, 128)],
                                 start=(k == 0), stop=(k == 3))
            nc.scalar.activation(qloc[m][:], ps[:], AF.Identity,
                                 bias=cb("bq", m))
        # k (full T, persistent)
        for m in range(4):
            for tch in range(2):
                c0 = tch * 512
                ps = psA.tile([P, 512], f32, tag="psA", name=f"psk{m}{tch}")
                for k in range(4):
                    nc.tensor.matmul(ps[:], wk_t[k][:, m * P:(m + 1) * P],
                                     s1b[k][:, c0:c0 + 512],
                                     start=(k == 0), stop=(k == 3))
                nc.scalar.activation(kkb[m][:, c0:c0 + 512], ps[:], AF.Identity,
                                     bias=cb("bk", m))
        # vT: [t, dv] with ones columns (65-block layout)
        for t in range(8):
            ps = psA.tile([P, 512], f32, tag="psA", name=f"psv{t}")
            for k in range(4):
                nc.tensor.matmul(ps[:], s1b[k][:, t * P:(t + 1) * P],
                                 wv_t[k][:], start=(k == 0), stop=False)
            nc.tensor.matmul(ps[:], ones1[:, 0:P], bvrow[:],
                             start=False, stop=True)
            src3 = ps[:].rearrange("p (h c) -> p h c", c=64)
            dst3 = vT[t][:, 0:520].rearrange("p (h c) -> p h c", c=65)[:, :, 0:64]
            nc.scalar.activation(dst3, src3, AF.Copy)
            onescol = vT[t][:, 0:520].rearrange("p (h c) -> p h c", c=65)[:, :, 64:65]
            nc.gpsimd.memset(onescol, 1.0)

        # =========== Phase 3: gates -> f ===========
        for tt in range(5):
            ps = psA.tile([P, 512], f32, tag="psA", name=f"psg{tt}")
            for k in range(4):
                nc.tensor.matmul(ps[:, 0:16], qloc[k][:, tt * P:(tt + 1) * P],
                                 gg_sb[k][:], start=(k == 0), stop=(k == 3))
            sgt = scp.tile([P, 16], f32, tag="sgt", name=f"sgt{tt}")
            nc.scalar.activation(sgt[:], ps[:, 0:16], AF.Sigmoid)
            gm = scp.tile([P, 8], f32, tag="gm", name=f"gm{tt}")
            nc.vector.tensor_tensor(gm[:], sgt[:, 0:8], sgt[:, 8:16], OP.mult)
            gd = scp.tile([P, 8], f32, tag="gd", name=f"gd{tt}")
            nc.vector.tensor_tensor(gd[:], sgt[:, 8:16], gm[:], OP.subtract)
            gs = scp.tile([P, 8], f32, tag="gs", name=f"gs{tt}")
            nc.vector.tensor_tensor(gs[:], gd[:], cb("sh", 0, 8), OP.mult)
            nc.vector.scalar_tensor_tensor(fsb[tt][:], gs[:], 1.0, sgt[:, 0:8],
                                           OP.add, OP.add)

        # =========== Phase 4: attention per head ===========
        for h in range(H):
            kt = h // 2
            pb = 64 * (h % 2)
            dgs = []
            dbts = []
            for j in range(5):
                dg = dgp.tile([P, P], bf16, tag="dg", name=f"dg{h}_{j}")
                nc.vector.tensor_scalar(dg[:], i128[:], fsb[j][:, h:h + 1], None,
                                        OP.mult)
                dgs.append(dg)
                dbt = dbp.tile([P, 1024], bf16, tag="db", name=f"db{h}_{j}")
                nc.sync.dma_start(dbt[:], dexp_d.ap()[h, j])
                dbts.append(dbt)
            psv = psB.tile([P, FRAME], f32, tag="psB", name=f"psav{h}")
            Pts = []
            for st in range(8):
                ps = psB.tile([P, FRAME], f32, tag="psB", name=f"pssc{h}_{st}")
                nc.tensor.matmul(ps[:, 0:512],
                                 kkb[kt][pb:pb + 64, st * P:(st + 1) * P],
                                 qloc[kt][pb:pb + 64, 0:512],
                                 start=True, stop=False)
                nc.tensor.matmul(ps[:, 512:FRAME],
                                 kkb[kt][pb:pb + 64, st * P:(st + 1) * P],
                                 qloc[kt][pb:pb + 64, 512:FRAME],
                                 start=True, stop=False)
                for j in range(5):
                    nc.tensor.matmul(ps[:, j * P:(j + 1) * P],
                                     dbts[j][:, st * P:(st + 1) * P], dgs[j][:],
                                     start=False, stop=(j == 4))
                Pt = Pp.tile([P, FRAME], bf16, tag="P", name=f"P{h}_{st}")
                nc.scalar.activation(Pt[:], ps[:], AF.Exp)
                Pts.append(Pt)
            for st in range(8):
                nc.tensor.matmul(psv[0:65, 0:512], vT[st][:, 65 * h:65 * h + 65],
                                 Pts[st][:, 0:512],
                                 start=(st == 0), stop=(st == 7))
                nc.tensor.matmul(psv[0:65, 512:FRAME],
                                 vT[st][:, 65 * h:65 * h + 65],
                                 Pts[st][:, 512:FRAME],
                                 start=(st == 0), stop=(st == 7))
            # normalize off the PE: recip (DVE) -> bcast (gpsimd) -> mult (DVE)
            rc = rcp.tile([1, FRAME], bf16, tag="rc", name=f"rc{h}")
            with nc.allow_low_precision(reason="softmax recip colsum, bf16 ok"):
                nc.vector.reciprocal(rc[:], psv[64:65, 0:FRAME])
            rcb = rbp.tile([64, FRAME], bf16, tag="rcb", name=f"rcb{h}")
            nc.gpsimd.partition_broadcast(rcb[:], rc[:], channels=64)
            nc.vector.tensor_tensor(ohat[kt][pb:pb + 64, :], psv[0:64, 0:FRAME],
                                    rcb[:], OP.mult)

        # =========== Phase 5: out-proj + residual -> s2 ===========
        for m in range(4):
            ps = psB.tile([P, FRAME], f32, tag="psB", name=f"pso{m}")
            for k in range(4):
                nc.tensor.matmul(ps[:, 0:512], wo_t[k][:, m * P:(m + 1) * P],
                                 ohat[k][:, 0:512], start=(k == 0), stop=(k == 3))
                nc.tensor.matmul(ps[:, 512:FRAME], wo_t[k][:, m * P:(m + 1) * P],
                                 ohat[k][:, 512:FRAME],
                                 start=(k == 0), stop=(k == 3))
            nc.vector.scalar_tensor_tensor(
                s2f[m][:], ps[:], cb("bo", m),
                s1b[m][:, bass.ds(toff_v, FRAME)], OP.add, OP.add)
            nc.vector.tensor_copy(s2b[m][:], s2f[m][:])

        # =========== Phase 6: gn1 stats + pair AllGather ===========
        nc.gpsimd.memset(stats[:], 0.0)
        sqscr = [scp.tile([P, OWN], bf16, tag="sqscr", name=f"sqs{m}")
                 for m in range(4)]
        for m in range(4):
            nc.vector.tensor_reduce(stats[:, m:m + 1],
                                    s2f[m][:, bass.ds(own0_v, OWN)],
                                    AX.X, OP.add)
            nc.scalar.activation(sqscr[m][:], s2f[m][:, bass.ds(own0_s, OWN)],
                                 AF.Square, accum_out=stats[:, 4 + m:5 + m])
        nc.vector.tensor_reduce(stats2[:, 0:1], stats[:, 0:4], AX.X, OP.add)
        nc.vector.tensor_reduce(stats2[:, 1:2], stats[:, 4:8], AX.X, OP.add)
        # partition-reduce via ones-matmul, tiny AllGather, sum+broadcast
        pss = psA.tile([P, 512], f32, tag="psA", name="pss1")
        nc.tensor.matmul(pss[0:1, 0:2], onesf[:, 0:1], stats2[:, 0:2],
                         start=True, stop=True)
        nc.vector.tensor_copy(ccs1[0:1, 0:2], pss[0:1, 0:2])
        cc1i = drp.tile([1, 8], f32, tag="cc1i", name="cc1i")
        cc1o = drp.tile([2, 8], f32, tag="cc1o", name="cc1o")
        nc.sync.dma_start(cc1i[:], ccs1[0:1, 0:8])
        nc.gpsimd.collective_compute(
            "AllGather", OP.bypass,
            replica_groups=[[0, 1], [2, 3], [4, 5], [6, 7]],
            ins=[cc1i[:]], outs=[cc1o[:]])
        nc.sync.dma_start(agb1[0:2, 0:8], cc1o[:])
        psb1 = psA.tile([P, 512], f32, tag="psA", name="psb1")
        nc.tensor.matmul(psb1[:, 0:2], onesf[0:2, 2:130],
                         agb1[0:2, 0:2], start=True, stop=True)
        nc.vector.tensor_copy(gla[:], psb1[:, 0:2])
        # r1 = 1/sqrt(var+eps); r1b = [r1, -m1*r1]
        n_inv = 1.0 / float(D * T)
        nc.vector.tensor_scalar(scr2[:, 0:2], gla[:, 0:2], n_inv, None, OP.mult)
        nc.vector.tensor_tensor(scr2[:, 2:3], scr2[:, 0:1], scr2[:, 0:1], OP.mult)
        nc.vector.tensor_tensor(scr2[:, 3:4], scr2[:, 1:2], scr2[:, 2:3],
                                OP.subtract)
        nc.vector.tensor_scalar(scr2[:, 3:4], scr2[:, 3:4], 1e-5, None, OP.add)
        nc.scalar.activation(scr2[:, 4:5], scr2[:, 3:4], AF.Sqrt)
        nc.vector.reciprocal(r1b[:, 0:1], scr2[:, 4:5])
        nc.vector.tensor_tensor(scr2[:, 5:6], scr2[:, 0:1], r1b[:, 0:1], OP.mult)
        nc.vector.tensor_scalar(r1b[:, 1:2], scr2[:, 5:6], -1.0, None, OP.mult)
        nc.vector.scalar_tensor_tensor(biasg[:, 0:8], cb("Wg", 0, 8),
                                       r1b[:, 1:2], cb("Wb", 0, 8),
                                       OP.mult, OP.add)

        # =========== Phase 7: pw1 (raw to SBUF) + GLU ===========
        pw1_t = []
        for k in range(4):
            w = wpA.tile([P, 2 * D], bf16, tag="wA", name=f"pw1g{k}")
            nc.sync.dma_start(w[:], pw1g_d.ap()[k * P:(k + 1) * P, :])
            pw1_t.append(w)
        praw = [Pp.tile([P, FRAME], bf16, tag="P", name=f"praw{m}")
                for m in range(8)]
        for m in range(8):
            ps = psB.tile([P, FRAME], f32, tag="psB", name=f"psp1{m}")
            for k in range(4):
                nc.tensor.matmul(ps[:, 0:512], pw1_t[k][:, m * P:(m + 1) * P],
                                 s2b[k][:, 0:512], start=(k == 0), stop=(k == 3))
                nc.tensor.matmul(ps[:, 512:FRAME], pw1_t[k][:, m * P:(m + 1) * P],
                                 s2b[k][:, 512:FRAME],
                                 start=(k == 0), stop=(k == 3))
            if m % 2 == 0:
                nc.scalar.activation(praw[m][:], ps[:], AF.Copy)
            else:
                nc.vector.tensor_copy(praw[m][:], ps[:])
        # apply gn1 scale/bias once the collective result lands
        for m in range(4):
            nc.vector.tensor_scalar(a_sb[m][:], praw[m][:], r1b[:, 0:1],
                                    biasg[:, m:m + 1], OP.mult, OP.add)
            nc.scalar.activation(sg_sb[m][:], praw[4 + m][:], AF.Sigmoid,
                                 bias=biasg[:, 4 + m:5 + m], scale=r1b[:, 0:1])
        # shifted glu buffer: glu2[:, i] = glu(frame col own0 + i - 15)
        for m in range(4):
            nc.gpsimd.memset(glu2[m][:], 0.0)
            nc.vector.tensor_tensor(glu2[m][:, bass.ds(gdst_v, 527)],
                                    a_sb[m][:, bass.ds(gsrc_v, 527)],
                                    sg_sb[m][:, bass.ds(gsrc_v, 527)], OP.mult)

        # =========== Phase 8: depthwise conv (diag matmuls, own cols) ===========
        for m in range(4):
            ps = psA.tile([P, 512], f32, tag="psA", name=f"psdc{m}")
            for k in range(KW):
                dg = dgp.tile([P, P], bf16, tag="dg", name=f"dwg{m}_{k}")
                nc.vector.tensor_scalar(dg[:], i128[:], cb("dw", m * KW + k),
                                        None, OP.mult)
                nc.tensor.matmul(ps[:], dg[:], glu2[m][:, k:k + 512],
                                 start=(k == 0), stop=(k == KW - 1))
            if m % 2 == 0:
                nc.scalar.activation(dcb[m][:], ps[:], AF.Copy)
            else:
                nc.vector.tensor_copy(dcb[m][:], ps[:])

        # =========== Phase 9: gn2 stats + pair AllGather ===========
        nc.gpsimd.memset(statsB[:], 0.0)
        sqscr2 = [scp.tile([P, OWN], bf16, tag="sqscr", name=f"sqs2{m}")
                  for m in range(4)]
        sc_t = scp.tile([P, 4], f32, tag="sct", name="sct")
        for m in range(4):
            nc.vector.tensor_reduce(sc_t[:, m:m + 1], dcb[m][:], AX.X, OP.add)
            nc.scalar.activation(sqscr2[m][:], dcb[m][:],
                                 AF.Square, accum_out=statsB[:, 4 + m:5 + m])
        # sum_adj = sc + 512*dwb ; sq_adj = sq + 2*dwb*sc + 512*dwb^2
        nc.vector.scalar_tensor_tensor(statsB[:, 0:4], cb("dwb", 0, 4),
                                       512.0, sc_t[:, 0:4], OP.mult, OP.add)
        t1 = scp.tile([P, 4], f32, tag="t1", name="t1")
        nc.vector.tensor_tensor(t1[:], cb("dwb", 0, 4), sc_t[:, 0:4], OP.mult)
        t2 = scp.tile([P, 4], f32, tag="t2", name="t2")
        nc.vector.scalar_tensor_tensor(t2[:], t1[:], 2.0, statsB[:, 4:8],
                                       OP.mult, OP.add)
        nc.vector.tensor_tensor(t1[:], cb("dwb", 0, 4), cb("dwb", 0, 4), OP.mult)
        nc.vector.scalar_tensor_tensor(statsB[:, 4:8], t1[:], 512.0, t2[:],
                                       OP.mult, OP.add)
        nc.vector.tensor_reduce(stats2B[:, 0:1], statsB[:, 0:4], AX.X, OP.add)
        nc.vector.tensor_reduce(stats2B[:, 1:2], statsB[:, 4:8], AX.X, OP.add)
        pss2 = psA.tile([P, 512], f32, tag="psA", name="pss2")
        nc.tensor.matmul(pss2[0:1, 0:2], onesf[:, 0:1], stats2B[:, 0:2],
                         start=True, stop=True)
        nc.vector.tensor_copy(ccs2[0:1, 0:2], pss2[0:1, 0:2])
        cc2i = drp.tile([1, 8], f32, tag="cc2i", name="cc2i")
        cc2o = drp.tile([2, 8], f32, tag="cc2o", name="cc2o")
        nc.sync.dma_start(cc2i[:], ccs2[0:1, 0:8])
        nc.gpsimd.collective_compute(
            "AllGather", OP.bypass,
            replica_groups=[[0, 1], [2, 3], [4, 5], [6, 7]],
            ins=[cc2i[:]], outs=[cc2o[:]])
        nc.sync.dma_start(agb2[0:2, 0:8], cc2o[:])
        psb2 = psA.tile([P, 512], f32, tag="psA", name="psb2")
        nc.tensor.matmul(psb2[:, 0:2], onesf[0:2, 2:130],
                         agb2[0:2, 0:2], start=True, stop=True)
        nc.vector.tensor_copy(glb[:], psb2[:, 0:2])
        nc.vector.tensor_scalar(scr2[:, 0:2], glb[:, 0:2], n_inv, None, OP.mult)
        nc.vector.tensor_tensor(scr2[:, 2:3], scr2[:, 0:1], scr2[:, 0:1], OP.mult)
        nc.vector.tensor_tensor(scr2[:, 3:4], scr2[:, 1:2], scr2[:, 2:3],
                                OP.subtract)
        nc.vector.tensor_scalar(scr2[:, 3:4], scr2[:, 3:4], 1e-5, None, OP.add)
        nc.scalar.activation(scr2[:, 4:5], scr2[:, 3:4], AF.Sqrt)
        nc.vector.reciprocal(r2b[:, 0:1], scr2[:, 4:5])
        nc.vector.tensor_scalar(r2b[:, 1:2], scr2[:, 0:1], -1.0, None, OP.mult)
        # sact = r2*g2 ; bact = sact*(dwb - m2) + b2g
        nc.vector.tensor_scalar(sact[:, 0:4], cb("g2", 0, 4), r2b[:, 0:1],
                                None, OP.mult)
        nc.vector.tensor_scalar(t1[:], cb("dwb", 0, 4), r2b[:, 1:2],
                                None, OP.add)
        nc.vector.tensor_tensor(t2[:], t1[:], sact[:, 0:4], OP.mult)
        nc.vector.tensor_tensor(bact[:, 0:4], t2[:], cb("b2g", 0, 4), OP.add)
        # silu over own region
        for m in range(4):
            nc.scalar.activation(slown[m][:], dcb[m][:],
                                 AF.Silu, bias=bact[:, m:m + 1],
                                 scale=sact[:, m:m + 1])

        # =========== Phase 10: pw2 + residual -> c2 ===========
        pw2_t = []
        for k in range(4):
            w = wpB.tile([P, D], bf16, tag="wB", name=f"pw2t{k}")
            nc.sync.dma_start(w[:], pw2t_d.ap()[k * P:(k + 1) * P, :])
            pw2_t.append(w)
        for m in range(4):
            ps = psA.tile([P, 512], f32, tag="psA", name=f"psp2{m}")
            for k in range(4):
                nc.tensor.matmul(ps[:], pw2_t[k][:, m * P:(m + 1) * P],
                                 slown[k][:], start=(k == 0), stop=(k == 3))
            nc.vector.scalar_tensor_tensor(
                c2f[m][:], ps[:], cb("bpw2", m),
                s2f[m][:, bass.ds(own0_v, OWN)], OP.add, OP.add)
            nc.vector.tensor_copy(c2b[m][:], c2f[m][:])

        # =========== Phase 11: FFN2 over own region ===========
        w1b_t = []
        for k in range(4):
            w = wpA.tile([P, FF], bf16, tag="wA", name=f"w1b{k}")
            nc.sync.dma_start(w[:], w1b_d.ap()[k * P:(k + 1) * P, :])
            w1b_t.append(w)
        w2b_t = []
        for k in range(16):
            w = wpB.tile([P, D], bf16, tag="wB", name=f"w2b{k}")
            nc.sync.dma_start(w[:], w2b_d.ap()[k * P:(k + 1) * P, :])
            w2b_t.append(w)
        h2s = []
        for m in range(16):
            ps = psA.tile([P, 512], f32, tag="psA", name=f"psf2{m}")
            for k in range(4):
                nc.tensor.matmul(ps[:], w1b_t[k][:, m * P:(m + 1) * P],
                                 c2b[k][:], start=(k == 0), stop=(k == 3))
            ht = hp.tile([P, 512], bf16, tag="h", name=f"h2_{m}")
            nc.scalar.activation(ht[:], ps[:], AF.Gelu, bias=cb("b1b", m))
            h2s.append(ht)
        for m in range(4):
            ps = psA.tile([P, 512], f32, tag="psA", name=f"psy{m}")
            for k in range(16):
                nc.tensor.matmul(ps[:], w2b_t[k][:, m * P:(m + 1) * P],
                                 h2s[k][:], start=(k == 0), stop=(k == 15))
            ysb = scp.tile([P, OWN], f32, tag="ysb", name=f"y{m}")
            nc.vector.scalar_tensor_tensor(ysb[:], ps[:], cb("b2b", m),
                                           c2f[m][:], OP.add, OP.add)
            nc.sync.dma_start(y_d[m * P:(m + 1) * P, :], ysb[:])

    nc.compile()
    return nc


def _host_prep(inputs):
    inp = {k: np.asarray(v) for k, v in inputs.items()}
    f32 = np.float32
    g1d = inp["rel_embed"][bucket1d(), :].astype(f32)   # [2047, H]

    tb = lambda a: np.ascontiguousarray(a, dtype=f32).astype(bfnp)
    shared = {
        "w1a": tb(inp["ff1_w1"]),
        "w2a": tb(inp["ff1_w2"] * 0.5),
        "wqkvo": tb(np.concatenate([inp["qkv_w"][:, :D] / 8.0,
                                    inp["qkv_w"][:, D:2 * D],
                                    inp["qkv_w"][:, 2 * D:],
                                    inp["out_w"]], axis=1)),
        "pw1g": tb(inp["pw1_w"].T * inp["gn1_g"][:, None]),
        "pw2t": tb(inp["pw2_w"].T),
        "w1b": tb(inp["ff2_w1"]),
        "w2b": tb(inp["ff2_w2"] * 0.5),
        "i128": np.eye(P, dtype=f32).astype(bfnp),
        "ones1": np.ones((1, P), f32).astype(bfnp),
        "onesf": np.ones((P, 130), f32),
        "bvrow": tb(inp["qkv_b"][2 * D:][None, :]),
    }
    gg = np.zeros((D, 16), f32)
    for h in range(H):
        gg[64 * h:64 * h + 64, h] = 8.0 * inp["gate_u"][h]
        gg[64 * h:64 * h + 64, 8 + h] = 8.0 * inp["gate_w"][h]
    shared["gg"] = gg.astype(bfnp)

    cbias = np.zeros((P, NCB), f32)

    def put(name, vec, n):
        v = np.asarray(vec, f32).reshape(n, P).T          # [128, n]
        cbias[:, _CB[name]:_CB[name] + n] = v

    put("b1a", inp["ff1_b1"], 16)
    put("b2a", inp["ff1_b2"] * 0.5, 4)
    put("bq", inp["qkv_b"][:D] / 8.0, 4)
    put("bk", inp["qkv_b"][D:2 * D], 4)
    put("bo", inp["out_b"], 4)
    pw1T = inp["pw1_w"].T * inp["gn1_g"][:, None]
    put("Wg", pw1T.sum(axis=0), 8)
    put("Wb", inp["pw1_w"] @ inp["gn1_b"] + inp["pw1_b"], 8)
    put("dwb", inp["dw_b"], 4)
    put("g2", inp["gn2_g"], 4)
    put("b2g", inp["gn2_b"], 4)
    put("bpw2", inp["pw2_b"], 4)
    put("b1b", inp["ff2_b1"], 16)
    put("b2b", inp["ff2_b2"] * 0.5, 4)
    cbias[:, _CB["sh"]:_CB["sh"] + 8] = np.asarray(inp["scale_h"], f32)[None, :]
    dw = np.asarray(inp["dw_w"][:, 0, :], f32)            # [D, KW]
    for m in range(4):
        cbias[:, _CB["dw"] + m * KW:_CB["dw"] + (m + 1) * KW] = \
            dw[m * P:(m + 1) * P, :]
    shared["cbias"] = cbias

    # per-parity Toeplitz expansion: dexp[h, j, r, s] = tab_p[1023+128j+r-s]
    dexps = []
    for p in range(2):
        tab = np.zeros((H, TABW), f32)
        jj = np.arange(TABW)
        idx = 2046 - 384 * p - jj
        valid = (idx >= 0) & (idx < 2 * T - 1)
        tab[:, valid] = g1d[idx[valid]].T
        j5 = np.arange(5)[:, None, None]
        r_ = np.arange(P)[None, :, None]
        s_ = np.arange(1024)[None, None, :]
        eidx = 1023 + 128 * j5 + r_ - s_          # [5, 128, 1024] in [0, 1662]
        dexps.append(np.ascontiguousarray(tab[:, eidx]).astype(bfnp))
    in_maps = []
    for c in range(NCORES):
        b, p = c // 2, c % 2
        m = dict(shared)
        m["xb"] = np.ascontiguousarray(inp["x"][b], dtype=f32).astype(bfnp)
        m["dexp"] = dexps[p]
        m["toff"] = np.array([[384 * p]], np.uint32)
        m["own0"] = np.array([[128 * p]], np.uint32)
        m["gsrc"] = np.array([[113 * p]], np.uint32)
        m["gdst"] = np.array([[15 * (1 - p)]], np.uint32)
        in_maps.append(m)
    return in_maps


def get_program():
    if "nc" not in _CACHE:
        _CACHE["nc"] = _build_program()
    return _CACHE["nc"]


def run_cores(inputs, trace=False, **kw):
    from concourse import bass_utils
    nc = get_program()
    in_maps = _host_prep(inputs)
    return bass_utils.run_bass_kernel_spmd(
        nc, in_maps, core_ids=list(range(NCORES)), trace=trace, **kw)


def kernel(**inputs):
    res = run_cores(inputs, trace=False)
    out = np.zeros((B, D, T), np.float32)
    for c in range(NCORES):
        b, p = c // 2, c % 2
        out[b][:, 512 * p:512 * p + 512] = res.results[c]["y"]
    return out


if __name__ == "__main__":
    get_program()
    print("BUILD+COMPILE OK")


# revision 12
# speedup vs baseline: 1.2618x; 1.1195x over previous
"""Trainium2 Bass kernel for nn_ConformerBlock_50525995270849.

Takes FULL unsharded inputs (as produced by setup_inputs()) and returns the
FULL [B, D, T] fp32 output, running on 8 NeuronCores via run_bass_kernel_spmd.

Sharding: core c = (batch b=c//2, T-half parity p=c%2). Each core computes
FFN1 + K/V over full T, attention for its 544-column local query frame
(global cols [480p, 480p+544) = own 512 + 32-col conv halo), and the conv
module + FFN2 for its own 512 columns. GroupNorm(1,C) stats are pair-reduced
with a tiny AllGather; partition reduce / broadcast are ones-matmuls on PE.
The gated relative-position bias is added to the score PSUM with per-t-block
diag(f) matmuls against precomputed diagonal table expansions.

Perf structure (v3):
- batched 3D-AP startup DMAs; FFN1 starts ~10us in
- attention: scores/bias/PV per head; psv copied to SBUF by ACT immediately
  (frees the single psv PSUM slot); softmax normalization (DVE recip ->
  gpsimd partition_broadcast -> DVE mult) runs entirely off the PE stream
- one 3-buf [128,544] PSUM tag for everything + dedicated 1-buf psv tag
- gn stats squares on DVE (no ACT table switches on the critical path);
  pw1 raw->SBUF so its matmuls run during the gn1 AllGather
- depthwise conv computes own-512 columns only, from a shifted glu buffer
"""
import sys
import os

sys.path.insert(0, "/opt/trn_rl_repo")

import numpy as np
import ml_dtypes

B, T, D, H, DH = 4, 1024, 512, 8, 64
FF = 4 * D
KW = 31
NB, MAXD = 320, 800
PAD = KW // 2
NCORES = 8
P = 128
FRAME = 544          # local query frame width (own 512 + 32 halo)
OWN = 512            # own region width
TABW = 1664          # per-core bias table width
GW = OWN + 2 * PAD   # shifted glu buffer width (542)
NJ = 5               # frame col blocks: 4x128 + 1x32
JW4 = FRAME - 4 * P  # width of the last block (32)

bfnp = ml_dtypes.bfloat16
_CACHE = {}


def bucket1d():
    half, thr = NB // 2, NB // 4
    r = np.arange(-(T - 1), T, dtype=np.int32)
    sign = (r >= 0).astype(np.int32)
    ap = np.abs(r)
    log_ratio = np.log(np.maximum(ap, 1).astype(np.float32) / thr) / np.float32(
        np.log(MAXD / thr))
    log_pos = np.minimum(np.rint(thr + log_ratio * (half - thr)).astype(np.int32),
                         half - 1)
    return np.clip(np.where(ap < thr, ap, log_pos) + sign * half, 0, NB - 1)


# column layout of the packed per-partition bias tensor cbias [128, NCB]
_CB = {}
_ncb = 0
for _name, _n in [("b1a", 16), ("b2a", 4), ("bq", 4), ("bk", 4), ("bo", 4),
                  ("Wg", 8), ("Wb", 8), ("dwb", 4), ("g2", 4), ("b2g", 4),
                  ("bpw2", 4), ("b1b", 16), ("b2b", 4), ("sh", 8), ("dw", 31 * 4)]:
    _CB[_name] = _ncb
    _ncb += _n
NCB = _ncb


def _build_program():
    import concourse.bass as bass
    import concourse.tile as tile
    from concourse import bacc, mybir
    from contextlib import ExitStack

    f32 = mybir.dt.float32
    bf16 = mybir.dt.bfloat16
    AF = mybir.ActivationFunctionType
    OP = mybir.AluOpType
    AX = mybir.AxisListType

    nc = bacc.Bacc("TRN2", target_bir_lowering=False, debug=False,
                   num_devices=NCORES)

    di = lambda n, s, dt: nc.dram_tensor(n, s, dt, kind="ExternalInput")
    xb_d = di("xb", [D, T], bf16)
    w1a_d = di("w1a", [D, FF], bf16)
    w2a_d = di("w2a", [FF, D], bf16)
    wqkvo_d = di("wqkvo", [D, 4 * D], bf16)   # [wq | wk | wv | wo]
    pw1g_d = di("pw1g", [D, 2 * D], bf16)
    pw2t_d = di("pw2t", [D, D], bf16)
    w1b_d = di("w1b", [D, FF], bf16)
    w2b_d = di("w2b", [FF, D], bf16)
    gg_d = di("gg", [D, 16], bf16)
    i128_d = di("i128", [P, P], bf16)
    ones1_d = di("ones1", [1, P], bf16)
    onesf_d = di("onesf", [P, 130], f32)
    bvrow_d = di("bvrow", [1, D], bf16)
    cbias_d = di("cbias", [P, NCB], f32)
    dexp_d = di("dexp", [H, NJ, P, 1024], bf16)
    toff_d = di("toff", [1, 1], mybir.dt.uint32)
    own0_d = di("own0", [1, 1], mybir.dt.uint32)
    gsrc_d = di("gsrc", [1, 1], mybir.dt.uint32)
    gdst_d = di("gdst", [1, 1], mybir.dt.uint32)
    y_d = nc.dram_tensor("y", [D, OWN], f32, kind="ExternalOutput").ap()

    with tile.TileContext(nc) as tc, ExitStack() as ctx:
        pool = lambda name, bufs, **kw: ctx.enter_context(
            tc.tile_pool(name=name, bufs=bufs, **kw))
        cp = pool("const", 1)
        big = pool("big", 1)
        wpC = pool("wpC", 4)        # [128, 2048] bf16 w1a k-major chunk tiles
        wpA = pool("wpA", 4)        # [128, 2048] bf16 weight row-tiles
        wpB = pool("wpB", 2)        # [128, 4096] bf16 w2a/w2b k-major tiles
        wpS = pool("wpS", 1)        # [128, 2048] pw2 k-major tile
        hp = pool("hp", 16)         # FFN hidden tiles [128, 512] bf16
        Pp = pool("Pp", 9)          # attention probs tiles [128, 544] bf16
        dgp = pool("dgp", 11)       # diag tiles [128, 128] bf16
        dbp = pool("dbp", 2)        # per-head bias table tiles [128, 5120]
        scp = pool("scp", 2)        # small scratch
        rcp = pool("rcp", 2)        # per-head recip rows
        rbp = pool("rbp", 1)        # per-head recip broadcast [64, FRAME]
        osp = pool("osp", 2)        # per-head psv SBUF copies [65, FRAME]
        drp = pool("drp", 1, space="DRAM")
        psP = pool("psP", 3, space="PSUM")   # [128, 544] f32, 2 banks each
        psV = pool("psV", 1, space="PSUM")   # psv accumulator, 2 banks

        # ---- registers (per-engine) for the dynamic offsets ----
        tr_v = nc.vector.alloc_register("toff_v")
        nc.vector.reg_load(tr_v, toff_d[0:1, 0:1])
        toff_v = nc.vector.snap(tr_v, donate=True, min_val=0, max_val=480)
        ow_v = nc.vector.alloc_register("own0_v")
        nc.vector.reg_load(ow_v, own0_d[0:1, 0:1])
        own0_v = nc.vector.snap(ow_v, donate=True, min_val=0, max_val=32)
        ow_s = nc.scalar.alloc_register("own0_s")
        nc.scalar.reg_load(ow_s, own0_d[0:1, 0:1])
        own0_s = nc.scalar.snap(ow_s, donate=True, min_val=0, max_val=32)
        tr_t = nc.tensor.alloc_register("toff_t")
        nc.tensor.reg_load(tr_t, toff_d[0:1, 0:1])
        toff_t = nc.tensor.snap(tr_t, donate=True, min_val=0, max_val=480)
        gs_v = nc.vector.alloc_register("gsrc_v")
        nc.vector.reg_load(gs_v, gsrc_d[0:1, 0:1])
        gsrc_v = nc.vector.snap(gs_v, donate=True, min_val=0, max_val=17)
        gd_v = nc.vector.alloc_register("gdst_v")
        nc.vector.reg_load(gd_v, gdst_d[0:1, 0:1])
        gdst_v = nc.vector.snap(gd_v, donate=True, min_val=0, max_val=15)

        bt = lambda tg, w, dt=bf16: big.tile([P, w], dt, tag=tg, name=tg)

        # ---- startup-critical DMAs first: xb halves + w1a m-chunks ----
        # xbh[t][p, k*512+c] = x[k*128+p, 512t+c]
        xsrc = xb_d.ap().rearrange("(k p) t -> p k t", k=4)
        xbh = [bt(f"xbh{t}", 2048) for t in range(2)]
        for t in range(2):
            nc.sync.dma_start(
                xbh[t][:].rearrange("p (k c) -> p k c", k=4),
                xsrc[:, :, 512 * t:512 * t + 512])
        # w1g[g][p, k*512+mo] = w1a[k*128+p, 512g+mo]  (m-group g = m//4)
        wsrc = w1a_d.ap().rearrange("(k p) c -> p k c", k=4)
        w1g = [wpC.tile([P, 2048], bf16, tag="wC", name=f"w1g{g}")
               for g in range(4)]
        for g in range(4):
            nc.sync.dma_start(
                w1g[g][:].rearrange("p (k c) -> p k c", k=4),
                wsrc[:, :, 512 * g:512 * g + 512])
        cbias = cp.tile([P, NCB], f32)
        nc.sync.dma_start(cbias[:], cbias_d.ap())
        # w2c[g][p, k*512+c] = w2a[(8g+k)*128+p, c]
        w2src = w2a_d.ap().rearrange("(k p) c -> p k c", k=16)
        w2c = [wpB.tile([P, 4096], bf16, tag="wB", name=f"w2c{g}")
               for g in range(2)]
        for g in range(2):
            nc.sync.dma_start(
                w2c[g][:].rearrange("p (k c) -> p k c", k=8),
                w2src[:, 8 * g:8 * g + 8, :])

        # ---- remaining constants ----
        i128 = cp.tile([P, P], bf16)
        nc.sync.dma_start(i128[:], i128_d.ap())
        ones1 = cp.tile([1, P], bf16)
        nc.sync.dma_start(ones1[:], ones1_d.ap())
        onesf = cp.tile([P, 130], f32)
        nc.sync.dma_start(onesf[:], onesf_d.ap())
        bvrow = cp.tile([1, D], bf16)
        nc.sync.dma_start(bvrow[:], bvrow_d.ap())
        ggc = cp.tile([P, 64], bf16)
        nc.sync.dma_start(ggc[:].rearrange("p (k c) -> p k c", k=4),
                          gg_d.ap().rearrange("(k p) c -> p k c", k=4))

        def cb(name, i, n=1):
            return cbias[:, _CB[name] + i:_CB[name] + i + n]

        s1b = [bt(f"s1b{k}", T) for k in range(4)]
        kkb = [bt(f"kkb{k}", T) for k in range(4)]
        vT = [bt(f"vT{t}", 520) for t in range(8)]
        qloc = [bt(f"qloc{k}", 640) for k in range(4)]
        fsb = [bt(f"fsb{t}", 8, f32) for t in range(NJ)]
        ohat = [bt(f"ohat{k}", FRAME) for k in range(4)]
        s2f = [bt(f"s2f{k}", FRAME) for k in range(4)]
        a_sb = [bt(f"a{m}", FRAME) for m in range(4)]
        sg_sb = [bt(f"sg{m}", FRAME) for m in range(4)]
        glu2 = [bt(f"glu2_{m}", GW) for m in range(4)]
        dcb = [bt(f"dcb{m}", OWN) for m in range(4)]
        slown = [bt(f"slown{m}", OWN) for m in range(4)]
        c2f = [bt(f"c2f{m}", OWN, f32) for m in range(4)]
        c2b = [bt(f"c2b{m}", OWN) for m in range(4)]
        stats = bt("stats", 8, f32)
        stats2 = bt("stats2", 2, f32)
        statsB = bt("statsB", 8, f32)
        stats2B = bt("stats2B", 2, f32)
        agb1 = bt("agb1", 8, f32)   # gn1 allgathered [2 part, 8]
        agb2 = bt("agb2", 8, f32)   # gn2 allgathered
        gla = bt("gla", 2, f32)     # gn1 reduced+broadcast [sum, sq]
        glb = bt("glb", 2, f32)     # gn2
        ccs1 = bt("ccs1", 8, f32)   # [1,8] staging for collective in
        ccs2 = bt("ccs2", 8, f32)
        nc.gpsimd.memset(ccs1[:], 0.0)
        nc.gpsimd.memset(ccs2[:], 0.0)
        r1b = bt("r1b", 2, f32)     # [r1, -m1*r1]
        r2b = bt("r2b", 2, f32)     # [r2, -m2]
        biasg = bt("biasg", 8, f32)
        sact = bt("sact", 4, f32)
        bact = bt("bact", 4, f32)
        scr2 = bt("scr2", 8, f32)   # scalar scratch columns
        # zero-pad qloc cols [544:640) so gate matmul block 4 stays finite
        for m in range(4):
            nc.gpsimd.memset(qloc[m][:, FRAME:640], 0.0)

        # =========== Phase 1: FFN1 over full T ===========
        for tch in range(2):
            hs = []
            for m in range(16):
                g, mo = m // 4, (m % 4) * P
                ps = psP.tile([P, 512], f32, tag="ps", name=f"ps1_{tch}_{m}")
                for k in range(4):
                    nc.tensor.matmul(ps[:], w1g[g][:, k * 512 + mo:k * 512 + mo + P],
                                     xbh[tch][:, k * 512:(k + 1) * 512],
                                     start=(k == 0), stop=(k == 3))
                ht = hp.tile([P, 512], bf16, tag="h", name=f"h_{tch}_{m}")
                nc.scalar.activation(ht[:], ps[:], AF.Gelu, bias=cb("b1a", m))
                hs.append(ht)
            c0 = tch * 512
            for m in range(4):
                ps = psP.tile([P, 512], f32, tag="ps", name=f"ps2_{tch}_{m}")
                for k in range(16):
                    nc.tensor.matmul(
                        ps[:], w2c[k // 8][:, (k % 8) * 512 + m * P:
                                           (k % 8) * 512 + (m + 1) * P],
                        hs[k][:], start=(k == 0), stop=(k == 15))
                nc.vector.scalar_tensor_tensor(
                    s1b[m][:, c0:c0 + 512], ps[:], cb("b2a", m),
                    xbh[tch][:, m * 512:(m + 1) * 512], OP.add, OP.add)

        # =========== Phase 2: QKV ===========
        wq_t, wk_t, wv_t, wo_t = [], [], [], []
        for k in range(4):
            w = wpA.tile([P, 4 * D], bf16, tag="wA", name=f"wqkvo{k}")
            nc.sync.dma_start(w[:], wqkvo_d.ap()[k * P:(k + 1) * P, :])
            wq_t.append(w[:, 0:D])
            wk_t.append(w[:, D:2 * D])
            wv_t.append(w[:, 2 * D:3 * D])
            wo_t.append(w[:, 3 * D:4 * D])
        # q directly on the local frame (dynamic-start rhs)
        for m in range(4):
            ps = psP.tile([P, FRAME], f32, tag="ps", name=f"psq{m}")
            for k in range(4):
                nc.tensor.matmul(ps[:, 0:512], wq_t[k][:, m * P:(m + 1) * P],
                                 s1b[k][:, bass.ds(toff_t, 512)],
                                 start=(k == 0), stop=(k == 3))
                nc.tensor.matmul(ps[:, 512:FRAME],
                                 wq_t[k][:, m * P:(m + 1) * P],
                                 s1b[k][:, bass.ds(toff_t + 512, JW4)],
                                 start=(k == 0), stop=(k == 3))
            nc.scalar.activation(qloc[m][:, 0:FRAME], ps[:], AF.Identity,
                                 bias=cb("bq", m))
        # k (full T, persistent)
        for m in range(4):
            for tch in range(2):
                c0 = tch * 512
                ps = psP.tile([P, 512], f32, tag="ps", name=f"psk{m}{tch}")
                for k in range(4):
                    nc.tensor.matmul(ps[:], wk_t[k][:, m * P:(m + 1) * P],
                                     s1b[k][:, c0:c0 + 512],
                                     start=(k == 0), stop=(k == 3))
                nc.scalar.activation(kkb[m][:, c0:c0 + 512], ps[:], AF.Identity,
                                     bias=cb("bk", m))
        # vT: [t, dv] with ones columns (65-block layout)
        for t in range(8):
            ps = psP.tile([P, 512], f32, tag="ps", name=f"psvp{t}")
            for k in range(4):
                nc.tensor.matmul(ps[:], s1b[k][:, t * P:(t + 1) * P],
                                 wv_t[k][:], start=(k == 0), stop=False)
            nc.tensor.matmul(ps[:], ones1[:, 0:P], bvrow[:],
                             start=False, stop=True)
            src3 = ps[:].rearrange("p (h c) -> p h c", c=64)
            dst3 = vT[t][:, 0:520].rearrange("p (h c) -> p h c", c=65)[:, :, 0:64]
            nc.scalar.activation(dst3, src3, AF.Copy)
            onescol = vT[t][:, 0:520].rearrange("p (h c) -> p h c", c=65)[:, :, 64:65]
            nc.gpsimd.memset(onescol, 1.0)

        # =========== Phase 3: gates -> f ===========
        for tt in range(NJ):
            ps = psP.tile([P, 512], f32, tag="ps", name=f"psg{tt}")
            for k in range(4):
                nc.tensor.matmul(ps[:, 0:16], qloc[k][:, tt * P:(tt + 1) * P],
                                 ggc[:, k * 16:(k + 1) * 16],
                                 start=(k == 0), stop=(k == 3))
            sgt = scp.tile([P, 16], f32, tag="sgt", name=f"sgt{tt}")
            nc.scalar.activation(sgt[:], ps[:, 0:16], AF.Sigmoid)
            gm = scp.tile([P, 8], f32, tag="gm", name=f"gm{tt}")
            nc.vector.tensor_tensor(gm[:], sgt[:, 0:8], sgt[:, 8:16], OP.mult)
            gd = scp.tile([P, 8], f32, tag="gd", name=f"gd{tt}")
            nc.vector.tensor_tensor(gd[:], sgt[:, 8:16], gm[:], OP.subtract)
            gs = scp.tile([P, 8], f32, tag="gs", name=f"gs{tt}")
            nc.vector.tensor_tensor(gs[:], gd[:], cb("sh", 0, 8), OP.mult)
            nc.vector.scalar_tensor_tensor(fsb[tt][:], gs[:], 1.0, sgt[:, 0:8],
                                           OP.add, OP.add)

        # =========== Phase 4: attention per head ===========
        osbs = []
        for h in range(H):
            kt = h // 2
            pb = 64 * (h % 2)
            jw = lambda j: JW4 if j == 4 else P
            dgs = []
            dbt = dbp.tile([P, NJ * 1024], bf16, tag="db", name=f"db{h}")
            nc.sync.dma_start(
                dbt[:].rearrange("p (j c) -> p j c", j=NJ),
                dexp_d.ap()[h].rearrange("j p c -> p j c"))
            for j in range(NJ):
                dg = dgp.tile([P, P], bf16, tag="dg", name=f"dg{h}_{j}")
                nc.vector.tensor_scalar(dg[:, 0:jw(j)], i128[:, 0:jw(j)],
                                        fsb[j][:, h:h + 1], None, OP.mult)
                dgs.append(dg)
            psv = psV.tile([P, FRAME], f32, tag="psv", name=f"psav{h}")
            Pts = []
            for st in range(8):
                ps = psP.tile([P, FRAME], f32, tag="ps", name=f"pssc{h}_{st}")
                nc.tensor.matmul(ps[:, 0:512],
                                 kkb[kt][pb:pb + 64, st * P:(st + 1) * P],
                                 qloc[kt][pb:pb + 64, 0:512],
                                 start=True, stop=False)
                nc.tensor.matmul(ps[:, 512:FRAME],
                                 kkb[kt][pb:pb + 64, st * P:(st + 1) * P],
                                 qloc[kt][pb:pb + 64, 512:FRAME],
                                 start=True, stop=False)
                for j in range(NJ):
                    nc.tensor.matmul(ps[:, j * P:j * P + jw(j)],
                                     dbt[:, j * 1024 + st * P:
                                         j * 1024 + (st + 1) * P],
                                     dgs[j][:, 0:jw(j)],
                                     start=False, stop=(j == 4))
                Pt = Pp.tile([P, FRAME], bf16, tag="P", name=f"P{h}_{st}")
                nc.scalar.activation(Pt[:], ps[:], AF.Exp)
                Pts.append(Pt)
            for st in range(8):
                nc.tensor.matmul(psv[0:65, 0:512], vT[st][:, 65 * h:65 * h + 65],
                                 Pts[st][:, 0:512],
                                 start=(st == 0), stop=(st == 7))
                nc.tensor.matmul(psv[0:65, 512:FRAME],
                                 vT[st][:, 65 * h:65 * h + 65],
                                 Pts[st][:, 512:FRAME],
                                 start=(st == 0), stop=(st == 7))
            # free the psv slot fast: ACT copy to SBUF, normalize off-band
            osb = osp.tile([65, FRAME], bf16, tag="osb", name=f"osb{h}")
            nc.scalar.activation(osb[:], psv[0:65, 0:FRAME], AF.Copy)
            osbs.append(osb)
            rc = rcp.tile([1, FRAME], bf16, tag="rc", name=f"rc{h}")
            with nc.allow_low_precision(reason="softmax recip colsum, bf16 ok"):
                nc.vector.reciprocal(rc[:], osb[64:65, 0:FRAME])
            rcb = rbp.tile([64, FRAME], bf16, tag="rcb", name=f"rcb{h}")
            nc.gpsimd.partition_broadcast(rcb[:], rc[:], channels=64)
            nc.vector.tensor_tensor(ohat[kt][pb:pb + 64, :], osb[0:64, 0:FRAME],
                                    rcb[:], OP.mult)

        # =========== Phase 5: out-proj + residual -> s2 ===========
        for m in range(4):
            ps = psP.tile([P, FRAME], f32, tag="ps", name=f"pso{m}")
            for k in range(4):
                nc.tensor.matmul(ps[:, 0:512], wo_t[k][:, m * P:(m + 1) * P],
                                 ohat[k][:, 0:512], start=(k == 0), stop=(k == 3))
                nc.tensor.matmul(ps[:, 512:FRAME], wo_t[k][:, m * P:(m + 1) * P],
                                 ohat[k][:, 512:FRAME],
                                 start=(k == 0), stop=(k == 3))
            nc.vector.scalar_tensor_tensor(
                s2f[m][:], ps[:], cb("bo", m),
                s1b[m][:, bass.ds(toff_v, FRAME)], OP.add, OP.add)

        # =========== Phase 6: gn1 stats + pair AllGather ===========
        sqt = [scp.tile([P, OWN], bf16, tag="sqt", name=f"sqt{m}")
               for m in range(4)]
        for m in range(4):
            nc.vector.tensor_reduce(stats[:, m:m + 1],
                                    s2f[m][:, bass.ds(own0_v, OWN)],
                                    AX.X, OP.add)
            nc.vector.tensor_tensor(sqt[m][:], s2f[m][:, bass.ds(own0_v, OWN)],
                                    s2f[m][:, bass.ds(own0_v, OWN)], OP.mult)
            nc.vector.tensor_reduce(stats[:, 4 + m:5 + m], sqt[m][:],
                                    AX.X, OP.add)
        nc.vector.tensor_reduce(stats2[:, 0:1], stats[:, 0:4], AX.X, OP.add)
        nc.vector.tensor_reduce(stats2[:, 1:2], stats[:, 4:8], AX.X, OP.add)
        # partition-reduce via ones-matmul, tiny AllGather, sum+broadcast
        pss = psP.tile([P, 512], f32, tag="ps", name="pss1")
        nc.tensor.matmul(pss[0:1, 0:2], onesf[:, 0:1], stats2[:, 0:2],
                         start=True, stop=True)
        nc.vector.tensor_copy(ccs1[0:1, 0:2], pss[0:1, 0:2])
        cc1i = drp.tile([1, 8], f32, tag="cc1i", name="cc1i")
        cc1o = drp.tile([2, 8], f32, tag="cc1o", name="cc1o")
        nc.sync.dma_start(cc1i[:], ccs1[0:1, 0:8])
        nc.gpsimd.collective_compute(
            "AllGather", OP.bypass,
            replica_groups=[[0, 1], [2, 3], [4, 5], [6, 7]],
            ins=[cc1i[:]], outs=[cc1o[:]])
        nc.sync.dma_start(agb1[0:2, 0:8], cc1o[:])
        psb1 = psP.tile([P, 512], f32, tag="ps", name="psb1")
        nc.tensor.matmul(psb1[:, 0:2], onesf[0:2, 2:130],
                         agb1[0:2, 0:2], start=True, stop=True)
        nc.vector.tensor_copy(gla[:], psb1[:, 0:2])
        # r1 = 1/sqrt(var+eps); r1b = [r1, -m1*r1]
        n_inv = 1.0 / float(D * T)
        nc.vector.tensor_scalar(scr2[:, 0:2], gla[:, 0:2], n_inv, None, OP.mult)
        nc.vector.tensor_tensor(scr2[:, 2:3], scr2[:, 0:1], scr2[:, 0:1], OP.mult)
        nc.vector.tensor_tensor(scr2[:, 3:4], scr2[:, 1:2], scr2[:, 2:3],
                                OP.subtract)
        nc.vector.tensor_scalar(scr2[:, 3:4], scr2[:, 3:4], 1e-5, None, OP.add)
        nc.scalar.activation(scr2[:, 4:5], scr2[:, 3:4], AF.Sqrt)
        nc.vector.reciprocal(r1b[:, 0:1], scr2[:, 4:5])
        nc.vector.tensor_tensor(scr2[:, 5:6], scr2[:, 0:1], r1b[:, 0:1], OP.mult)
        nc.vector.tensor_scalar(r1b[:, 1:2], scr2[:, 5:6], -1.0, None, OP.mult)
        nc.vector.scalar_tensor_tensor(biasg[:, 0:8], cb("Wg", 0, 8),
                                       r1b[:, 1:2], cb("Wb", 0, 8),
                                       OP.mult, OP.add)

        # =========== Phase 7: pw1 (raw to SBUF) + GLU ===========
        pw1_t = []
        for k in range(4):
            w = wpA.tile([P, 2 * D], bf16, tag="wA", name=f"pw1g{k}")
            nc.sync.dma_start(w[:], pw1g_d.ap()[k * P:(k + 1) * P, :])
            pw1_t.append(w)
        praw = [Pp.tile([P, FRAME], bf16, tag="P", name=f"praw{m}")
                for m in range(8)]
        for m in range(8):
            ps = psP.tile([P, FRAME], f32, tag="ps", name=f"psp1{m}")
            for k in range(4):
                nc.tensor.matmul(ps[:, 0:512], pw1_t[k][:, m * P:(m + 1) * P],
                                 s2f[k][:, 0:512], start=(k == 0), stop=(k == 3))
                nc.tensor.matmul(ps[:, 512:FRAME], pw1_t[k][:, m * P:(m + 1) * P],
                                 s2f[k][:, 512:FRAME],
                                 start=(k == 0), stop=(k == 3))
            if m % 2 == 0:
                nc.scalar.activation(praw[m][:], ps[:], AF.Copy)
            else:
                nc.vector.tensor_copy(praw[m][:], ps[:])
        # apply gn1 scale/bias once the collective result lands
        for m in range(4):
            nc.vector.tensor_scalar(a_sb[m][:], praw[m][:], r1b[:, 0:1],
                                    biasg[:, m:m + 1], OP.mult, OP.add)
            nc.scalar.activation(sg_sb[m][:], praw[4 + m][:], AF.Sigmoid,
                                 bias=biasg[:, 4 + m:5 + m], scale=r1b[:, 0:1])
        # shifted glu buffer: glu2[:, i] = glu(frame col own0 + i - 15)
        for m in range(4):
            nc.gpsimd.memset(glu2[m][:], 0.0)
            nc.vector.tensor_tensor(glu2[m][:, bass.ds(gdst_v, 527)],
                                    a_sb[m][:, bass.ds(gsrc_v, 527)],
                                    sg_sb[m][:, bass.ds(gsrc_v, 527)], OP.mult)

        # =========== Phase 8: depthwise conv (diag matmuls, own cols) ===========
        for m in range(4):
            ps = psP.tile([P, 512], f32, tag="ps", name=f"psdc{m}")
            for k in range(KW):
                dg = dgp.tile([P, P], bf16, tag="dg", name=f"dwg{m}_{k}")
                nc.vector.tensor_scalar(dg[:], i128[:], cb("dw", m * KW + k),
                                        None, OP.mult)
                nc.tensor.matmul(ps[:], dg[:], glu2[m][:, k:k + 512],
                                 start=(k == 0), stop=(k == KW - 1))
            if m % 2 == 0:
                nc.scalar.activation(dcb[m][:], ps[:], AF.Copy)
            else:
                nc.vector.tensor_copy(dcb[m][:], ps[:])

        # =========== Phase 9: gn2 stats + pair AllGather ===========
        sqt2 = [scp.tile([P, OWN], bf16, tag="sqt", name=f"sqt2{m}")
                for m in range(4)]
        sc_t = scp.tile([P, 4], f32, tag="sct", name="sct")
        for m in range(4):
            nc.vector.tensor_reduce(sc_t[:, m:m + 1], dcb[m][:], AX.X, OP.add)
            nc.vector.tensor_tensor(sqt2[m][:], dcb[m][:], dcb[m][:], OP.mult)
            nc.vector.tensor_reduce(statsB[:, 4 + m:5 + m], sqt2[m][:],
                                    AX.X, OP.add)
        # sum_adj = sc + 512*dwb ; sq_adj = sq + 2*dwb*sc + 512*dwb^2
        nc.vector.scalar_tensor_tensor(statsB[:, 0:4], cb("dwb", 0, 4),
                                       512.0, sc_t[:, 0:4], OP.mult, OP.add)
        t1 = scp.tile([P, 4], f32, tag="t1", name="t1")
        nc.vector.tensor_tensor(t1[:], cb("dwb", 0, 4), sc_t[:, 0:4], OP.mult)
        t2 = scp.tile([P, 4], f32, tag="t2", name="t2")
        nc.vector.scalar_tensor_tensor(t2[:], t1[:], 2.0, statsB[:, 4:8],
                                       OP.mult, OP.add)
        nc.vector.tensor_tensor(t1[:], cb("dwb", 0, 4), cb("dwb", 0, 4), OP.mult)
        nc.vector.scalar_tensor_tensor(statsB[:, 4:8], t1[:], 512.0, t2[:],
                                       OP.mult, OP.add)
        nc.vector.tensor_reduce(stats2B[:, 0:1], statsB[:, 0:4], AX.X, OP.add)
        nc.vector.tensor_reduce(stats2B[:, 1:2], statsB[:, 4:8], AX.X, OP.add)
        pss2 = psP.tile([P, 512], f32, tag="ps", name="pss2")
        nc.tensor.matmul(pss2[0:1, 0:2], onesf[:, 0:1], stats2B[:, 0:2],
                         start=True, stop=True)
        nc.vector.tensor_copy(ccs2[0:1, 0:2], pss2[0:1, 0:2])
        cc2i = drp.tile([1, 8], f32, tag="cc2i", name="cc2i")
        cc2o = drp.tile([2, 8], f32, tag="cc2o", name="cc2o")
        nc.sync.dma_start(cc2i[:], ccs2[0:1, 0:8])
        nc.gpsimd.collective_compute(
            "AllGather", OP.bypass,
            replica_groups=[[0, 1], [2, 3], [4, 5], [6, 7]],
            ins=[cc2i[:]], outs=[cc2o[:]])
        nc.sync.dma_start(agb2[0:2, 0:8], cc2o[:])
        psb2 = psP.tile([P, 512], f32, tag="ps", name="psb2")
        nc.tensor.matmul(psb2[:, 0:2], onesf[0:2, 2:130],
                         agb2[0:2, 0:2], start=True, stop=True)
        nc.vector.tensor_copy(glb[:], psb2[:, 0:2])
        nc.vector.tensor_scalar(scr2[:, 0:2], glb[:, 0:2], n_inv, None, OP.mult)
        nc.vector.tensor_tensor(scr2[:, 2:3], scr2[:, 0:1], scr2[:, 0:1], OP.mult)
        nc.vector.tensor_tensor(scr2[:, 3:4], scr2[:, 1:2], scr2[:, 2:3],
                                OP.subtract)
        nc.vector.tensor_scalar(scr2[:, 3:4], scr2[:, 3:4], 1e-5, None, OP.add)
        nc.scalar.activation(scr2[:, 4:5], scr2[:, 3:4], AF.Sqrt)
        nc.vector.reciprocal(r2b[:, 0:1], scr2[:, 4:5])
        nc.vector.tensor_scalar(r2b[:, 1:2], scr2[:, 0:1], -1.0, None, OP.mult)
        # sact = r2*g2 ; bact = sact*(dwb - m2) + b2g
        nc.vector.tensor_scalar(sact[:, 0:4], cb("g2", 0, 4), r2b[:, 0:1],
                                None, OP.mult)
        nc.vector.tensor_scalar(t1[:], cb("dwb", 0, 4), r2b[:, 1:2],
                                None, OP.add)
        nc.vector.tensor_tensor(t2[:], t1[:], sact[:, 0:4], OP.mult)
        nc.vector.tensor_tensor(bact[:, 0:4], t2[:], cb("b2g", 0, 4), OP.add)
        # silu over own region
        for m in range(4):
            nc.scalar.activation(slown[m][:], dcb[m][:],
                                 AF.Silu, bias=bact[:, m:m + 1],
                                 scale=sact[:, m:m + 1])

        # =========== Phase 10: pw2 + residual -> c2 ===========
        pw2c = wpS.tile([P, 2048], bf16, tag="wS", name="pw2c")
        nc.sync.dma_start(pw2c[:].rearrange("p (k c) -> p k c", k=4),
                          pw2t_d.ap().rearrange("(k p) c -> p k c", k=4))
        for m in range(4):
            ps = psP.tile([P, 512], f32, tag="ps", name=f"psp2{m}")
            for k in range(4):
                nc.tensor.matmul(ps[:], pw2c[:, k * 512 + m * P:
                                             k * 512 + (m + 1) * P],
                                 slown[k][:], start=(k == 0), stop=(k == 3))
            nc.vector.scalar_tensor_tensor(
                c2f[m][:], ps[:], cb("bpw2", m),
                s2f[m][:, bass.ds(own0_v, OWN)], OP.add, OP.add)
            nc.vector.tensor_copy(c2b[m][:], c2f[m][:])

        # =========== Phase 11: FFN2 over own region ===========
        w1b_t = []
        for k in range(4):
            w = wpA.tile([P, FF], bf16, tag="wA", name=f"w1b{k}")
            nc.sync.dma_start(w[:], w1b_d.ap()[k * P:(k + 1) * P, :])
            w1b_t.append(w)
        w2bsrc = w2b_d.ap().rearrange("(k p) c -> p k c", k=16)
        w2bc = [wpB.tile([P, 4096], bf16, tag="wB", name=f"w2bc{g}")
                for g in range(2)]
        for g in range(2):
            nc.sync.dma_start(
                w2bc[g][:].rearrange("p (k c) -> p k c", k=8),
                w2bsrc[:, 8 * g:8 * g + 8, :])
        h2s = []
        for m in range(16):
            ps = psP.tile([P, 512], f32, tag="ps", name=f"psf2{m}")
            for k in range(4):
                nc.tensor.matmul(ps[:], w1b_t[k][:, m * P:(m + 1) * P],
                                 c2b[k][:], start=(k == 0), stop=(k == 3))
            ht = hp.tile([P, 512], bf16, tag="h", name=f"h2_{m}")
            nc.scalar.activation(ht[:], ps[:], AF.Gelu, bias=cb("b1b", m))
            h2s.append(ht)
        for m in range(4):
            ps = psP.tile([P, 512], f32, tag="ps", name=f"psy{m}")
            for k in range(16):
                nc.tensor.matmul(
                    ps[:], w2bc[k // 8][:, (k % 8) * 512 + m * P:
                                       (k % 8) * 512 + (m + 1) * P],
                    h2s[k][:], start=(k == 0), stop=(k == 15))
            ysb = scp.tile([P, OWN], f32, tag="ysb", name=f"y{m}")
            nc.vector.scalar_tensor_tensor(ysb[:], ps[:], cb("b2b", m),
                                           c2f[m][:], OP.add, OP.add)
            nc.sync.dma_start(y_d[m * P:(m + 1) * P, :], ysb[:])

    nc.compile()
    return nc


def _host_prep(inputs):
    inp = {k: np.asarray(v) for k, v in inputs.items()}
    f32 = np.float32
    g1d = inp["rel_embed"][bucket1d(), :].astype(f32)   # [2047, H]

    tb = lambda a: np.ascontiguousarray(a, dtype=f32).astype(bfnp)
    shared = {
        "w1a": tb(inp["ff1_w1"]),
        "w2a": tb(inp["ff1_w2"] * 0.5),
        "wqkvo": tb(np.concatenate([inp["qkv_w"][:, :D] / 8.0,
                                    inp["qkv_w"][:, D:2 * D],
                                    inp["qkv_w"][:, 2 * D:],
                                    inp["out_w"]], axis=1)),
        "pw1g": tb(inp["pw1_w"].T * inp["gn1_g"][:, None]),
        "pw2t": tb(inp["pw2_w"].T),
        "w1b": tb(inp["ff2_w1"]),
        "w2b": tb(inp["ff2_w2"] * 0.5),
        "i128": np.eye(P, dtype=f32).astype(bfnp),
        "ones1": np.ones((1, P), f32).astype(bfnp),
        "onesf": np.ones((P, 130), f32),
        "bvrow": tb(inp["qkv_b"][2 * D:][None, :]),
    }
    gg = np.zeros((D, 16), f32)
    for h in range(H):
        gg[64 * h:64 * h + 64, h] = 8.0 * inp["gate_u"][h]
        gg[64 * h:64 * h + 64, 8 + h] = 8.0 * inp["gate_w"][h]
    shared["gg"] = gg.astype(bfnp)

    cbias = np.zeros((P, NCB), f32)

    def put(name, vec, n):
        v = np.asarray(vec, f32).reshape(n, P).T          # [128, n]
        cbias[:, _CB[name]:_CB[name] + n] = v

    put("b1a", inp["ff1_b1"], 16)
    put("b2a", inp["ff1_b2"] * 0.5, 4)
    put("bq", inp["qkv_b"][:D] / 8.0, 4)
    put("bk", inp["qkv_b"][D:2 * D], 4)
    put("bo", inp["out_b"], 4)
    pw1T = inp["pw1_w"].T * inp["gn1_g"][:, None]
    put("Wg", pw1T.sum(axis=0), 8)
    put("Wb", inp["pw1_w"] @ inp["gn1_b"] + inp["pw1_b"], 8)
    put("dwb", inp["dw_b"], 4)
    put("g2", inp["gn2_g"], 4)
    put("b2g", inp["gn2_b"], 4)
    put("bpw2", inp["pw2_b"], 4)
    put("b1b", inp["ff2_b1"], 16)
    put("b2b", inp["ff2_b2"] * 0.5, 4)
    cbias[:, _CB["sh"]:_CB["sh"] + 8] = np.asarray(inp["scale_h"], f32)[None, :]
    dw = np.asarray(inp["dw_w"][:, 0, :], f32)            # [D, KW]
    for m in range(4):
        cbias[:, _CB["dw"] + m * KW:_CB["dw"] + (m + 1) * KW] = \
            dw[m * P:(m + 1) * P, :]
    shared["cbias"] = cbias

    # per-parity Toeplitz expansion: dexp[h, j, r, s] = tab_p[1023+128j+r-s]
    # where tab_p[jj] = g1d[2046 - 480p - jj]  (toff = 480p)
    dexps = []
    for p in range(2):
        tab = np.zeros((H, TABW), f32)
        jj = np.arange(TABW)
        idx = 2046 - 480 * p - jj
        valid = (idx >= 0) & (idx < 2 * T - 1)
        tab[:, valid] = g1d[idx[valid]].T
        j5 = np.arange(NJ)[:, None, None]
        r_ = np.arange(P)[None, :, None]
        s_ = np.arange(1024)[None, None, :]
        eidx = 1023 + 128 * j5 + r_ - s_          # [5, 128, 1024] in [0, 1662]
        dexps.append(np.ascontiguousarray(tab[:, eidx]).astype(bfnp))
    in_maps = []
    for c in range(NCORES):
        b, p = c // 2, c % 2
        m = dict(shared)
        m["xb"] = np.ascontiguousarray(inp["x"][b], dtype=f32).astype(bfnp)
        m["dexp"] = dexps[p]
        m["toff"] = np.array([[480 * p]], np.uint32)
        m["own0"] = np.array([[32 * p]], np.uint32)
        m["gsrc"] = np.array([[17 * p]], np.uint32)
        m["gdst"] = np.array([[15 * (1 - p)]], np.uint32)
        in_maps.append(m)
    return in_maps


def get_program():
    if "nc" not in _CACHE:
        _CACHE["nc"] = _build_program()
    return _CACHE["nc"]


def run_cores(inputs, trace=False, **kw):
    from concourse import bass_utils
    nc = get_program()
    in_maps = _host_prep(inputs)
    return bass_utils.run_bass_kernel_spmd(
        nc, in_maps, core_ids=list(range(NCORES)), trace=trace, **kw)


def kernel(**inputs):
    res = run_cores(inputs, trace=False)
    out = np.zeros((B, D, T), np.float32)
    for c in range(NCORES):
        b, p = c // 2, c % 2
        out[b][:, 512 * p:512 * p + 512] = res.results[c]["y"]
    return out


if __name__ == "__main__":
    get_program()
    print("BUILD+COMPILE OK")


# revision 22
# speedup vs baseline: 1.2704x; 1.0068x over previous
"""Trainium2 Bass kernel for nn_ConformerBlock_50525995270849.

Takes FULL unsharded inputs (as produced by setup_inputs()) and returns the
FULL [B, D, T] fp32 output, running on 8 NeuronCores via run_bass_kernel_spmd.

Sharding: core c = (batch b=c//2, T-half parity p=c%2). Each core computes
FFN1 + K/V over full T, attention for its 544-column local query frame
(global cols [480p, 480p+544) = own 512 + 32-col conv halo), and the conv
module + FFN2 for its own 512 columns. GroupNorm(1,C) stats are pair-reduced
with a tiny AllGather; partition reduce / broadcast are ones-matmuls on PE.
The gated relative-position bias is added to the score PSUM with per-t-block
diag(f) matmuls against precomputed diagonal table expansions.

Perf structure (v3):
- batched 3D-AP startup DMAs; FFN1 starts ~10us in
- attention: scores/bias/PV per head; psv copied to SBUF by ACT immediately
  (frees the single psv PSUM slot); softmax normalization (DVE recip ->
  gpsimd partition_broadcast -> DVE mult) runs entirely off the PE stream
- one 3-buf [128,544] PSUM tag for everything + dedicated 1-buf psv tag
- gn stats squares on DVE (no ACT table switches on the critical path);
  pw1 raw->SBUF so its matmuls run during the gn1 AllGather
- depthwise conv computes own-512 columns only, from a shifted glu buffer
"""
import sys
import os

sys.path.insert(0, "/opt/trn_rl_repo")

import numpy as np
import ml_dtypes

B, T, D, H, DH = 4, 1024, 512, 8, 64
FF = 4 * D
KW = 31
NB, MAXD = 320, 800
PAD = KW // 2
NCORES = 8
P = 128
FRAME = 544          # local query frame width (own 512 + 32 halo)
OWN = 512            # own region width
TABW = 1664          # per-core bias table width
GW = OWN + 2 * PAD   # shifted glu buffer width (542)
NJ = 5               # frame col blocks: 4x128 + 1x32
JW4 = FRAME - 4 * P  # width of the last block (32)

bfnp = ml_dtypes.bfloat16
_CACHE = {}


def bucket1d():
    half, thr = NB // 2, NB // 4
    r = np.arange(-(T - 1), T, dtype=np.int32)
    sign = (r >= 0).astype(np.int32)
    ap = np.abs(r)
    log_ratio = np.log(np.maximum(ap, 1).astype(np.float32) / thr) / np.float32(
        np.log(MAXD / thr))
    log_pos = np.minimum(np.rint(thr + log_ratio * (half - thr)).astype(np.int32),
                         half - 1)
    return np.clip(np.where(ap < thr, ap, log_pos) + sign * half, 0, NB - 1)


# column layout of the packed per-partition bias tensor cbias [128, NCB]
_CB = {}
_ncb = 0
for _name, _n in [("b1a", 16), ("b2a", 4), ("bq", 4), ("bk", 4), ("bo", 4),
                  ("Wg", 8), ("Wb", 8), ("dwb", 4), ("g2", 4), ("b2g", 4),
                  ("bpw2", 4), ("b1b", 16), ("b2b", 4), ("sh", 8), ("dw", 31 * 4)]:
    _CB[_name] = _ncb
    _ncb += _n
NCB = _ncb


def _build_program():
    import concourse.bass as bass
    import concourse.tile as tile
    from concourse import bacc, mybir
    from contextlib import ExitStack

    f32 = mybir.dt.float32
    bf16 = mybir.dt.bfloat16
    AF = mybir.ActivationFunctionType
    OP = mybir.AluOpType
    AX = mybir.AxisListType

    nc = bacc.Bacc("TRN2", target_bir_lowering=False, debug=False,
                   num_devices=NCORES)

    di = lambda n, s, dt: nc.dram_tensor(n, s, dt, kind="ExternalInput")
    xb_d = di("xb", [D, T], bf16)
    w1a_d = di("w1a", [D, FF], bf16)
    w2a_d = di("w2a", [FF, D], bf16)
    wqkvo_d = di("wqkvo", [D, 4 * D], bf16)   # [wq | wk | wv | wo]
    pw1g_d = di("pw1g", [D, 2 * D], bf16)
    pw2t_d = di("pw2t", [D, D], bf16)
    w1b_d = di("w1b", [D, FF], bf16)
    w2b_d = di("w2b", [FF, D], bf16)
    gg_d = di("gg", [D, 16], bf16)
    i128_d = di("i128", [P, P], bf16)
    ones1_d = di("ones1", [1, P], bf16)
    onesf_d = di("onesf", [P, 130], f32)
    bvrow_d = di("bvrow", [1, D], bf16)
    cbias_d = di("cbias", [P, NCB], f32)
    dexp_d = di("dexp", [H, NJ, P, 1024], bf16)
    dwdiag_d = di("dwdiag", [4, P, KW * P], bf16)
    toff_d = di("toff", [1, 1], mybir.dt.uint32)
    own0_d = di("own0", [1, 1], mybir.dt.uint32)
    gsrc_d = di("gsrc", [1, 1], mybir.dt.uint32)
    gdst_d = di("gdst", [1, 1], mybir.dt.uint32)
    y_d = nc.dram_tensor("y", [D, OWN], f32, kind="ExternalOutput").ap()

    with tile.TileContext(nc) as tc, ExitStack() as ctx:
        pool = lambda name, bufs, **kw: ctx.enter_context(
            tc.tile_pool(name=name, bufs=bufs, **kw))
        cp = pool("const", 1)
        big = pool("big", 1)
        wpC = pool("wpC", 4)        # [128, 2048] bf16 w1a k-major chunk tiles
        wpA = pool("wpA", 4)        # [128, 2048] bf16 weight row-tiles
        wpB = pool("wpB", 2)        # [128, 4096] bf16 w2a/w2b k-major tiles
        wpS = pool("wpS", 1)        # [128, 2048] pw2 k-major tile
        hp = pool("hp", 16)         # FFN hidden tiles [128, 512] bf16
        Pp = pool("Pp", 9)          # attention probs tiles [128, 544] bf16
        dgp = pool("dgp", 11)       # diag tiles [128, 128] bf16
        dwp = pool("dwp", 2)        # conv diag chunk tiles [128, 2048] bf16
        dbp = pool("dbp", 2)        # per-head bias table tiles [128, 5120]
        scp = pool("scp", 2)        # small scratch
        rcp = pool("rcp", 2)        # per-head recip rows
        rbp = pool("rbp", 1)        # per-head recip broadcast [64, FRAME]
        osp = pool("osp", 2)        # per-head psv SBUF copies [65, FRAME]
        drp = pool("drp", 1, space="DRAM")
        psP = pool("psP", 3, space="PSUM")   # [128, 544] f32, 2 banks each
        psV = pool("psV", 1, space="PSUM")   # psv accumulator, 2 banks

        # ---- registers (per-engine) for the dynamic offsets ----
        tr_v = nc.vector.alloc_register("toff_v")
        nc.vector.reg_load(tr_v, toff_d[0:1, 0:1])
        toff_v = nc.vector.snap(tr_v, donate=True, min_val=0, max_val=480)
        ow_v = nc.vector.alloc_register("own0_v")
        nc.vector.reg_load(ow_v, own0_d[0:1, 0:1])
        own0_v = nc.vector.snap(ow_v, donate=True, min_val=0, max_val=32)
        ow_s = nc.scalar.alloc_register("own0_s")
        nc.scalar.reg_load(ow_s, own0_d[0:1, 0:1])
        own0_s = nc.scalar.snap(ow_s, donate=True, min_val=0, max_val=32)
        tr_t = nc.tensor.alloc_register("toff_t")
        nc.tensor.reg_load(tr_t, toff_d[0:1, 0:1])
        toff_t = nc.tensor.snap(tr_t, donate=True, min_val=0, max_val=480)
        gs_v = nc.vector.alloc_register("gsrc_v")
        nc.vector.reg_load(gs_v, gsrc_d[0:1, 0:1])
        gsrc_v = nc.vector.snap(gs_v, donate=True, min_val=0, max_val=17)
        gd_v = nc.vector.alloc_register("gdst_v")
        nc.vector.reg_load(gd_v, gdst_d[0:1, 0:1])
        gdst_v = nc.vector.snap(gd_v, donate=True, min_val=0, max_val=15)

        bt = lambda tg, w, dt=bf16: big.tile([P, w], dt, tag=tg, name=tg)

        # ---- startup-critical DMAs first: xb half 0 + w1a m-chunk 0 ----
        # xbh[t][p, k*512+c] = x[k*128+p, 512t+c]
        xsrc = xb_d.ap().rearrange("(k p) t -> p k t", k=4)
        xbh = [bt(f"xbh{t}", 2048) for t in range(2)]
        # w1g[g][p, k*512+mo] = w1a[k*128+p, 512g+mo]  (m-group g = m//4)
        wsrc = w1a_d.ap().rearrange("(k p) c -> p k c", k=4)
        w1g = [wpC.tile([P, 2048], bf16, tag="wC", name=f"w1g{g}")
               for g in range(4)]
        dma_xbh = lambda t: nc.sync.dma_start(
            xbh[t][:].rearrange("p (k c) -> p k c", k=4),
            xsrc[:, :, 512 * t:512 * t + 512])
        dma_w1g = lambda g: nc.sync.dma_start(
            w1g[g][:].rearrange("p (k c) -> p k c", k=4),
            wsrc[:, :, 512 * g:512 * g + 512])
        dma_xbh(0)
        dma_w1g(0)
        cbias = cp.tile([P, NCB], f32)
        nc.sync.dma_start(cbias[:], cbias_d.ap())
        dma_w1g(1)
        dma_xbh(1)
        dma_w1g(2)
        dma_w1g(3)
        # w2c[g][p, k*512+c] = w2a[(8g+k)*128+p, c]
        w2src = w2a_d.ap().rearrange("(k p) c -> p k c", k=16)
        w2c = [wpB.tile([P, 4096], bf16, tag="wB", name=f"w2c{g}")
               for g in range(2)]
        for g in range(2):
            nc.sync.dma_start(
                w2c[g][:].rearrange("p (k c) -> p k c", k=8),
                w2src[:, 8 * g:8 * g + 8, :])

        # ---- remaining constants ----
        i128 = cp.tile([P, P], bf16)
        nc.sync.dma_start(i128[:], i128_d.ap())
        ones1 = cp.tile([1, P], bf16)
        nc.sync.dma_start(ones1[:], ones1_d.ap())
        onesf = cp.tile([P, 130], f32)
        nc.sync.dma_start(onesf[:], onesf_d.ap())
        bvrow = cp.tile([1, D], bf16)
        nc.sync.dma_start(bvrow[:], bvrow_d.ap())
        ggc = cp.tile([P, 64], bf16)
        nc.sync.dma_start(ggc[:].rearrange("p (k c) -> p k c", k=4),
                          gg_d.ap().rearrange("(k p) c -> p k c", k=4))

        def cb(name, i, n=1):
            return cbias[:, _CB[name] + i:_CB[name] + i + n]

        s1b = [bt(f"s1b{k}", T) for k in range(4)]
        kkb = [bt(f"kkb{k}", T) for k in range(4)]
        vT = [bt(f"vT{t}", 520) for t in range(8)]
        qloc = [bt(f"qloc{k}", 640) for k in range(4)]
        fsb = [bt(f"fsb{t}", 8, f32) for t in range(NJ)]
        ohat = [bt(f"ohat{k}", FRAME) for k in range(4)]
        s2f = [bt(f"s2f{k}", FRAME) for k in range(4)]
        a_sb = [bt(f"a{m}", FRAME) for m in range(4)]
        sg_sb = [bt(f"sg{m}", FRAME) for m in range(4)]
        glu2 = [bt(f"glu2_{m}", GW) for m in range(4)]
        dcb = [bt(f"dcb{m}", OWN) for m in range(4)]
        slown = [bt(f"slown{m}", OWN) for m in range(4)]
        c2f = [bt(f"c2f{m}", OWN) for m in range(4)]
        stats = bt("stats", 8, f32)
        stats2 = bt("stats2", 2, f32)
        statsB = bt("statsB", 8, f32)
        stats2B = bt("stats2B", 2, f32)
        agb1 = bt("agb1", 8, f32)   # gn1 allgathered [2 part, 8]
        agb2 = bt("agb2", 8, f32)   # gn2 allgathered
        gla = bt("gla", 2, f32)     # gn1 reduced+broadcast [sum, sq]
        glb = bt("glb", 2, f32)     # gn2
        ccs1 = bt("ccs1", 8, f32)   # [1,8] staging for collective in
        ccs2 = bt("ccs2", 8, f32)
        nc.gpsimd.memset(ccs1[:], 0.0)
        nc.gpsimd.memset(ccs2[:], 0.0)
        r1b = bt("r1b", 2, f32)     # [r1, -m1*r1]
        r2b = bt("r2b", 2, f32)     # [r2, -m2]
        biasg = bt("biasg", 8, f32)
        sact = bt("sact", 4, f32)
        bact = bt("bact", 4, f32)
        scr2 = bt("scr2", 8, f32)   # scalar scratch columns
        # zero-pad qloc cols [544:640) so gate matmul block 4 stays finite
        for m in range(4):
            nc.gpsimd.memset(qloc[m][:, FRAME:640], 0.0)

        # =========== Phase 1: FFN1 over full T ===========
        for tch in range(2):
            hs = []
            for m in range(16):
                g, mo = m // 4, (m % 4) * P
                ps = psP.tile([P, 512], f32, tag="ps", name=f"ps1_{tch}_{m}")
                for k in range(4):
                    nc.tensor.matmul(ps[:], w1g[g][:, k * 512 + mo:k * 512 + mo + P],
                                     xbh[tch][:, k * 512:(k + 1) * 512],
                                     start=(k == 0), stop=(k == 3))
                ht = hp.tile([P, 512], bf16, tag="h", name=f"h_{tch}_{m}")
                nc.scalar.activation(ht[:], ps[:], AF.Gelu, bias=cb("b1a", m))
                hs.append(ht)
            c0 = tch * 512
            for m in range(4):
                ps = psP.tile([P, 512], f32, tag="ps", name=f"ps2_{tch}_{m}")
                for k in range(16):
                    nc.tensor.matmul(
                        ps[:], w2c[k // 8][:, (k % 8) * 512 + m * P:
                                           (k % 8) * 512 + (m + 1) * P],
                        hs[k][:], start=(k == 0), stop=(k == 15))
                nc.vector.scalar_tensor_tensor(
                    s1b[m][:, c0:c0 + 512], ps[:], cb("b2a", m),
                    xbh[tch][:, m * 512:(m + 1) * 512], OP.add, OP.add)

        # =========== Phase 2: QKV ===========
        wq_t, wk_t, wv_t, wo_t = [], [], [], []
        for k in range(4):
            w = wpA.tile([P, 4 * D], bf16, tag="wA", name=f"wqkvo{k}")
            nc.sync.dma_start(w[:], wqkvo_d.ap()[k * P:(k + 1) * P, :])
            wq_t.append(w[:, 0:D])
            wk_t.append(w[:, D:2 * D])
            wv_t.append(w[:, 2 * D:3 * D])
            wo_t.append(w[:, 3 * D:4 * D])
        # q directly on the local frame (dynamic-start rhs)
        for m in range(4):
            ps = psP.tile([P, FRAME], f32, tag="ps", name=f"psq{m}")
            for k in range(4):
                nc.tensor.matmul(ps[:, 0:512], wq_t[k][:, m * P:(m + 1) * P],
                                 s1b[k][:, bass.ds(toff_t, 512)],
                                 start=(k == 0), stop=(k == 3))
                nc.tensor.matmul(ps[:, 512:FRAME],
                                 wq_t[k][:, m * P:(m + 1) * P],
                                 s1b[k][:, bass.ds(toff_t + 512, JW4)],
                                 start=(k == 0), stop=(k == 3))
            nc.scalar.activation(qloc[m][:, 0:FRAME], ps[:], AF.Identity,
                                 bias=cb("bq", m))
        # k (full T, persistent)
        for m in range(4):
            for tch in range(2):
                c0 = tch * 512
                ps = psP.tile([P, 512], f32, tag="ps", name=f"psk{m}{tch}")
                for k in range(4):
                    nc.tensor.matmul(ps[:], wk_t[k][:, m * P:(m + 1) * P],
                                     s1b[k][:, c0:c0 + 512],
                                     start=(k == 0), stop=(k == 3))
                nc.scalar.activation(kkb[m][:, c0:c0 + 512], ps[:], AF.Identity,
                                     bias=cb("bk", m))
        # vT: [t, dv] with ones columns (65-block layout)
        for t in range(8):
            ps = psP.tile([P, 512], f32, tag="ps", name=f"psvp{t}")
            for k in range(4):
                nc.tensor.matmul(ps[:], s1b[k][:, t * P:(t + 1) * P],
                                 wv_t[k][:], start=(k == 0), stop=False)
            nc.tensor.matmul(ps[:], ones1[:, 0:P], bvrow[:],
                             start=False, stop=True)
            src3 = ps[:].rearrange("p (h c) -> p h c", c=64)
            dst3 = vT[t][:, 0:520].rearrange("p (h c) -> p h c", c=65)[:, :, 0:64]
            nc.scalar.activation(dst3, src3, AF.Copy)
            onescol = vT[t][:, 0:520].rearrange("p (h c) -> p h c", c=65)[:, :, 64:65]
            nc.gpsimd.memset(onescol, 1.0)

        # =========== Phase 3: gates -> f ===========
        for tt in range(NJ):
            ps = psP.tile([P, 512], f32, tag="ps", name=f"psg{tt}")
            for k in range(4):
                nc.tensor.matmul(ps[:, 0:16], qloc[k][:, tt * P:(tt + 1) * P],
                                 ggc[:, k * 16:(k + 1) * 16],
                                 start=(k == 0), stop=(k == 3))
            sgt = scp.tile([P, 16], f32, tag="sgt", name=f"sgt{tt}")
            nc.scalar.activation(sgt[:], ps[:, 0:16], AF.Sigmoid)
            gm = scp.tile([P, 8], f32, tag="gm", name=f"gm{tt}")
            nc.vector.tensor_tensor(gm[:], sgt[:, 0:8], sgt[:, 8:16], OP.mult)
            gd = scp.tile([P, 8], f32, tag="gd", name=f"gd{tt}")
            nc.vector.tensor_tensor(gd[:], sgt[:, 8:16], gm[:], OP.subtract)
            gs = scp.tile([P, 8], f32, tag="gs", name=f"gs{tt}")
            nc.vector.tensor_tensor(gs[:], gd[:], cb("sh", 0, 8), OP.mult)
            nc.vector.scalar_tensor_tensor(fsb[tt][:], gs[:], 1.0, sgt[:, 0:8],
                                           OP.add, OP.add)

        # =========== Phase 4: attention per head ===========
        osbs = []
        for h in range(H):
            kt = h // 2
            pb = 64 * (h % 2)
            jw = lambda j: JW4 if j == 4 else P
            dgs = []
            dbt = dbp.tile([P, NJ * 1024], bf16, tag="db", name=f"db{h}")
            nc.sync.dma_start(
                dbt[:].rearrange("p (j c) -> p j c", j=NJ),
                dexp_d.ap()[h].rearrange("j p c -> p j c"))
            for j in range(NJ):
                dg = dgp.tile([P, P], bf16, tag="dg", name=f"dg{h}_{j}")
                nc.vector.tensor_scalar(dg[:, 0:jw(j)], i128[:, 0:jw(j)],
                                        fsb[j][:, h:h + 1], None, OP.mult)
                dgs.append(dg)
            psv = psV.tile([P, FRAME], f32, tag="psv", name=f"psav{h}")
            Pts = []
            for st in range(8):
                ps = psP.tile([P, FRAME], f32, tag="ps", name=f"pssc{h}_{st}")
                nc.tensor.matmul(ps[:, 0:512],
                                 kkb[kt][pb:pb + 64, st * P:(st + 1) * P],
                                 qloc[kt][pb:pb + 64, 0:512],
                                 start=True, stop=False)
                nc.tensor.matmul(ps[:, 512:FRAME],
                                 kkb[kt][pb:pb + 64, st * P:(st + 1) * P],
                                 qloc[kt][pb:pb + 64, 512:FRAME],
                                 start=True, stop=False)
                for j in range(NJ):
                    nc.tensor.matmul(ps[:, j * P:j * P + jw(j)],
                                     dbt[:, j * 1024 + st * P:
                                         j * 1024 + (st + 1) * P],
                                     dgs[j][:, 0:jw(j)],
                                     start=False, stop=(j == 4))
                Pt = Pp.tile([P, FRAME], bf16, tag="P", name=f"P{h}_{st}")
                nc.scalar.activation(Pt[:], ps[:], AF.Exp)
                Pts.append(Pt)
            for st in range(8):
                nc.tensor.matmul(psv[0:65, 0:512], vT[st][:, 65 * h:65 * h + 65],
                                 Pts[st][:, 0:512],
                                 start=(st == 0), stop=(st == 7))
                nc.tensor.matmul(psv[0:65, 512:FRAME],
                                 vT[st][:, 65 * h:65 * h + 65],
                                 Pts[st][:, 512:FRAME],
                                 start=(st == 0), stop=(st == 7))
            # free the psv slot fast: ACT copy to SBUF, normalize off-band
            osb = osp.tile([65, FRAME], bf16, tag="osb", name=f"osb{h}")
            nc.scalar.activation(osb[:], psv[0:65, 0:FRAME], AF.Copy)
            osbs.append(osb)
            rc = rcp.tile([1, FRAME], bf16, tag="rc", name=f"rc{h}")
            with nc.allow_low_precision(reason="softmax recip colsum, bf16 ok"):
                nc.vector.reciprocal(rc[:], osb[64:65, 0:FRAME])
            rcb = rbp.tile([64, FRAME], bf16, tag="rcb", name=f"rcb{h}")
            nc.gpsimd.partition_broadcast(rcb[:], rc[:], channels=64)
            nc.vector.tensor_tensor(ohat[kt][pb:pb + 64, :], osb[0:64, 0:FRAME],
                                    rcb[:], OP.mult)

        # =========== Phase 5: out-proj + residual -> s2 ===========
        for m in range(4):
            ps = psP.tile([P, FRAME], f32, tag="ps", name=f"pso{m}")
            for k in range(4):
                nc.tensor.matmul(ps[:, 0:512], wo_t[k][:, m * P:(m + 1) * P],
                                 ohat[k][:, 0:512], start=(k == 0), stop=(k == 3))
                nc.tensor.matmul(ps[:, 512:FRAME], wo_t[k][:, m * P:(m + 1) * P],
                                 ohat[k][:, 512:FRAME],
                                 start=(k == 0), stop=(k == 3))
            nc.vector.scalar_tensor_tensor(
                s2f[m][:], ps[:], cb("bo", m),
                s1b[m][:, bass.ds(toff_v, FRAME)], OP.add, OP.add)

        # =========== Phase 6a: pw1 raw matmuls (cover the gn1 AllGather) ===========
        # pw1c[g] holds k = 2g, 2g+1 (k-major packed); DMA starts at FFN1 end
        pw1src = pw1g_d.ap().rearrange("(k p) c -> p k c", k=4)
        pw1c = [wpC.tile([P, 2048], bf16, tag="wC", name=f"pw1c{g}")
                for g in range(2)]
        for g in range(2):
            nc.sync.dma_start(
                pw1c[g][:].rearrange("p (k c) -> p k c", k=2),
                pw1src[:, 2 * g:2 * g + 2, :])
        praw = [Pp.tile([P, FRAME], bf16, tag="P", name=f"praw{m}")
                for m in range(8)]
        for m in range(8):
            ps = psP.tile([P, FRAME], f32, tag="ps", name=f"psp1{m}")
            for k in range(4):
                w = pw1c[k // 2][:, (k % 2) * 1024 + m * P:
                                 (k % 2) * 1024 + (m + 1) * P]
                nc.tensor.matmul(ps[:, 0:512], w,
                                 s2f[k][:, 0:512], start=(k == 0), stop=(k == 3))
                nc.tensor.matmul(ps[:, 512:FRAME], w,
                                 s2f[k][:, 512:FRAME],
                                 start=(k == 0), stop=(k == 3))
            if m % 2 == 0:
                nc.scalar.activation(praw[m][:], ps[:], AF.Copy)
            else:
                nc.vector.tensor_copy(praw[m][:], ps[:])

        # =========== Phase 6b: gn1 stats + pair AllGather ===========
        sqt = [scp.tile([P, OWN], bf16, tag="sqt", name=f"sqt{m}")
               for m in range(4)]
        for m in range(4):
            nc.vector.tensor_reduce(stats[:, m:m + 1],
                                    s2f[m][:, bass.ds(own0_v, OWN)],
                                    AX.X, OP.add)
            nc.vector.tensor_tensor(sqt[m][:], s2f[m][:, bass.ds(own0_v, OWN)],
                                    s2f[m][:, bass.ds(own0_v, OWN)], OP.mult)
            nc.vector.tensor_reduce(stats[:, 4 + m:5 + m], sqt[m][:],
                                    AX.X, OP.add)
        nc.vector.tensor_reduce(stats2[:, 0:1], stats[:, 0:4], AX.X, OP.add)
        nc.vector.tensor_reduce(stats2[:, 1:2], stats[:, 4:8], AX.X, OP.add)
        # partition-reduce via ones-matmul, tiny AllGather, sum+broadcast
        pss = psP.tile([P, 512], f32, tag="ps", name="pss1")
        nc.tensor.matmul(pss[0:1, 0:2], onesf[:, 0:1], stats2[:, 0:2],
                         start=True, stop=True)
        nc.vector.tensor_copy(ccs1[0:1, 0:2], pss[0:1, 0:2])
        cc1i = drp.tile([1, 8], f32, tag="cc1i", name="cc1i")
        cc1o = drp.tile([2, 8], f32, tag="cc1o", name="cc1o")
        nc.sync.dma_start(cc1i[:], ccs1[0:1, 0:8])
        nc.gpsimd.collective_compute(
            "AllGather", OP.bypass,
            replica_groups=[[0, 1], [2, 3], [4, 5], [6, 7]],
            ins=[cc1i[:]], outs=[cc1o[:]])
        nc.sync.dma_start(agb1[0:2, 0:8], cc1o[:])
        psb1 = psP.tile([P, 512], f32, tag="ps", name="psb1")
        nc.tensor.matmul(psb1[:, 0:2], onesf[0:2, 2:130],
                         agb1[0:2, 0:2], start=True, stop=True)
        nc.vector.tensor_copy(gla[:], psb1[:, 0:2])
        # r1 = 1/sqrt(var+eps); r1b = [r1, -m1*r1]
        n_inv = 1.0 / float(D * T)
        nc.vector.tensor_scalar(scr2[:, 0:2], gla[:, 0:2], n_inv, None, OP.mult)
        nc.vector.tensor_tensor(scr2[:, 2:3], scr2[:, 0:1], scr2[:, 0:1], OP.mult)
        nc.vector.tensor_tensor(scr2[:, 3:4], scr2[:, 1:2], scr2[:, 2:3],
                                OP.subtract)
        nc.vector.tensor_scalar(scr2[:, 3:4], scr2[:, 3:4], 1e-5, None, OP.add)
        nc.scalar.activation(scr2[:, 4:5], scr2[:, 3:4], AF.Sqrt)
        nc.vector.reciprocal(r1b[:, 0:1], scr2[:, 4:5])
        nc.vector.tensor_tensor(scr2[:, 5:6], scr2[:, 0:1], r1b[:, 0:1], OP.mult)
        nc.vector.tensor_scalar(r1b[:, 1:2], scr2[:, 5:6], -1.0, None, OP.mult)
        nc.vector.scalar_tensor_tensor(biasg[:, 0:8], cb("Wg", 0, 8),
                                       r1b[:, 1:2], cb("Wb", 0, 8),
                                       OP.mult, OP.add)

        # =========== Phase 7: GLU (apply gn1 scale/bias to raw pw1) ===========
        for m in range(4):
            nc.vector.tensor_scalar(a_sb[m][:], praw[m][:], r1b[:, 0:1],
                                    biasg[:, m:m + 1], OP.mult, OP.add)
            nc.scalar.activation(sg_sb[m][:], praw[4 + m][:], AF.Sigmoid,
                                 bias=biasg[:, 4 + m:5 + m], scale=r1b[:, 0:1])
        # shifted glu buffer: glu2[:, i] = glu(frame col own0 + i - 15)
        for m in range(4):
            nc.gpsimd.memset(glu2[m][:], 0.0)
            nc.vector.tensor_tensor(glu2[m][:, bass.ds(gdst_v, 527)],
                                    a_sb[m][:, bass.ds(gsrc_v, 527)],
                                    sg_sb[m][:, bass.ds(gsrc_v, 527)], OP.mult)

        # =========== Phase 8: depthwise conv (precomputed diag matmuls) ===========
        for m in range(4):
            dwA = dwp.tile([P, 2048], bf16, tag="dw", name=f"dwA{m}")
            nc.sync.dma_start(dwA[:], dwdiag_d.ap()[m][:, 0:2048])
            dwB = dwp.tile([P, 2048], bf16, tag="dw", name=f"dwB{m}")
            nc.sync.dma_start(dwB[:, 0:(KW - 16) * P],
                              dwdiag_d.ap()[m][:, 2048:KW * P])
            ps = psP.tile([P, 512], f32, tag="ps", name=f"psdc{m}")
            for k in range(KW):
                dg = (dwA[:, k * P:(k + 1) * P] if k < 16
                      else dwB[:, (k - 16) * P:(k - 15) * P])
                nc.tensor.matmul(ps[:], dg, glu2[m][:, k:k + 512],
                                 start=(k == 0), stop=(k == KW - 1))
            if m % 2 == 0:
                nc.scalar.activation(dcb[m][:], ps[:], AF.Copy)
            else:
                nc.vector.tensor_copy(dcb[m][:], ps[:])

        # =========== Phase 9: gn2 stats + pair AllGather ===========
        sqt2 = [scp.tile([P, OWN], bf16, tag="sqt", name=f"sqt2{m}")
                for m in range(4)]
        sc_t = scp.tile([P, 4], f32, tag="sct", name="sct")
        for m in range(4):
            nc.vector.tensor_reduce(sc_t[:, m:m + 1], dcb[m][:], AX.X, OP.add)
            nc.vector.tensor_tensor(sqt2[m][:], dcb[m][:], dcb[m][:], OP.mult)
            nc.vector.tensor_reduce(statsB[:, 4 + m:5 + m], sqt2[m][:],
                                    AX.X, OP.add)
        # sum_adj = sc + 512*dwb ; sq_adj = sq + 2*dwb*sc + 512*dwb^2
        nc.vector.scalar_tensor_tensor(statsB[:, 0:4], cb("dwb", 0, 4),
                                       512.0, sc_t[:, 0:4], OP.mult, OP.add)
        t1 = scp.tile([P, 4], f32, tag="t1", name="t1")
        nc.vector.tensor_tensor(t1[:], cb("dwb", 0, 4), sc_t[:, 0:4], OP.mult)
        t2 = scp.tile([P, 4], f32, tag="t2", name="t2")
        nc.vector.scalar_tensor_tensor(t2[:], t1[:], 2.0, statsB[:, 4:8],
                                       OP.mult, OP.add)
        nc.vector.tensor_tensor(t1[:], cb("dwb", 0, 4), cb("dwb", 0, 4), OP.mult)
        nc.vector.scalar_tensor_tensor(statsB[:, 4:8], t1[:], 512.0, t2[:],
                                       OP.mult, OP.add)
        nc.vector.tensor_reduce(stats2B[:, 0:1], statsB[:, 0:4], AX.X, OP.add)
        nc.vector.tensor_reduce(stats2B[:, 1:2], statsB[:, 4:8], AX.X, OP.add)
        pss2 = psP.tile([P, 512], f32, tag="ps", name="pss2")
        nc.tensor.matmul(pss2[0:1, 0:2], onesf[:, 0:1], stats2B[:, 0:2],
                         start=True, stop=True)
        nc.vector.tensor_copy(ccs2[0:1, 0:2], pss2[0:1, 0:2])
        cc2i = drp.tile([1, 8], f32, tag="cc2i", name="cc2i")
        cc2o = drp.tile([2, 8], f32, tag="cc2o", name="cc2o")
        nc.sync.dma_start(cc2i[:], ccs2[0:1, 0:8])
        nc.gpsimd.collective_compute(
            "AllGather", OP.bypass,
            replica_groups=[[0, 1], [2, 3], [4, 5], [6, 7]],
            ins=[cc2i[:]], outs=[cc2o[:]])
        nc.sync.dma_start(agb2[0:2, 0:8], cc2o[:])
        psb2 = psP.tile([P, 512], f32, tag="ps", name="psb2")
        nc.tensor.matmul(psb2[:, 0:2], onesf[0:2, 2:130],
                         agb2[0:2, 0:2], start=True, stop=True)
        nc.vector.tensor_copy(glb[:], psb2[:, 0:2])
        nc.vector.tensor_scalar(scr2[:, 0:2], glb[:, 0:2], n_inv, None, OP.mult)
        nc.vector.tensor_tensor(scr2[:, 2:3], scr2[:, 0:1], scr2[:, 0:1], OP.mult)
        nc.vector.tensor_tensor(scr2[:, 3:4], scr2[:, 1:2], scr2[:, 2:3],
                                OP.subtract)
        nc.vector.tensor_scalar(scr2[:, 3:4], scr2[:, 3:4], 1e-5, None, OP.add)
        nc.scalar.activation(scr2[:, 4:5], scr2[:, 3:4], AF.Sqrt)
        nc.vector.reciprocal(r2b[:, 0:1], scr2[:, 4:5])
        nc.vector.tensor_scalar(r2b[:, 1:2], scr2[:, 0:1], -1.0, None, OP.mult)
        # sact = r2*g2 ; bact = sact*(dwb - m2) + b2g
        nc.vector.tensor_scalar(sact[:, 0:4], cb("g2", 0, 4), r2b[:, 0:1],
                                None, OP.mult)
        nc.vector.tensor_scalar(t1[:], cb("dwb", 0, 4), r2b[:, 1:2],
                                None, OP.add)
        nc.vector.tensor_tensor(t2[:], t1[:], sact[:, 0:4], OP.mult)
        nc.vector.tensor_tensor(bact[:, 0:4], t2[:], cb("b2g", 0, 4), OP.add)
        # silu over own region
        for m in range(4):
            nc.scalar.activation(slown[m][:], dcb[m][:],
                                 AF.Silu, bias=bact[:, m:m + 1],
                                 scale=sact[:, m:m + 1])

        # =========== Phase 10: pw2 + residual -> c2 ===========
        pw2c = wpS.tile([P, 2048], bf16, tag="wS", name="pw2c")
        nc.sync.dma_start(pw2c[:].rearrange("p (k c) -> p k c", k=4),
                          pw2t_d.ap().rearrange("(k p) c -> p k c", k=4))
        for m in range(4):
            ps = psP.tile([P, 512], f32, tag="ps", name=f"psp2{m}")
            for k in range(4):
                nc.tensor.matmul(ps[:], pw2c[:, k * 512 + m * P:
                                             k * 512 + (m + 1) * P],
                                 slown[k][:], start=(k == 0), stop=(k == 3))
            nc.vector.scalar_tensor_tensor(
                c2f[m][:], ps[:], cb("bpw2", m),
                s2f[m][:, bass.ds(own0_v, OWN)], OP.add, OP.add)

        # =========== Phase 11: FFN2 over own region ===========
        w1b_t = []
        for k in range(4):
            w = wpA.tile([P, FF], bf16, tag="wA", name=f"w1b{k}")
            nc.sync.dma_start(w[:], w1b_d.ap()[k * P:(k + 1) * P, :])
            w1b_t.append(w)
        w2bsrc = w2b_d.ap().rearrange("(k p) c -> p k c", k=16)
        w2bc = [wpB.tile([P, 4096], bf16, tag="wB", name=f"w2bc{g}")
                for g in range(2)]
        for g in range(2):
            nc.sync.dma_start(
                w2bc[g][:].rearrange("p (k c) -> p k c", k=8),
                w2bsrc[:, 8 * g:8 * g + 8, :])
        h2s = []
        for m in range(16):
            ps = psP.tile([P, 512], f32, tag="ps", name=f"psf2{m}")
            for k in range(4):
                nc.tensor.matmul(ps[:], w1b_t[k][:, m * P:(m + 1) * P],
                                 c2f[k][:], start=(k == 0), stop=(k == 3))
            ht = hp.tile([P, 512], bf16, tag="h", name=f"h2_{m}")
            nc.scalar.activation(ht[:], ps[:], AF.Gelu, bias=cb("b1b", m))
            h2s.append(ht)
        for m in range(4):
            ps = psP.tile([P, 512], f32, tag="ps", name=f"psy{m}")
            for k in range(16):
                nc.tensor.matmul(
                    ps[:], w2bc[k // 8][:, (k % 8) * 512 + m * P:
                                       (k % 8) * 512 + (m + 1) * P],
                    h2s[k][:], start=(k == 0), stop=(k == 15))
            ysb = scp.tile([P, OWN], f32, tag="ysb", name=f"y{m}")
            nc.vector.scalar_tensor_tensor(ysb[:], ps[:], cb("b2b", m),
                                           c2f[m][:], OP.add, OP.add)
            nc.sync.dma_start(y_d[m * P:(m + 1) * P, :], ysb[:])

    nc.compile()
    return nc


def _host_prep(inputs):
    inp = {k: np.asarray(v) for k, v in inputs.items()}
    f32 = np.float32
    g1d = inp["rel_embed"][bucket1d(), :].astype(f32)   # [2047, H]

    tb = lambda a: np.ascontiguousarray(a, dtype=f32).astype(bfnp)
    shared = {
        "w1a": tb(inp["ff1_w1"]),
        "w2a": tb(inp["ff1_w2"] * 0.5),
        "wqkvo": tb(np.concatenate([inp["qkv_w"][:, :D] / 8.0,
                                    inp["qkv_w"][:, D:2 * D],
                                    inp["qkv_w"][:, 2 * D:],
                                    inp["out_w"]], axis=1)),
        "pw1g": tb(inp["pw1_w"].T * inp["gn1_g"][:, None]),
        "pw2t": tb(inp["pw2_w"].T),
        "w1b": tb(inp["ff2_w1"]),
        "w2b": tb(inp["ff2_w2"] * 0.5),
        "i128": np.eye(P, dtype=f32).astype(bfnp),
        "ones1": np.ones((1, P), f32).astype(bfnp),
        "onesf": np.ones((P, 130), f32),
        "bvrow": tb(inp["qkv_b"][2 * D:][None, :]),
    }
    gg = np.zeros((D, 16), f32)
    for h in range(H):
        gg[64 * h:64 * h + 64, h] = 8.0 * inp["gate_u"][h]
        gg[64 * h:64 * h + 64, 8 + h] = 8.0 * inp["gate_w"][h]
    shared["gg"] = gg.astype(bfnp)

    cbias = np.zeros((P, NCB), f32)

    def put(name, vec, n):
        v = np.asarray(vec, f32).reshape(n, P).T          # [128, n]
        cbias[:, _CB[name]:_CB[name] + n] = v

    put("b1a", inp["ff1_b1"], 16)
    put("b2a", inp["ff1_b2"] * 0.5, 4)
    put("bq", inp["qkv_b"][:D] / 8.0, 4)
    put("bk", inp["qkv_b"][D:2 * D], 4)
    put("bo", inp["out_b"], 4)
    pw1T = inp["pw1_w"].T * inp["gn1_g"][:, None]
    put("Wg", pw1T.sum(axis=0), 8)
    put("Wb", inp["pw1_w"] @ inp["gn1_b"] + inp["pw1_b"], 8)
    put("dwb", inp["dw_b"], 4)
    put("g2", inp["gn2_g"], 4)
    put("b2g", inp["gn2_b"], 4)
    put("bpw2", inp["pw2_b"], 4)
    put("b1b", inp["ff2_b1"], 16)
    put("b2b", inp["ff2_b2"] * 0.5, 4)
    cbias[:, _CB["sh"]:_CB["sh"] + 8] = np.asarray(inp["scale_h"], f32)[None, :]
    dw = np.asarray(inp["dw_w"][:, 0, :], f32)            # [D, KW]
    for m in range(4):
        cbias[:, _CB["dw"] + m * KW:_CB["dw"] + (m + 1) * KW] = \
            dw[m * P:(m + 1) * P, :]
    shared["cbias"] = cbias
    # precomputed depthwise-conv diag tiles: dwdiag[m, p, k*128+c] = d(p==c)*dw
    dwd = np.zeros((4, P, KW, P), f32)
    pp = np.arange(P)
    for m in range(4):
        dwd[m, pp, :, pp] = dw[m * P + pp, :]
    shared["dwdiag"] = dwd.reshape(4, P, KW * P).astype(bfnp)

    # per-parity Toeplitz expansion: dexp[h, j, r, s] = tab_p[1023+128j+r-s]
    # where tab_p[jj] = g1d[2046 - 480p - jj]  (toff = 480p)
    dexps = []
    for p in range(2):
        tab = np.zeros((H, TABW), f32)
        jj = np.arange(TABW)
        idx = 2046 - 480 * p - jj
        valid = (idx >= 0) & (idx < 2 * T - 1)
        tab[:, valid] = g1d[idx[valid]].T
        j5 = np.arange(NJ)[:, None, None]
        r_ = np.arange(P)[None, :, None]
        s_ = np.arange(1024)[None, None, :]
        eidx = 1023 + 128 * j5 + r_ - s_          # [5, 128, 1024] in [0, 1662]
        dexps.append(np.ascontiguousarray(tab[:, eidx]).astype(bfnp))
    in_maps = []
    for c in range(NCORES):
        b, p = c // 2, c % 2
        m = dict(shared)
        m["xb"] = np.ascontiguousarray(inp["x"][b], dtype=f32).astype(bfnp)
        m["dexp"] = dexps[p]
        m["toff"] = np.array([[480 * p]], np.uint32)
        m["own0"] = np.array([[32 * p]], np.uint32)
        m["gsrc"] = np.array([[17 * p]], np.uint32)
        m["gdst"] = np.array([[15 * (1 - p)]], np.uint32)
        in_maps.append(m)
    return in_maps


def get_program():
    if "nc" not in _CACHE:
        _CACHE["nc"] = _build_program()
    return _CACHE["nc"]


def run_cores(inputs, trace=False, **kw):
    from concourse import bass_utils
    nc = get_program()
    in_maps = _host_prep(inputs)
    return bass_utils.run_bass_kernel_spmd(
        nc, in_maps, core_ids=list(range(NCORES)), trace=trace, **kw)


def kernel(**inputs):
    res = run_cores(inputs, trace=False)
    out = np.zeros((B, D, T), np.float32)
    for c in range(NCORES):
        b, p = c // 2, c % 2
        out[b][:, 512 * p:512 * p + 512] = res.results[c]["y"]
    return out


if __name__ == "__main__":
    get_program()
    print("BUILD+COMPILE OK")


# revision 26
# speedup vs baseline: 1.3324x; 1.0488x over previous
"""Trainium2 Bass kernel for nn_ConformerBlock_50525995270849.

Takes FULL unsharded inputs (as produced by setup_inputs()) and returns the
FULL [B, D, T] fp32 output, running on 8 NeuronCores via run_bass_kernel_spmd.

Sharding: core c = (batch b=c//2, T-half parity p=c%2). Each core computes
FFN1 + K/V over full T, attention for its 544-column local query frame
(global cols [480p, 480p+544) = own 512 + 32-col conv halo), and the conv
module + FFN2 for its own 512 columns. GroupNorm(1,C) stats are pair-reduced
with a tiny AllGather; partition reduce / broadcast are ones-matmuls on PE.
The gated relative-position bias is added to the score PSUM with per-t-block
diag(f) matmuls against precomputed diagonal table expansions.

Perf structure (v3):
- batched 3D-AP startup DMAs; FFN1 starts ~10us in
- attention: scores/bias/PV per head; psv copied to SBUF by ACT immediately
  (frees the single psv PSUM slot); softmax normalization (DVE recip ->
  gpsimd partition_broadcast -> DVE mult) runs entirely off the PE stream
- one 3-buf [128,544] PSUM tag for everything + dedicated 1-buf psv tag
- gn stats squares on DVE (no ACT table switches on the critical path);
  pw1 raw->SBUF so its matmuls run during the gn1 AllGather
- depthwise conv computes own-512 columns only, from a shifted glu buffer
"""
import sys
import os

sys.path.insert(0, "/opt/trn_rl_repo")

import numpy as np
import ml_dtypes

B, T, D, H, DH = 4, 1024, 512, 8, 64
FF = 4 * D
KW = 31
NB, MAXD = 320, 800
PAD = KW // 2
NCORES = 8
P = 128
FRAME = 544          # local query frame width (own 512 + 32 halo)
OWN = 512            # own region width
TABW = 1664          # per-core bias table width
GW = OWN + 2 * PAD   # shifted glu buffer width (542)
NJ = 5               # frame col blocks: 4x128 + 1x32
JW4 = FRAME - 4 * P  # width of the last block (32)

bfnp = ml_dtypes.bfloat16
_CACHE = {}


def bucket1d():
    half, thr = NB // 2, NB // 4
    r = np.arange(-(T - 1), T, dtype=np.int32)
    sign = (r >= 0).astype(np.int32)
    ap = np.abs(r)
    log_ratio = np.log(np.maximum(ap, 1).astype(np.float32) / thr) / np.float32(
        np.log(MAXD / thr))
    log_pos = np.minimum(np.rint(thr + log_ratio * (half - thr)).astype(np.int32),
                         half - 1)
    return np.clip(np.where(ap < thr, ap, log_pos) + sign * half, 0, NB - 1)


# column layout of the packed per-partition bias tensor cbias [128, NCB]
_CB = {}
_ncb = 0
for _name, _n in [("b1a", 16), ("b2a", 4), ("bq", 4), ("bk", 4), ("bo", 4),
                  ("Wg", 8), ("Wb", 8), ("dwb", 4), ("g2", 4), ("b2g", 4),
                  ("bpw2", 4), ("b1b", 16), ("b2b", 4), ("sh", 8), ("dw", 31 * 4)]:
    _CB[_name] = _ncb
    _ncb += _n
NCB = _ncb


def _build_program():
    import concourse.bass as bass
    import concourse.tile as tile
    from concourse import bacc, mybir
    from contextlib import ExitStack

    f32 = mybir.dt.float32
    bf16 = mybir.dt.bfloat16
    AF = mybir.ActivationFunctionType
    OP = mybir.AluOpType
    AX = mybir.AxisListType

    nc = bacc.Bacc("TRN2", target_bir_lowering=False, debug=False,
                   num_devices=NCORES)

    di = lambda n, s, dt: nc.dram_tensor(n, s, dt, kind="ExternalInput")
    xb_d = di("xb", [D, T], bf16)
    w1a_d = di("w1a", [D, FF], bf16)
    w2a_d = di("w2a", [FF, D], bf16)
    wqkvo_d = di("wqkvo", [D, 4 * D], bf16)   # [wq | wk | wv | wo]
    pw1g_d = di("pw1g", [D, 2 * D], bf16)
    pw2t_d = di("pw2t", [D, D], bf16)
    w1b_d = di("w1b", [D, FF], bf16)
    w2b_d = di("w2b", [FF, D], bf16)
    gg_d = di("gg", [D, 16], bf16)
    i128_d = di("i128", [P, P], bf16)
    ones1_d = di("ones1", [1, P], bf16)
    onesf_d = di("onesf", [P, 130], f32)
    bvrow_d = di("bvrow", [1, D], bf16)
    cbias_d = di("cbias", [P, NCB], f32)
    dexp_d = di("dexp", [H, NJ, P, 1024], bf16)
    dwdiag_d = di("dwdiag", [4, P, KW * P], bf16)
    toff_d = di("toff", [1, 1], mybir.dt.uint32)
    own0_d = di("own0", [1, 1], mybir.dt.uint32)
    gsrc_d = di("gsrc", [1, 1], mybir.dt.uint32)
    gdst_d = di("gdst", [1, 1], mybir.dt.uint32)
    y_d = nc.dram_tensor("y", [D, OWN], f32, kind="ExternalOutput").ap()

    with tile.TileContext(nc) as tc, ExitStack() as ctx:
        pool = lambda name, bufs, **kw: ctx.enter_context(
            tc.tile_pool(name=name, bufs=bufs, **kw))
        cp = pool("const", 1)
        big = pool("big", 1)
        wpC = pool("wpC", 4)        # [128, 2048] bf16 w1a k-major chunk tiles
        wpA = pool("wpA", 4)        # [128, 2048] bf16 weight row-tiles
        wpB = pool("wpB", 2)        # [128, 4096] bf16 w2a/w2b k-major tiles
        wpS = pool("wpS", 1)        # [128, 2048] pw2 k-major tile
        hp = pool("hp", 16)         # FFN hidden tiles [128, 512] bf16
        Pp = pool("Pp", 9)          # attention probs tiles [128, 544] bf16
        dgp = pool("dgp", 11)       # diag tiles [128, 128] bf16
        dwp = pool("dwp", 2)        # conv diag chunk tiles [128, 2048] bf16
        dbp = pool("dbp", 2)        # per-head bias table tiles [128, 5120]
        scp = pool("scp", 2)        # small scratch
        rcp = pool("rcp", 2)        # per-head recip rows
        rbp = pool("rbp", 2)        # per-head recip broadcast [64, FRAME]
        osp = pool("osp", 2)        # per-head psv SBUF copies [65, FRAME]
        drp = pool("drp", 1, space="DRAM")
        psP = pool("psP", 3, space="PSUM")   # [128, 544] f32, 2 banks each
        psV = pool("psV", 1, space="PSUM")   # psv accumulator, 2 banks

        # ---- registers (per-engine) for the dynamic offsets ----
        tr_v = nc.vector.alloc_register("toff_v")
        nc.vector.reg_load(tr_v, toff_d[0:1, 0:1])
        toff_v = nc.vector.snap(tr_v, donate=True, min_val=0, max_val=480)
        ow_v = nc.vector.alloc_register("own0_v")
        nc.vector.reg_load(ow_v, own0_d[0:1, 0:1])
        own0_v = nc.vector.snap(ow_v, donate=True, min_val=0, max_val=32)
        ow_s = nc.scalar.alloc_register("own0_s")
        nc.scalar.reg_load(ow_s, own0_d[0:1, 0:1])
        own0_s = nc.scalar.snap(ow_s, donate=True, min_val=0, max_val=32)
        tr_t = nc.tensor.alloc_register("toff_t")
        nc.tensor.reg_load(tr_t, toff_d[0:1, 0:1])
        toff_t = nc.tensor.snap(tr_t, donate=True, min_val=0, max_val=480)
        gs_v = nc.vector.alloc_register("gsrc_v")
        nc.vector.reg_load(gs_v, gsrc_d[0:1, 0:1])
        gsrc_v = nc.vector.snap(gs_v, donate=True, min_val=0, max_val=17)
        gd_v = nc.vector.alloc_register("gdst_v")
        nc.vector.reg_load(gd_v, gdst_d[0:1, 0:1])
        gdst_v = nc.vector.snap(gd_v, donate=True, min_val=0, max_val=15)

        bt = lambda tg, w, dt=bf16: big.tile([P, w], dt, tag=tg, name=tg)

        # ---- startup-critical DMAs first: xb half 0 + w1a m-chunk 0 ----
        # xbh[t][p, k*512+c] = x[k*128+p, 512t+c]
        xsrc = xb_d.ap().rearrange("(k p) t -> p k t", k=4)
        xbh = [bt(f"xbh{t}", 2048) for t in range(2)]
        # w1g[g][p, k*512+mo] = w1a[k*128+p, 512g+mo]  (m-group g = m//4)
        wsrc = w1a_d.ap().rearrange("(k p) c -> p k c", k=4)
        w1g = [wpC.tile([P, 2048], bf16, tag="wC", name=f"w1g{g}")
               for g in range(4)]
        dma_xbh = lambda t: nc.sync.dma_start(
            xbh[t][:].rearrange("p (k c) -> p k c", k=4),
            xsrc[:, :, 512 * t:512 * t + 512])
        dma_w1g = lambda g: nc.sync.dma_start(
            w1g[g][:].rearrange("p (k c) -> p k c", k=4),
            wsrc[:, :, 512 * g:512 * g + 512])
        dma_xbh(0)
        dma_w1g(0)
        cbias = cp.tile([P, NCB], f32)
        nc.sync.dma_start(cbias[:], cbias_d.ap())
        dma_w1g(1)
        dma_xbh(1)
        dma_w1g(2)
        dma_w1g(3)
        # w2c[g][p, k*512+c] = w2a[(8g+k)*128+p, c]
        w2src = w2a_d.ap().rearrange("(k p) c -> p k c", k=16)
        w2c = [wpB.tile([P, 4096], bf16, tag="wB", name=f"w2c{g}")
               for g in range(2)]
        for g in range(2):
            nc.sync.dma_start(
                w2c[g][:].rearrange("p (k c) -> p k c", k=8),
                w2src[:, 8 * g:8 * g + 8, :])

        # ---- remaining constants ----
        i128 = cp.tile([P, P], bf16)
        nc.sync.dma_start(i128[:], i128_d.ap())
        ones1 = cp.tile([1, P], bf16)
        nc.sync.dma_start(ones1[:], ones1_d.ap())
        onesf = cp.tile([P, 130], f32)
        nc.sync.dma_start(onesf[:], onesf_d.ap())
        bvrow = cp.tile([1, D], bf16)
        nc.sync.dma_start(bvrow[:], bvrow_d.ap())
        ggc = cp.tile([P, 64], bf16)
        nc.sync.dma_start(ggc[:].rearrange("p (k c) -> p k c", k=4),
                          gg_d.ap().rearrange("(k p) c -> p k c", k=4))

        def cb(name, i, n=1):
            return cbias[:, _CB[name] + i:_CB[name] + i + n]

        s1b = [bt(f"s1b{k}", T) for k in range(4)]
        kkb = [bt(f"kkb{k}", T) for k in range(4)]
        vT = [bt(f"vT{t}", 520) for t in range(8)]
        qloc = [bt(f"qloc{k}", 640) for k in range(4)]
        fsb = [bt(f"fsb{t}", 8, f32) for t in range(NJ)]
        ohat = [bt(f"ohat{k}", FRAME) for k in range(4)]
        s2f = [bt(f"s2f{k}", FRAME) for k in range(4)]
        a_sb = [bt(f"a{m}", FRAME) for m in range(4)]
        sg_sb = [bt(f"sg{m}", FRAME) for m in range(4)]
        glu2 = [bt(f"glu2_{m}", GW) for m in range(4)]
        dcb = [bt(f"dcb{m}", OWN) for m in range(4)]
        slown = [bt(f"slown{m}", OWN) for m in range(4)]
        c2f = [bt(f"c2f{m}", OWN) for m in range(4)]
        stats = bt("stats", 8, f32)
        stats2 = bt("stats2", 2, f32)
        statsB = bt("statsB", 8, f32)
        stats2B = bt("stats2B", 2, f32)
        agb1 = bt("agb1", 8, f32)   # gn1 allgathered [2 part, 8]
        agb2 = bt("agb2", 8, f32)   # gn2 allgathered
        gla = bt("gla", 2, f32)     # gn1 reduced+broadcast [sum, sq]
        glb = bt("glb", 2, f32)     # gn2
        ccs1 = bt("ccs1", 8, f32)   # [1,8] staging for collective in
        ccs2 = bt("ccs2", 8, f32)
        nc.gpsimd.memset(ccs1[:], 0.0)
        nc.gpsimd.memset(ccs2[:], 0.0)
        r1b = bt("r1b", 2, f32)     # [r1, -m1*r1]
        r2b = bt("r2b", 2, f32)     # [r2, -m2]
        biasg = bt("biasg", 8, f32)
        sact = bt("sact", 4, f32)
        bact = bt("bact", 4, f32)
        scr2 = bt("scr2", 8, f32)   # scalar scratch columns
        # zero-pad qloc cols [544:640) so gate matmul block 4 stays finite
        for m in range(4):
            nc.gpsimd.memset(qloc[m][:, FRAME:640], 0.0)

        # =========== Phase 1: FFN1 over full T ===========
        for tch in range(2):
            hs = []
            for m in range(16):
                g, mo = m // 4, (m % 4) * P
                ps = psP.tile([P, 512], f32, tag="ps", name=f"ps1_{tch}_{m}")
                for k in range(4):
                    nc.tensor.matmul(ps[:], w1g[g][:, k * 512 + mo:k * 512 + mo + P],
                                     xbh[tch][:, k * 512:(k + 1) * 512],
                                     start=(k == 0), stop=(k == 3))
                ht = hp.tile([P, 512], bf16, tag="h", name=f"h_{tch}_{m}")
                nc.scalar.activation(ht[:], ps[:], AF.Gelu, bias=cb("b1a", m))
                hs.append(ht)
            c0 = tch * 512
            for m in range(4):
                ps = psP.tile([P, 512], f32, tag="ps", name=f"ps2_{tch}_{m}")
                for k in range(16):
                    nc.tensor.matmul(
                        ps[:], w2c[k // 8][:, (k % 8) * 512 + m * P:
                                           (k % 8) * 512 + (m + 1) * P],
                        hs[k][:], start=(k == 0), stop=(k == 15))
                nc.vector.scalar_tensor_tensor(
                    s1b[m][:, c0:c0 + 512], ps[:], cb("b2a", m),
                    xbh[tch][:, m * 512:(m + 1) * 512], OP.add, OP.add)

        # =========== Phase 2: QKV ===========
        wq_t, wk_t, wv_t, wo_t = [], [], [], []
        for k in range(4):
            w = wpA.tile([P, 4 * D], bf16, tag="wA", name=f"wqkvo{k}")
            nc.sync.dma_start(w[:], wqkvo_d.ap()[k * P:(k + 1) * P, :])
            wq_t.append(w[:, 0:D])
            wk_t.append(w[:, D:2 * D])
            wv_t.append(w[:, 2 * D:3 * D])
            wo_t.append(w[:, 3 * D:4 * D])
        # q directly on the local frame (dynamic-start rhs)
        for m in range(4):
            ps = psP.tile([P, FRAME], f32, tag="ps", name=f"psq{m}")
            for k in range(4):
                nc.tensor.matmul(ps[:, 0:512], wq_t[k][:, m * P:(m + 1) * P],
                                 s1b[k][:, bass.ds(toff_t, 512)],
                                 start=(k == 0), stop=(k == 3))
                nc.tensor.matmul(ps[:, 512:FRAME],
                                 wq_t[k][:, m * P:(m + 1) * P],
                                 s1b[k][:, bass.ds(toff_t + 512, JW4)],
                                 start=(k == 0), stop=(k == 3))
            nc.scalar.activation(qloc[m][:, 0:FRAME], ps[:], AF.Identity,
                                 bias=cb("bq", m))
        # k (full T, persistent)
        for m in range(4):
            for tch in range(2):
                c0 = tch * 512
                ps = psP.tile([P, 512], f32, tag="ps", name=f"psk{m}{tch}")
                for k in range(4):
                    nc.tensor.matmul(ps[:], wk_t[k][:, m * P:(m + 1) * P],
                                     s1b[k][:, c0:c0 + 512],
                                     start=(k == 0), stop=(k == 3))
                nc.scalar.activation(kkb[m][:, c0:c0 + 512], ps[:], AF.Identity,
                                     bias=cb("bk", m))
        # vT: [t, dv] with ones columns (65-block layout)
        for t in range(8):
            ps = psP.tile([P, 512], f32, tag="ps", name=f"psvp{t}")
            for k in range(4):
                nc.tensor.matmul(ps[:], s1b[k][:, t * P:(t + 1) * P],
                                 wv_t[k][:], start=(k == 0), stop=False)
            nc.tensor.matmul(ps[:], ones1[:, 0:P], bvrow[:],
                             start=False, stop=True)
            src3 = ps[:].rearrange("p (h c) -> p h c", c=64)
            dst3 = vT[t][:, 0:520].rearrange("p (h c) -> p h c", c=65)[:, :, 0:64]
            nc.scalar.activation(dst3, src3, AF.Copy)
            onescol = vT[t][:, 0:520].rearrange("p (h c) -> p h c", c=65)[:, :, 64:65]
            nc.gpsimd.memset(onescol, 1.0)

        # =========== Phase 3: gates -> f ===========
        for tt in range(NJ):
            ps = psP.tile([P, 512], f32, tag="ps", name=f"psg{tt}")
            for k in range(4):
                nc.tensor.matmul(ps[:, 0:16], qloc[k][:, tt * P:(tt + 1) * P],
                                 ggc[:, k * 16:(k + 1) * 16],
                                 start=(k == 0), stop=(k == 3))
            sgt = scp.tile([P, 16], f32, tag="sgt", name=f"sgt{tt}")
            nc.scalar.activation(sgt[:], ps[:, 0:16], AF.Sigmoid)
            gm = scp.tile([P, 8], f32, tag="gm", name=f"gm{tt}")
            nc.vector.tensor_tensor(gm[:], sgt[:, 0:8], sgt[:, 8:16], OP.mult)
            gd = scp.tile([P, 8], f32, tag="gd", name=f"gd{tt}")
            nc.vector.tensor_tensor(gd[:], sgt[:, 8:16], gm[:], OP.subtract)
            gs = scp.tile([P, 8], f32, tag="gs", name=f"gs{tt}")
            nc.vector.tensor_tensor(gs[:], gd[:], cb("sh", 0, 8), OP.mult)
            nc.vector.scalar_tensor_tensor(fsb[tt][:], gs[:], 1.0, sgt[:, 0:8],
                                           OP.add, OP.add)

        # pre-attention pair sync: absorbs accumulated core drift on the idle
        # gpsimd/cc stream so the gn1 AllGather later sees minimal skew
        cc0i = drp.tile([1, 8], f32, tag="cc0i", name="cc0i")
        cc0o = drp.tile([2, 8], f32, tag="cc0o", name="cc0o")
        scrap = bt("scrap", 8, f32)
        nc.sync.dma_start(cc0i[:], ccs1[0:1, 0:8])
        nc.gpsimd.collective_compute(
            "AllGather", OP.bypass,
            replica_groups=[[0, 1], [2, 3], [4, 5], [6, 7]],
            ins=[cc0i[:]], outs=[cc0o[:]])
        nc.sync.dma_start(scrap[0:2, 0:8], cc0o[:])

        # =========== Phase 4: attention per head ===========
        osbs = []
        for h in range(H):
            kt = h // 2
            pb = 64 * (h % 2)
            jw = lambda j: JW4 if j == 4 else P
            dgs = []
            dbt = dbp.tile([P, NJ * 1024], bf16, tag="db", name=f"db{h}")
            nc.sync.dma_start(
                dbt[:].rearrange("p (j c) -> p j c", j=NJ),
                dexp_d.ap()[h].rearrange("j p c -> p j c"))
            for j in range(NJ):
                dg = dgp.tile([P, P], bf16, tag="dg", name=f"dg{h}_{j}")
                nc.vector.tensor_scalar(dg[:, 0:jw(j)], i128[:, 0:jw(j)],
                                        fsb[j][:, h:h + 1], None, OP.mult)
                dgs.append(dg)
            psv = psV.tile([P, FRAME], f32, tag="psv", name=f"psav{h}")
            Pts = []
            for st in range(8):
                ps = psP.tile([P, FRAME], f32, tag="ps", name=f"pssc{h}_{st}")
                nc.tensor.matmul(ps[:, 0:512],
                                 kkb[kt][pb:pb + 64, st * P:(st + 1) * P],
                                 qloc[kt][pb:pb + 64, 0:512],
                                 start=True, stop=False)
                nc.tensor.matmul(ps[:, 512:FRAME],
                                 kkb[kt][pb:pb + 64, st * P:(st + 1) * P],
                                 qloc[kt][pb:pb + 64, 512:FRAME],
                                 start=True, stop=False)
                for j in range(NJ):
                    nc.tensor.matmul(ps[:, j * P:j * P + jw(j)],
                                     dbt[:, j * 1024 + st * P:
                                         j * 1024 + (st + 1) * P],
                                     dgs[j][:, 0:jw(j)],
                                     start=False, stop=(j == 4))
                Pt = Pp.tile([P, FRAME], bf16, tag="P", name=f"P{h}_{st}")
                nc.scalar.activation(Pt[:], ps[:], AF.Exp)
                Pts.append(Pt)
            for st in range(8):
                nc.tensor.matmul(psv[0:65, 0:512], vT[st][:, 65 * h:65 * h + 65],
                                 Pts[st][:, 0:512],
                                 start=(st == 0), stop=(st == 7))
                nc.tensor.matmul(psv[0:65, 512:FRAME],
                                 vT[st][:, 65 * h:65 * h + 65],
                                 Pts[st][:, 512:FRAME],
                                 start=(st == 0), stop=(st == 7))
            # free the psv slot fast: ACT copy to SBUF, normalize off-band
            osb = osp.tile([65, FRAME], bf16, tag="osb", name=f"osb{h}")
            nc.scalar.activation(osb[:], psv[0:65, 0:FRAME], AF.Copy)
            osbs.append(osb)
            rc = rcp.tile([1, FRAME], bf16, tag="rc", name=f"rc{h}")
            with nc.allow_low_precision(reason="softmax recip colsum, bf16 ok"):
                nc.vector.reciprocal(rc[:], osb[64:65, 0:FRAME])
            rcb = rbp.tile([64, FRAME], bf16, tag="rcb", name=f"rcb{h}")
            nc.gpsimd.partition_broadcast(rcb[:], rc[:], channels=64)
            nc.vector.tensor_tensor(ohat[kt][pb:pb + 64, :], osb[0:64, 0:FRAME],
                                    rcb[:], OP.mult)

        # =========== Phase 5: out-proj + residual -> s2 ===========
        for m in range(4):
            ps = psP.tile([P, FRAME], f32, tag="ps", name=f"pso{m}")
            for k in range(4):
                nc.tensor.matmul(ps[:, 0:512], wo_t[k][:, m * P:(m + 1) * P],
                                 ohat[k][:, 0:512], start=(k == 0), stop=(k == 3))
                nc.tensor.matmul(ps[:, 512:FRAME], wo_t[k][:, m * P:(m + 1) * P],
                                 ohat[k][:, 512:FRAME],
                                 start=(k == 0), stop=(k == 3))
            nc.vector.scalar_tensor_tensor(
                s2f[m][:], ps[:], cb("bo", m),
                s1b[m][:, bass.ds(toff_v, FRAME)], OP.add, OP.add)

        # =========== Phase 6a: gn1 stats (DVE) emitted first so the DVE
        # queue reaches them before the pw1 epilogue copies ===========
        sqt = [scp.tile([P, OWN], bf16, tag="sqt", name=f"sqt{m}")
               for m in range(4)]
        for m in range(4):
            nc.vector.tensor_reduce(stats[:, m:m + 1],
                                    s2f[m][:, bass.ds(own0_v, OWN)],
                                    AX.X, OP.add)
            nc.vector.tensor_tensor(sqt[m][:], s2f[m][:, bass.ds(own0_v, OWN)],
                                    s2f[m][:, bass.ds(own0_v, OWN)], OP.mult)
            nc.vector.tensor_reduce(stats[:, 4 + m:5 + m], sqt[m][:],
                                    AX.X, OP.add)
        nc.vector.tensor_reduce(stats2[:, 0:1], stats[:, 0:4], AX.X, OP.add)
        nc.vector.tensor_reduce(stats2[:, 1:2], stats[:, 4:8], AX.X, OP.add)

        # =========== Phase 6b: pw1 raw matmuls (cover the gn1 AllGather) ===========
        # pw1c[g] holds k = 2g, 2g+1 (k-major packed); DMA starts at FFN1 end
        pw1src = pw1g_d.ap().rearrange("(k p) c -> p k c", k=4)
        pw1c = [wpC.tile([P, 2048], bf16, tag="wC", name=f"pw1c{g}")
                for g in range(2)]
        for g in range(2):
            nc.sync.dma_start(
                pw1c[g][:].rearrange("p (k c) -> p k c", k=2),
                pw1src[:, 2 * g:2 * g + 2, :])
        praw = [Pp.tile([P, FRAME], bf16, tag="P", name=f"praw{m}")
                for m in range(8)]
        for m in range(8):
            ps = psP.tile([P, FRAME], f32, tag="ps", name=f"psp1{m}")
            for k in range(4):
                w = pw1c[k // 2][:, (k % 2) * 1024 + m * P:
                                 (k % 2) * 1024 + (m + 1) * P]
                nc.tensor.matmul(ps[:, 0:512], w,
                                 s2f[k][:, 0:512], start=(k == 0), stop=(k == 3))
                nc.tensor.matmul(ps[:, 512:FRAME], w,
                                 s2f[k][:, 512:FRAME],
                                 start=(k == 0), stop=(k == 3))
            nc.scalar.activation(praw[m][:], ps[:], AF.Copy)

        # =========== Phase 6c: gn1 pair AllGather ===========
        # partition-reduce via ones-matmul, tiny AllGather, sum+broadcast
        pss = psP.tile([P, 512], f32, tag="ps", name="pss1")
        nc.tensor.matmul(pss[0:1, 0:2], onesf[:, 0:1], stats2[:, 0:2],
                         start=True, stop=True)
        nc.vector.tensor_copy(ccs1[0:1, 0:2], pss[0:1, 0:2])
        cc1i = drp.tile([1, 8], f32, tag="cc1i", name="cc1i")
        cc1o = drp.tile([2, 8], f32, tag="cc1o", name="cc1o")
        nc.sync.dma_start(cc1i[:], ccs1[0:1, 0:8])
        nc.gpsimd.collective_compute(
            "AllGather", OP.bypass,
            replica_groups=[[0, 1], [2, 3], [4, 5], [6, 7]],
            ins=[cc1i[:]], outs=[cc1o[:]])
        nc.sync.dma_start(agb1[0:2, 0:8], cc1o[:])
        psb1 = psP.tile([P, 512], f32, tag="ps", name="psb1")
        nc.tensor.matmul(psb1[:, 0:2], onesf[0:2, 2:130],
                         agb1[0:2, 0:2], start=True, stop=True)
        nc.vector.tensor_copy(gla[:], psb1[:, 0:2])
        # r1 = 1/sqrt(var+eps); r1b = [r1, -m1*r1]
        n_inv = 1.0 / float(D * T)
        nc.vector.tensor_scalar(scr2[:, 0:2], gla[:, 0:2], n_inv, None, OP.mult)
        nc.vector.tensor_tensor(scr2[:, 2:3], scr2[:, 0:1], scr2[:, 0:1], OP.mult)
        nc.vector.tensor_tensor(scr2[:, 3:4], scr2[:, 1:2], scr2[:, 2:3],
                                OP.subtract)
        nc.vector.tensor_scalar(scr2[:, 3:4], scr2[:, 3:4], 1e-5, None, OP.add)
        nc.scalar.activation(scr2[:, 4:5], scr2[:, 3:4], AF.Sqrt)
        nc.vector.reciprocal(r1b[:, 0:1], scr2[:, 4:5])
        nc.vector.tensor_tensor(scr2[:, 5:6], scr2[:, 0:1], r1b[:, 0:1], OP.mult)
        nc.vector.tensor_scalar(r1b[:, 1:2], scr2[:, 5:6], -1.0, None, OP.mult)
        nc.vector.scalar_tensor_tensor(biasg[:, 0:8], cb("Wg", 0, 8),
                                       r1b[:, 1:2], cb("Wb", 0, 8),
                                       OP.mult, OP.add)

        # =========== Phase 7: GLU (apply gn1 scale/bias to raw pw1) ===========
        for m in range(4):
            nc.vector.tensor_scalar(a_sb[m][:], praw[m][:], r1b[:, 0:1],
                                    biasg[:, m:m + 1], OP.mult, OP.add)
            nc.scalar.activation(sg_sb[m][:], praw[4 + m][:], AF.Sigmoid,
                                 bias=biasg[:, 4 + m:5 + m], scale=r1b[:, 0:1])
        # shifted glu buffer: glu2[:, i] = glu(frame col own0 + i - 15)
        for m in range(4):
            nc.gpsimd.memset(glu2[m][:], 0.0)
            nc.vector.tensor_tensor(glu2[m][:, bass.ds(gdst_v, 527)],
                                    a_sb[m][:, bass.ds(gsrc_v, 527)],
                                    sg_sb[m][:, bass.ds(gsrc_v, 527)], OP.mult)

        # =========== Phase 8: depthwise conv (precomputed diag matmuls) ===========
        for m in range(4):
            dwA = dwp.tile([P, 2048], bf16, tag="dw", name=f"dwA{m}")
            nc.sync.dma_start(dwA[:], dwdiag_d.ap()[m][:, 0:2048])
            dwB = dwp.tile([P, 2048], bf16, tag="dw", name=f"dwB{m}")
            nc.sync.dma_start(dwB[:, 0:(KW - 16) * P],
                              dwdiag_d.ap()[m][:, 2048:KW * P])
            ps = psP.tile([P, 512], f32, tag="ps", name=f"psdc{m}")
            for k in range(KW):
                dg = (dwA[:, k * P:(k + 1) * P] if k < 16
                      else dwB[:, (k - 16) * P:(k - 15) * P])
                nc.tensor.matmul(ps[:], dg, glu2[m][:, k:k + 512],
                                 start=(k == 0), stop=(k == KW - 1))
            nc.scalar.activation(dcb[m][:], ps[:], AF.Copy)

        # =========== Phase 9: gn2 stats + pair AllGather ===========
        sqt2 = [scp.tile([P, OWN], bf16, tag="sqt", name=f"sqt2{m}")
                for m in range(4)]
        sc_t = scp.tile([P, 4], f32, tag="sct", name="sct")
        for m in range(4):
            nc.vector.tensor_reduce(sc_t[:, m:m + 1], dcb[m][:], AX.X, OP.add)
            nc.vector.tensor_tensor(sqt2[m][:], dcb[m][:], dcb[m][:], OP.mult)
            nc.vector.tensor_reduce(statsB[:, 4 + m:5 + m], sqt2[m][:],
                                    AX.X, OP.add)
        # sum_adj = sc + 512*dwb ; sq_adj = sq + 2*dwb*sc + 512*dwb^2
        nc.vector.scalar_tensor_tensor(statsB[:, 0:4], cb("dwb", 0, 4),
                                       512.0, sc_t[:, 0:4], OP.mult, OP.add)
        t1 = scp.tile([P, 4], f32, tag="t1", name="t1")
        nc.vector.tensor_tensor(t1[:], cb("dwb", 0, 4), sc_t[:, 0:4], OP.mult)
        t2 = scp.tile([P, 4], f32, tag="t2", name="t2")
        nc.vector.scalar_tensor_tensor(t2[:], t1[:], 2.0, statsB[:, 4:8],
                                       OP.mult, OP.add)
        nc.vector.tensor_tensor(t1[:], cb("dwb", 0, 4), cb("dwb", 0, 4), OP.mult)
        nc.vector.scalar_tensor_tensor(statsB[:, 4:8], t1[:], 512.0, t2[:],
                                       OP.mult, OP.add)
        nc.vector.tensor_reduce(stats2B[:, 0:1], statsB[:, 0:4], AX.X, OP.add)
        nc.vector.tensor_reduce(stats2B[:, 1:2], statsB[:, 4:8], AX.X, OP.add)
        pss2 = psP.tile([P, 512], f32, tag="ps", name="pss2")
        nc.tensor.matmul(pss2[0:1, 0:2], onesf[:, 0:1], stats2B[:, 0:2],
                         start=True, stop=True)
        nc.vector.tensor_copy(ccs2[0:1, 0:2], pss2[0:1, 0:2])
        cc2i = drp.tile([1, 8], f32, tag="cc2i", name="cc2i")
        cc2o = drp.tile([2, 8], f32, tag="cc2o", name="cc2o")
        nc.sync.dma_start(cc2i[:], ccs2[0:1, 0:8])
        nc.gpsimd.collective_compute(
            "AllGather", OP.bypass,
            replica_groups=[[0, 1], [2, 3], [4, 5], [6, 7]],
            ins=[cc2i[:]], outs=[cc2o[:]])
        nc.sync.dma_start(agb2[0:2, 0:8], cc2o[:])
        psb2 = psP.tile([P, 512], f32, tag="ps", name="psb2")
        nc.tensor.matmul(psb2[:, 0:2], onesf[0:2, 2:130],
                         agb2[0:2, 0:2], start=True, stop=True)
        nc.vector.tensor_copy(glb[:], psb2[:, 0:2])
        nc.vector.tensor_scalar(scr2[:, 0:2], glb[:, 0:2], n_inv, None, OP.mult)
        nc.vector.tensor_tensor(scr2[:, 2:3], scr2[:, 0:1], scr2[:, 0:1], OP.mult)
        nc.vector.tensor_tensor(scr2[:, 3:4], scr2[:, 1:2], scr2[:, 2:3],
                                OP.subtract)
        nc.vector.tensor_scalar(scr2[:, 3:4], scr2[:, 3:4], 1e-5, None, OP.add)
        nc.scalar.activation(scr2[:, 4:5], scr2[:, 3:4], AF.Sqrt)
        nc.vector.reciprocal(r2b[:, 0:1], scr2[:, 4:5])
        nc.vector.tensor_scalar(r2b[:, 1:2], scr2[:, 0:1], -1.0, None, OP.mult)
        # sact = r2*g2 ; bact = sact*(dwb - m2) + b2g
        nc.vector.tensor_scalar(sact[:, 0:4], cb("g2", 0, 4), r2b[:, 0:1],
                                None, OP.mult)
        nc.vector.tensor_scalar(t1[:], cb("dwb", 0, 4), r2b[:, 1:2],
                                None, OP.add)
        nc.vector.tensor_tensor(t2[:], t1[:], sact[:, 0:4], OP.mult)
        nc.vector.tensor_tensor(bact[:, 0:4], t2[:], cb("b2g", 0, 4), OP.add)
        # silu over own region
        for m in range(4):
            nc.scalar.activation(slown[m][:], dcb[m][:],
                                 AF.Silu, bias=bact[:, m:m + 1],
                                 scale=sact[:, m:m + 1])

        # =========== Phase 10: pw2 + residual -> c2 ===========
        pw2c = wpS.tile([P, 2048], bf16, tag="wS", name="pw2c")
        nc.sync.dma_start(pw2c[:].rearrange("p (k c) -> p k c", k=4),
                          pw2t_d.ap().rearrange("(k p) c -> p k c", k=4))
        for m in range(4):
            ps = psP.tile([P, 512], f32, tag="ps", name=f"psp2{m}")
            for k in range(4):
                nc.tensor.matmul(ps[:], pw2c[:, k * 512 + m * P:
                                             k * 512 + (m + 1) * P],
                                 slown[k][:], start=(k == 0), stop=(k == 3))
            nc.vector.scalar_tensor_tensor(
                c2f[m][:], ps[:], cb("bpw2", m),
                s2f[m][:, bass.ds(own0_v, OWN)], OP.add, OP.add)

        # =========== Phase 11: FFN2 over own region ===========
        w1b_t = []
        for k in range(4):
            w = wpA.tile([P, FF], bf16, tag="wA", name=f"w1b{k}")
            nc.sync.dma_start(w[:], w1b_d.ap()[k * P:(k + 1) * P, :])
            w1b_t.append(w)
        w2bsrc = w2b_d.ap().rearrange("(k p) c -> p k c", k=16)
        w2bc = [wpB.tile([P, 4096], bf16, tag="wB", name=f"w2bc{g}")
                for g in range(2)]
        for g in range(2):
            nc.sync.dma_start(
                w2bc[g][:].rearrange("p (k c) -> p k c", k=8),
                w2bsrc[:, 8 * g:8 * g + 8, :])
        h2s = []
        for m in range(16):
            ps = psP.tile([P, 512], f32, tag="ps", name=f"psf2{m}")
            for k in range(4):
                nc.tensor.matmul(ps[:], w1b_t[k][:, m * P:(m + 1) * P],
                                 c2f[k][:], start=(k == 0), stop=(k == 3))
            ht = hp.tile([P, 512], bf16, tag="h", name=f"h2_{m}")
            nc.scalar.activation(ht[:], ps[:], AF.Gelu, bias=cb("b1b", m))
            h2s.append(ht)
        for m in range(4):
            ps = psP.tile([P, 512], f32, tag="ps", name=f"psy{m}")
            for k in range(16):
                nc.tensor.matmul(
                    ps[:], w2bc[k // 8][:, (k % 8) * 512 + m * P:
                                       (k % 8) * 512 + (m + 1) * P],
                    h2s[k][:], start=(k == 0), stop=(k == 15))
            ysb = scp.tile([P, OWN], f32, tag="ysb", name=f"y{m}")
            nc.vector.scalar_tensor_tensor(ysb[:], ps[:], cb("b2b", m),
                                           c2f[m][:], OP.add, OP.add)
            nc.sync.dma_start(y_d[m * P:(m + 1) * P, :], ysb[:])

    nc.compile()
    return nc


def _host_prep(inputs):
    inp = {k: np.asarray(v) for k, v in inputs.items()}
    f32 = np.float32
    g1d = inp["rel_embed"][bucket1d(), :].astype(f32)   # [2047, H]

    tb = lambda a: np.ascontiguousarray(a, dtype=f32).astype(bfnp)
    shared = {
        "w1a": tb(inp["ff1_w1"]),
        "w2a": tb(inp["ff1_w2"] * 0.5),
        "wqkvo": tb(np.concatenate([inp["qkv_w"][:, :D] / 8.0,
                                    inp["qkv_w"][:, D:2 * D],
                                    inp["qkv_w"][:, 2 * D:],
                                    inp["out_w"]], axis=1)),
        "pw1g": tb(inp["pw1_w"].T * inp["gn1_g"][:, None]),
        "pw2t": tb(inp["pw2_w"].T),
        "w1b": tb(inp["ff2_w1"]),
        "w2b": tb(inp["ff2_w2"] * 0.5),
        "i128": np.eye(P, dtype=f32).astype(bfnp),
        "ones1": np.ones((1, P), f32).astype(bfnp),
        "onesf": np.ones((P, 130), f32),
        "bvrow": tb(inp["qkv_b"][2 * D:][None, :]),
    }
    gg = np.zeros((D, 16), f32)
    for h in range(H):
        gg[64 * h:64 * h + 64, h] = 8.0 * inp["gate_u"][h]
        gg[64 * h:64 * h + 64, 8 + h] = 8.0 * inp["gate_w"][h]
    shared["gg"] = gg.astype(bfnp)

    cbias = np.zeros((P, NCB), f32)

    def put(name, vec, n):
        v = np.asarray(vec, f32).reshape(n, P).T          # [128, n]
        cbias[:, _CB[name]:_CB[name] + n] = v

    put("b1a", inp["ff1_b1"], 16)
    put("b2a", inp["ff1_b2"] * 0.5, 4)
    put("bq", inp["qkv_b"][:D] / 8.0, 4)
    put("bk", inp["qkv_b"][D:2 * D], 4)
    put("bo", inp["out_b"], 4)
    pw1T = inp["pw1_w"].T * inp["gn1_g"][:, None]
    put("Wg", pw1T.sum(axis=0), 8)
    put("Wb", inp["pw1_w"] @ inp["gn1_b"] + inp["pw1_b"], 8)
    put("dwb", inp["dw_b"], 4)
    put("g2", inp["gn2_g"], 4)
    put("b2g", inp["gn2_b"], 4)
    put("bpw2", inp["pw2_b"], 4)
    put("b1b", inp["ff2_b1"], 16)
    put("b2b", inp["ff2_b2"] * 0.5, 4)
    cbias[:, _CB["sh"]:_CB["sh"] + 8] = np.asarray(inp["scale_h"], f32)[None, :]
    dw = np.asarray(inp["dw_w"][:, 0, :], f32)            # [D, KW]
    for m in range(4):
        cbias[:, _CB["dw"] + m * KW:_CB["dw"] + (m + 1) * KW] = \
            dw[m * P:(m + 1) * P, :]
    shared["cbias"] = cbias
    # precomputed depthwise-conv diag tiles: dwdiag[m, p, k*128+c] = d(p==c)*dw
    dwd = np.zeros((4, P, KW, P), f32)
    pp = np.arange(P)
    for m in range(4):
        dwd[m, pp, :, pp] = dw[m * P + pp, :]
    shared["dwdiag"] = dwd.reshape(4, P, KW * P).astype(bfnp)

    # per-parity Toeplitz expansion: dexp[h, j, r, s] = tab_p[1023+128j+r-s]
    # where tab_p[jj] = g1d[2046 - 480p - jj]  (toff = 480p)
    dexps = []
    for p in range(2):
        tab = np.zeros((H, TABW), f32)
        jj = np.arange(TABW)
        idx = 2046 - 480 * p - jj
        valid = (idx >= 0) & (idx < 2 * T - 1)
        tab[:, valid] = g1d[idx[valid]].T
        j5 = np.arange(NJ)[:, None, None]
        r_ = np.arange(P)[None, :, None]
        s_ = np.arange(1024)[None, None, :]
        eidx = 1023 + 128 * j5 + r_ - s_          # [5, 128, 1024] in [0, 1662]
        dexps.append(np.ascontiguousarray(tab[:, eidx]).astype(bfnp))
    in_maps = []
    for c in range(NCORES):
        b, p = c // 2, c % 2
        m = dict(shared)
        m["xb"] = np.ascontiguousarray(inp["x"][b], dtype=f32).astype(bfnp)
        m["dexp"] = dexps[p]
        m["toff"] = np.array([[480 * p]], np.uint32)
        m["own0"] = np.array([[32 * p]], np.uint32)
        m["gsrc"] = np.array([[17 * p]], np.uint32)
        m["gdst"] = np.array([[15 * (1 - p)]], np.uint32)
        in_maps.append(m)
    return in_maps


def get_program():
    if "nc" not in _CACHE:
        _CACHE["nc"] = _build_program()
    return _CACHE["nc"]


def run_cores(inputs, trace=False, **kw):
    from concourse import bass_utils
    nc = get_program()
    in_maps = _host_prep(inputs)
    return bass_utils.run_bass_kernel_spmd(
        nc, in_maps, core_ids=list(range(NCORES)), trace=trace, **kw)


def kernel(**inputs):
    res = run_cores(inputs, trace=False)
    out = np.zeros((B, D, T), np.float32)
    for c in range(NCORES):
        b, p = c // 2, c % 2
        out[b][:, 512 * p:512 * p + 512] = res.results[c]["y"]
    return out


if __name__ == "__main__":
    get_program()
    print("BUILD+COMPILE OK")
